# revision 27
# baseline (speedup 1.0000x reference)
"""Swin-style windowed-attention block on 8 TRN2 NeuronCores (data-parallel over batch).

v2: compact-pair layout (both windows' 49 valid tokens at rows 0:98 of a 128-row
pair tile, zero pad rows 98:128). Pair-wide attention with post-exp multiplicative
bias (exp(S+b) = exp(S)*exp(b) with a host-precomputed exp(bias) table that also
zeroes cross-window blocks and pad rows), no augmented-K matmuls, no head-realign
or v-shift DMAs. Activation-table thrash removed (DVE Newton rsqrt; chunk-pair
interleaving batches exp/gelu). Elementwise spread across ACT / DVE / Pool.
"""

import sys

sys.path.insert(0, "/opt/trn_rl_repo")

import numpy as np
import ml_dtypes

import concourse.bass as bass
import concourse.bacc as bacc
import concourse.tile as tile
import concourse.mybir as mybir
from concourse.bass_utils import run_bass_kernel_spmd

BF16 = ml_dtypes.bfloat16
FP8 = ml_dtypes.float8_e4m3
FP32 = mybir.dt.float32
BF16_DT = mybir.dt.bfloat16
FP8_DT = mybir.dt.float8e4
INT32 = mybir.dt.int32
W8SCALE = 64.0

# ---- static geometry ----
WH, WW = 7, 7
S = 49                     # valid tokens per window
C = 256                    # channels
NH = 8                     # heads
HD = 32                    # head dim
NWIN = 256                 # windows per batch image
B = 8                      # batch == number of cores
GRID = 16                  # 16x16 window grid
SCALE = HD ** -0.5
EPS = 1e-5
MASK_VAL = -30000.0

NPAIR = NWIN // 2          # 128 window pairs per core
PPC = 8                    # pairs per chunk
NCHUNK = NPAIR // PPC      # 16 chunks
TPP = 128                  # tile rows per pair (98 valid + 30 zero pad)
VPP = 2 * S                # 98 valid tokens per pair
TPC = PPC * TPP            # 1024 padded tokens per chunk
VPC = PPC * VPP            # 784 valid tokens per chunk
NTOK = NWIN * S            # 12544 valid tokens per core
NTOKP = NPAIR * TPP        # 16384 padded tokens per core

RSQRT_MAGIC = 0x5F3759DF

ActF = mybir.ActivationFunctionType
Alu = mybir.AluOpType


# --------------------------------------------------------------------------
# host-side preparation
# --------------------------------------------------------------------------

def _relative_position_index():
    ch, cw = np.arange(WH), np.arange(WW)
    coords = np.stack(np.meshgrid(ch, cw, indexing="ij")).reshape(2, -1)
    rel = coords[:, :, None] - coords[:, None, :]
    rel = rel.transpose(1, 2, 0).astype(np.int64)
    rel[..., 0] += WH - 1
    rel[..., 1] += WW - 1
    rel[..., 0] *= 2 * WW - 1
    return rel.sum(-1)                                    # (S, S)


def _window_mask_types():
    """Per-window mask type: 0 none, 1 bottom-row, 2 right-col, 3 corner."""
    h = w = GRID
    s1, s2 = WH - WH // 2, WW - WW // 2
    m = np.zeros((h, w, WH, WW, WH, WW), dtype=bool)
    m[-1, :, :s1, :, s1:, :] = True
    m[-1, :, s1:, :, :s1, :] = True
    m[:, -1, :, :s2, :, s2:] = True
    m[:, -1, :, s2:, :, :s2] = True
    m = m.reshape(h * w, S, S)
    types = np.zeros(NWIN, dtype=np.int64)
    rr, cc = np.divmod(np.arange(NWIN), GRID)
    types[(rr == GRID - 1) & (cc < GRID - 1)] = 1
    types[(rr < GRID - 1) & (cc == GRID - 1)] = 2
    types[(rr == GRID - 1) & (cc == GRID - 1)] = 3
    masks = np.zeros((4, S, S), dtype=np.float32)
    masks[1] = np.where(m[GRID * (GRID - 1)], MASK_VAL, 0.0)
    masks[2] = np.where(m[GRID - 1], MASK_VAL, 0.0)
    masks[3] = np.where(m[NWIN - 1], MASK_VAL, 0.0)
    return types, masks


def _pair_types():
    types, _ = _window_mask_types()
    combos = []
    ptype = np.zeros(NPAIR, dtype=np.int64)
    for j in range(NPAIR):
        c = (int(types[2 * j]), int(types[2 * j + 1]))
        if c not in combos:
            combos.append(c)
        ptype[j] = combos.index(c)
    assert len(combos) <= 4, combos
    while len(combos) < 4:
        combos.append((0, 0))
    return ptype, combos


_PTYPE, _PCOMBOS = _pair_types()


def _tile_kxoc(wT):
    """[K, OC] -> [128, K//128, OC] with K = 128*kt + p."""
    K, OC = wT.shape
    return np.ascontiguousarray(wT.reshape(K // 128, 128, OC).transpose(1, 0, 2))


def host_prep(inputs):
    x = np.asarray(inputs["x"], dtype=np.float32)          # (B, N, S, C)
    qkv_w = np.asarray(inputs["qkv_w"], dtype=np.float32)
    qkv_b = np.asarray(inputs["qkv_b"], dtype=np.float32)
    proj_w = np.asarray(inputs["proj_w"], dtype=np.float32)
    proj_b = np.asarray(inputs["proj_b"], dtype=np.float32)
    n1g = np.asarray(inputs["norm1_g"], dtype=np.float32)
    n1b = np.asarray(inputs["norm1_b"], dtype=np.float32)
    n2g = np.asarray(inputs["norm2_g"], dtype=np.float32)
    n2b = np.asarray(inputs["norm2_b"], dtype=np.float32)
    w1 = np.asarray(inputs["mlp_w1"], dtype=np.float32)
    b1 = np.asarray(inputs["mlp_b1"], dtype=np.float32)
    w2 = np.asarray(inputs["mlp_w2"], dtype=np.float32)
    b2 = np.asarray(inputs["mlp_b2"], dtype=np.float32)
    table = np.asarray(inputs["bias_table"], dtype=np.float32)

    # fold layernorm affine into the following matmuls
    qkv_w_f = qkv_w * n1g[None, :]
    qkv_b_f = qkv_b + qkv_w @ n1b
    w1_f = w1 * n2g[None, :]
    b1_f = b1 + w1 @ n2b

    wq = qkv_w_f[0:C] * SCALE
    bq = qkv_b_f[0:C] * SCALE
    wk = qkv_w_f[C:2 * C]
    bk = qkv_b_f[C:2 * C]
    wv = qkv_w_f[2 * C:3 * C]
    bv = qkv_b_f[2 * C:3 * C]

    common = {
        "wq": _tile_kxoc(wq.T).astype(BF16),
        "wk": _tile_kxoc(wk.T).astype(BF16),
        "wv": _tile_kxoc(wv.T).astype(BF16),
        "wp": _tile_kxoc(proj_w.T).astype(BF16),
        "w1": _tile_kxoc(w1_f.T * W8SCALE).astype(FP8),
        "w2": _tile_kxoc(w2.T * W8SCALE).astype(FP8),
    }

    # exp(bias + mask) multiplicative table: eb[t_row, ptype, h, q_row]
    # t_row/q_row = 49*r + local; zero on cross-window blocks and pad rows.
    rel = _relative_position_index()
    bias_sht = table[rel].transpose(2, 0, 1)               # [h, s, t]
    _, masks = _window_mask_types()                        # [4, s, t]
    eb = np.zeros((TPP, 4, NH, VPP), dtype=np.float32)
    for pt, (tA, tB) in enumerate(_PCOMBOS):
        for r, wt in ((0, tA), (1, tB)):
            blk = np.exp(bias_sht + masks[wt][None])       # [h, s, t]
            eb[S * r:S * r + S, pt, :, S * r:S * r + S] = blk.transpose(2, 0, 1)
    common["eb"] = eb.astype(BF16)

    extra = {
        "bq_nz": bool(np.any(bq != 0.0) or np.any(bk != 0.0)),
        "bv_nz": bool(np.any(bv != 0.0)),
        "pb_nz": bool(np.any(proj_b != 0.0)),
        "b1_nz": bool(np.any(b1_f != 0.0)),
        "b2_nz": bool(np.any(b2 != 0.0)),
    }
    if extra["bq_nz"]:
        common["bq"] = np.ascontiguousarray(bq.reshape(2, 128).T).astype(np.float32)
        common["bk"] = np.ascontiguousarray(bk.reshape(2, 128).T).astype(np.float32)
    if extra["b1_nz"]:
        common["b1"] = np.ascontiguousarray(b1_f.reshape(8, 128).T).astype(np.float32)
    if extra["bv_nz"]:
        common["bvbc"] = np.tile(bv[None, :], (128, 1)).astype(np.float32)
    if extra["pb_nz"]:
        common["pbbc"] = np.tile(proj_b[None, :], (128, 1)).astype(np.float32)
    if extra["b2_nz"]:
        common["b2bc"] = np.tile(b2[None, :], (128, 1)).astype(np.float32)

    in_maps = []
    for b in range(B):
        m = dict(common)
        xp = np.zeros((NPAIR, TPP, C), dtype=np.float32)
        xp[:, :VPP, :] = x[b].reshape(NPAIR, VPP, C)
        m["x"] = xp.reshape(NTOKP, C)
        in_maps.append(m)
    return in_maps, extra


# --------------------------------------------------------------------------
# kernel builder
# --------------------------------------------------------------------------

def build_program(n_pairs=NPAIR, flags=None):
    flags = flags or {}
    assert n_pairs % (2 * PPC) == 0
    n_chunks = n_pairs // PPC

    nc = bacc.Bacc("TRN2", target_bir_lowering=False, debug=False)

    ext = {}
    ext["x"] = nc.dram_tensor("x", [n_pairs * TPP, C], FP32, kind="ExternalInput")
    ext["out"] = nc.dram_tensor("out", [n_pairs * VPP, C], FP32, kind="ExternalOutput")
    ext["wq"] = nc.dram_tensor("wq", [128, 2, C], BF16_DT, kind="ExternalInput")
    ext["wk"] = nc.dram_tensor("wk", [128, 2, C], BF16_DT, kind="ExternalInput")
    ext["wv"] = nc.dram_tensor("wv", [128, 2, C], BF16_DT, kind="ExternalInput")
    ext["wp"] = nc.dram_tensor("wp", [128, 2, C], BF16_DT, kind="ExternalInput")
    ext["w1"] = nc.dram_tensor("w1", [128, 2, 4 * C], FP8_DT, kind="ExternalInput")
    ext["w2"] = nc.dram_tensor("w2", [128, 8, C], FP8_DT, kind="ExternalInput")
    ext["eb"] = nc.dram_tensor("eb", [TPP, 4, NH, VPP], BF16_DT, kind="ExternalInput")
    for name, shape, flg in (
        ("bq", [128, 2], "bq_nz"), ("bk", [128, 2], "bq_nz"),
        ("b1", [128, 8], "b1_nz"),
        ("bvbc", [128, C], "bv_nz"), ("pbbc", [128, C], "pb_nz"),
        ("b2bc", [128, C], "b2_nz"),
    ):
        ext[name] = (nc.dram_tensor(name, shape, FP32, kind="ExternalInput")
                     if flags.get(flg) else None)

    with tile.TileContext(nc) as tc:
        _body(tc, n_chunks, ext)

    nc.compile()
    return nc


def _body(tc, n_chunks, ext):
    nc = tc.nc
    import contextlib
    with contextlib.ExitStack() as ctx:
        const = ctx.enter_context(tc.tile_pool(name="const", bufs=1))
        cst = {}
        for name, shape, dt in (
            ("wq", [128, 2, C], BF16_DT), ("wk", [128, 2, C], BF16_DT),
            ("wv", [128, 2, C], BF16_DT), ("wp", [128, 2, C], BF16_DT),
            ("w1", [128, 2, 4 * C], FP8_DT), ("w2", [128, 8, C], FP8_DT),
            ("eb", [TPP, 4, NH, VPP], BF16_DT),
            ("bq", [128, 2], FP32), ("bk", [128, 2], FP32),
            ("b1", [128, 8], FP32),
            ("bvbc", [128, C], FP32), ("pbbc", [128, C], FP32),
            ("b2bc", [128, C], FP32),
        ):
            if ext.get(name) is None:
                cst[name] = None
                continue
            t = const.tile(shape, dt, tag=name, name=name)
            nc.sync.dma_start(out=t[:], in_=ext[name].ap())
            cst[name] = t
        ident = const.tile([128, 128], BF16_DT, tag="ident", name="ident")
        from concourse.masks import make_identity
        make_identity(nc, ident[:])
        cst["ident"] = ident

        pools = {}
        for name, bufs in (("xp", 4), ("xnp", 2), ("xnT", 2), ("attnT", 2),
                           ("xn2T", 2), ("qkp", 2), ("qk3p", 2), ("vsp", 2),
                           ("etp", 2), ("et2p", 2), ("atp", 2), ("x2p", 2),
                           ("hp", 2), ("statp", 2)):
            pools[name] = ctx.enter_context(tc.tile_pool(name=name, bufs=bufs))
        # PSUM budget (8 banks): ps_S 4 (one bank per concurrent PE row-tile),
        # ps_wide 2, ps_tok 2 (shared ring: v/psa/proj/mlp2/transpose drains)
        for name, bufs in (("ps_wide", 2), ("ps_tok", 2), ("ps_S", 1)):
            pools[name] = ctx.enter_context(
                tc.tile_pool(name=name, bufs=bufs, space="PSUM"))
        pools["ps_tr"] = pools["ps_tok"]
        pools["_ident"] = cst["ident"]

        import os
        n_phases = int(os.environ.get("PHASES", "4"))

        def _store_dbg(ci, t):
            dst = ext["out"][ci * VPC:, :]
            dst_ap = bass.AP(tensor=dst.tensor, offset=dst.offset,
                             ap=[[C, VPP], [VPP * C, PPC], [1, C]])
            nc.sync.dma_start(out=dst_ap, in_=t[0:VPP, :, :])

        st = {}
        for cp in range(n_chunks // 2):
            a, b = 2 * cp, 2 * cp + 1
            st[a] = _phase_in(tc, a, ext, cst, pools)
            st[b] = _phase_in(tc, b, ext, cst, pools)
            if n_phases < 2:
                _store_dbg(a, st[a]["x_t"])
                _store_dbg(b, st[b]["x_t"])
                del st[a], st[b]
                continue
            _phase_attn(tc, a, cst, pools, st[a])
            _phase_attn(tc, b, cst, pools, st[b])
            if n_phases < 3:
                _store_dbg(a, st[a]["x_t"])
                _store_dbg(b, st[b]["x_t"])
                del st[a], st[b]
                continue
            _phase_proj(tc, a, cst, pools, st[a])
            _phase_proj(tc, b, cst, pools, st[b])
            if n_phases < 4:
                _store_dbg(a, st[a]["x2_t"])
                _store_dbg(b, st[b]["x2_t"])
                del st[a], st[b]
                continue
            _phase_mlp(tc, a, ext, cst, pools, st[a])
            _phase_mlp(tc, b, ext, cst, pools, st[b])
            del st[a], st[b]


def _layernorm(nc, pools, x_t, xn_t):
    """x_t [128, PPC, 256] f32 -> xn_t bf16 ((x-mu)*rstd).

    Stats on DVE (bn_stats), rstd via quake-rsqrt + 2 Newton steps on DVE
    (avoids the Sqrt activation-table load), apply on Pool."""
    statp = pools["statp"]
    mv = statp.tile([128, PPC, 2], FP32, tag="mv", name="mv")
    for j in range(PPC):
        bnst = statp.tile([128, 6], FP32, tag="bnst", name="bnst")
        nc.vector.bn_stats(bnst[:], x_t[:, j, :])
        nc.vector.bn_aggr(mv[:, j, :], bnst[:])
    var = statp.tile([128, PPC], FP32, tag="var", name="var")
    rst = statp.tile([128, PPC], FP32, tag="rst", name="rst")
    tmp = statp.tile([128, PPC], FP32, tag="tmp", name="tmp")
    nc.vector.tensor_scalar(out=var[:], in0=mv[:, :, 1], scalar1=EPS,
                            scalar2=None, op0=Alu.add)
    nc.vector.tensor_scalar(out=rst[:].bitcast(INT32), in0=var[:].bitcast(INT32),
                            scalar1=1, scalar2=None, op0=Alu.logical_shift_right)
    nc.vector.tensor_scalar(out=rst[:].bitcast(INT32), in0=rst[:].bitcast(INT32),
                            scalar1=-1, scalar2=RSQRT_MAGIC,
                            op0=Alu.mult, op1=Alu.add)
    for _ in range(2):
        nc.vector.tensor_tensor(out=tmp[:], in0=rst[:], in1=rst[:], op=Alu.mult)
        nc.vector.tensor_tensor(out=tmp[:], in0=tmp[:], in1=var[:], op=Alu.mult)
        nc.vector.tensor_scalar(out=tmp[:], in0=tmp[:], scalar1=-0.5,
                                scalar2=1.5, op0=Alu.mult, op1=Alu.add)
        nc.vector.tensor_tensor(out=rst[:], in0=rst[:], in1=tmp[:], op=Alu.mult)
    for j in range(PPC):
        nc.gpsimd.tensor_scalar(
            out=xn_t[:, j, :], in0=x_t[:, j, :],
            scalar1=mv[:, j, 0:1], scalar2=rst[:, j:j + 1],
            op0=Alu.subtract, op1=Alu.mult,
        )


def _transposes(nc, pools, src_t, dst_T, drain_eng):
    """src_t [128, PPC, 256] -> dst_T [128, 2, PPC, 128] via PE + drain."""
    ident = pools["_ident"]
    for j in range(PPC):
        ps = pools["ps_tr"].tile([128, 2, 128], BF16_DT, tag="tok", name="trps")
        for ch in range(2):
            nc.tensor.transpose(ps[:, ch, :], src_t[:, j, 128 * ch:128 * (ch + 1)],
                                ident[:, :])
        if hasattr(drain_eng, "tensor_copy"):
            drain_eng.tensor_copy(out=dst_T[:, :, j, :], in_=ps[:])
        else:
            drain_eng.copy(out=dst_T[:, :, j, :], in_=ps[:])


def _phase_in(tc, ci, ext, cst, pools):
    """Load x, LN1, transpose, QKV -> qT/kT/vp."""
    nc = tc.nc
    s = {}

    x_t = pools["xp"].tile([128, PPC, C], FP32, tag="xo", name="x_t")
    nc.sync.dma_start(
        out=x_t[:],
        in_=ext["x"][ci * TPC:(ci + 1) * TPC, :].rearrange("(j p) c -> p j c", p=TPP),
    )
    s["x_t"] = x_t

    xn_t = pools["xnp"].tile([128, PPC, C], BF16_DT, tag="xn", name="xn_t")
    _layernorm(nc, pools, x_t, xn_t)
    xnT = pools["xnT"].tile([128, 2, PPC, 128], BF16_DT, tag="xnT", name="xnT")
    _transposes(nc, pools, xn_t, xnT, nc.vector)

    # q, k: channel-major slabs; drain on ACT (identity/copy, bias optional)
    qT = pools["qkp"].tile([128, 2, PPC, 128], BF16_DT, tag="qT", name="qT")
    kT = pools["qkp"].tile([128, 2, PPC, 128], BF16_DT, tag="kT", name="kT")
    for s2 in range(2):
        jsl = slice(4 * s2, 4 * s2 + 4)
        for (dstT, wname, bname) in ((qT, "wq", "bq"), (kT, "wk", "bk")):
            w_sb = cst[wname]
            for octl in range(2):
                ps = pools["ps_wide"].tile([128, 4, 128], FP32, tag="wide",
                                           name="qkps")
                for kt in range(2):
                    nc.tensor.matmul(
                        ps[:],
                        lhsT=w_sb[:, kt, 128 * octl:128 * (octl + 1)],
                        rhs=xnT[:, kt, jsl, :],
                        start=(kt == 0), stop=(kt == 1),
                    )
                if cst[bname] is not None:
                    nc.scalar.activation(
                        dstT[:, octl, jsl, :], ps[:], ActF.Identity,
                        bias=cst[bname][:, octl:octl + 1])
                else:
                    nc.scalar.activation(dstT[:, octl, jsl, :], ps[:], ActF.Copy)
    s["qT"], s["kT"] = qT, kT

    # PE matmul row-tile base 96 is unsupported; realign the hm==3 head rows
    # (h = 3, 7) of q/k to partition base 0 via one small DMA each.
    qk3 = pools["qk3p"].tile([32, 2, 2, PPC, 128], BF16_DT, tag="qk3", name="qk3")
    nc.sync.dma_start(out=qk3[0:32, 0], in_=qT[96:128, :, :, :])
    nc.sync.dma_start(out=qk3[0:32, 1], in_=kT[96:128, :, :, :])
    s["qk3"] = qk3

    # v: token-major, drain straight into pair-local vp (no base shift)
    vp = pools["vsp"].tile([128, PPC, NH, HD + 1], BF16_DT, tag="vp", name="vp")
    for j in range(PPC):
        ps = pools["ps_tok"].tile([128, C], FP32, tag="tok", name="vps")
        for kt in range(2):
            nc.tensor.matmul(
                ps[:], lhsT=xnT[:, kt, j, :], rhs=cst["wv"][:, kt, :],
                start=(kt == 0), stop=(kt == 1),
            )
        nc.vector.memset(vp[:, j, :, 0:1], 1.0)
        if cst["bvbc"] is not None:
            nc.vector.tensor_add(ps[:], ps[:], cst["bvbc"][:])
        nc.scalar.copy(
            out=vp[:, j, :, 1:HD + 1],
            in_=ps[:].rearrange("p (h d) -> p h d", h=NH),
        )
    s["vp"] = vp
    return s


def _phase_attn(tc, ci, cst, pools, s):
    """S = K^T Q pair-wide, exp, *exp(bias), A = et2 @ [1|v]."""
    nc = tc.nc
    qT, kT, vp = s["qT"], s["kT"], s["vp"]
    attn_t = pools["atp"].tile([128, PPC, C], BF16_DT, tag="attn", name="attn_t")
    if ci < 2:
        # first use of each ring buffer: seed pad rows (never valid-read,
        # but must be finite/initialized for the pair transposes). 96-aligned
        # partition start; rows 96:98 are re-written by the attn drain below.
        nc.vector.memset(attn_t[96:128, :, :], 0.0)
    qk3 = s["qk3"]
    for j in range(PPC):
        pt = int(_PTYPE[ci * PPC + j])
        et = pools["etp"].tile([128, NH, VPP], BF16_DT, tag="et", name="et")
        for hh in range(2):
            # h4-stride = 2KB: each concurrent PE row-tile owns its own bank
            pss = pools["ps_S"].tile([128, 4, 512], FP32, tag="S", name="pss")
            for h4 in range(4):
                h = 4 * hh + h4
                octl, hm = h // 4, h % 4
                if hm == 3:
                    lhsT = qk3[0:32, 1, octl, j, :]
                    rhs = qk3[0:32, 0, octl, j, 0:VPP]
                    base = 0
                else:
                    base = 32 * hm
                    lhsT = kT[base:base + 32, octl, j, :]
                    rhs = qT[base:base + 32, octl, j, 0:VPP]
                nc.tensor.matmul(
                    pss[:, h4, 0:VPP], lhsT=lhsT, rhs=rhs,
                    start=True, stop=True,
                    tile_position=(base, 0),
                )
            nc.scalar.activation(et[:, 4 * hh:4 * hh + 4, :], pss[:, :, 0:VPP],
                                 ActF.Exp)
        et2 = pools["et2p"].tile([128, NH, VPP], BF16_DT, tag="et2", name="et2")
        nc.vector.scalar_tensor_tensor(
            out=et2[:], in0=et[:], scalar=1.0,
            in1=cst["eb"][:, pt, :, :],
            op0=Alu.mult, op1=Alu.mult,
        )
        psa = pools["ps_tok"].tile([VPP, NH, HD + 1], FP32, tag="tok", name="psa")
        for h in range(NH):
            nc.tensor.matmul(
                psa[:, h, :], lhsT=et2[:, h, :], rhs=vp[:, j, h, :],
                start=True, stop=True,
            )
        rec = pools["statp"].tile([VPP, NH], FP32, tag="rec", name="rec")
        nc.vector.tensor_scalar_max(out=rec[:], in0=psa[:, :, 0], scalar1=1e-30)
        nc.vector.reciprocal(rec[:], rec[:])
        rec_b = bass.AP(tensor=rec[:].tensor, offset=rec[:].offset,
                        ap=list(rec[:].ap) + [[0, HD]])
        nc.vector.tensor_mul(
            attn_t[0:VPP, j, :].rearrange("p (h d) -> p h d", h=NH),
            psa[:, :, 1:HD + 1], rec_b,
        )
    s["attn_t"] = attn_t


def _phase_proj(tc, ci, cst, pools, s):
    """attn transpose, proj + resid1, LN2, transpose -> xn2T (fp8)."""
    nc = tc.nc
    attnT = pools["attnT"].tile([128, 2, PPC, 128], BF16_DT, tag="attnT",
                                name="attnT")
    _transposes(nc, pools, s["attn_t"], attnT, nc.vector)

    x2_t = pools["x2p"].tile([128, PPC, C], FP32, tag="x2", name="x2_t")
    for j in range(PPC):
        ps = pools["ps_tok"].tile([128, C], FP32, tag="tok", name="prps")
        for kt in range(2):
            nc.tensor.matmul(
                ps[:], lhsT=attnT[:, kt, j, :], rhs=cst["wp"][:, kt, :],
                start=(kt == 0), stop=(kt == 1),
            )
        if cst["pbbc"] is not None:
            nc.vector.tensor_add(ps[:], ps[:], cst["pbbc"][:])
        nc.vector.scalar_tensor_tensor(
            out=x2_t[:, j, :], in0=ps[:], scalar=1.0,
            in1=s["x_t"][:, j, :], op0=Alu.mult, op1=Alu.add,
        )
    s["x2_t"] = x2_t

    xn2_t = pools["xnp"].tile([128, PPC, C], BF16_DT, tag="xn", name="xn2_t")
    _layernorm(nc, pools, x2_t, xn2_t)
    xn2T = pools["xn2T"].tile([128, 2, PPC, 128], FP8_DT, tag="xn2T", name="xn2T")
    _transposes(nc, pools, xn2_t, xn2T, nc.scalar)
    s["xn2T"] = xn2T


def _phase_mlp(tc, ci, ext, cst, pools, s):
    """MLP (fp8 DoubleRow) + resid2, store."""
    nc = tc.nc
    xn2T = s["xn2T"]
    hT = pools["hp"].tile([128, 8, PPC, 128], FP8_DT, tag="hT", name="hT")
    if ci < 2:
        nc.vector.memset(hT[:, :, :, VPP:128], 0.0)
    for s2 in range(2):
        jsl = slice(4 * s2, 4 * s2 + 4)
        for m in range(8):
            ps = pools["ps_wide"].tile([128, 4, 128], FP32, tag="wide",
                                       name="m1ps")
            nc.tensor.matmul(
                ps[:], lhsT=cst["w1"][:, :, 128 * m:128 * (m + 1)],
                rhs=xn2T[:, :, jsl, :],
                start=True, stop=True,
                perf_mode=mybir.MatmulPerfMode.DoubleRow,
            )
            bias = (cst["b1"][:, m:m + 1] if cst["b1"] is not None else 0.0)
            import os
            gelu_f = ActF.Identity if os.environ.get("GELU_ID") else ActF.Gelu
            nc.scalar.activation(
                hT[:, m, jsl, 0:VPP], ps[:, :, 0:VPP],
                gelu_f, bias=bias, scale=1.0 / W8SCALE,
            )

    out_t = pools["xp"].tile([128, PPC, C], FP32, tag="xo", name="out_t")
    for j in range(PPC):
        ps = pools["ps_tok"].tile([128, C], FP32, tag="tok", name="m2ps")
        for k2 in range(4):
            nc.tensor.matmul(
                ps[:], lhsT=hT[:, 2 * k2:2 * k2 + 2, j, :],
                rhs=cst["w2"][:, 2 * k2:2 * k2 + 2, :],
                start=(k2 == 0), stop=(k2 == 3),
                perf_mode=mybir.MatmulPerfMode.DoubleRow,
            )
        if cst["b2bc"] is not None:
            nc.vector.tensor_add(ps[:], ps[:], cst["b2bc"][:])
        nc.vector.scalar_tensor_tensor(
            out=out_t[:, j, :], in0=ps[:], scalar=1.0 / W8SCALE,
            in1=s["x2_t"][:, j, :], op0=Alu.mult, op1=Alu.add,
        )

    # compact store: pair (ci*PPC + j) valid rows 0:VPP
    dst = ext["out"][ci * VPC:, :]
    dst_ap = bass.AP(
        tensor=dst.tensor, offset=dst.offset,
        ap=[[C, VPP], [VPP * C, PPC], [1, C]],
    )
    nc.sync.dma_start(out=dst_ap, in_=out_t[0:VPP, :, :])


# --------------------------------------------------------------------------
# entry point
# --------------------------------------------------------------------------

_CACHE = {}


def _get_program(key_flags):
    if key_flags not in _CACHE:
        _CACHE[key_flags] = build_program(NPAIR, flags=dict(key_flags))
    return _CACHE[key_flags]


def kernel(**inputs):
    in_maps, extra = host_prep(inputs)
    nc = _get_program(tuple(sorted(extra.items())))
    res = run_bass_kernel_spmd(nc, in_maps, core_ids=list(range(B)))
    out = np.stack([res.results[i]["out"] for i in range(B)], axis=0)
    return out.reshape(B, NWIN, S, C).astype(np.float32)


# revision 29
# speedup vs baseline: 1.8582x; 1.8582x over previous
"""Swin-style windowed-attention block on 8 TRN2 NeuronCores (data-parallel over batch).

v2: compact-pair layout (both windows' 49 valid tokens at rows 0:98 of a 128-row
pair tile, zero pad rows 98:128). Pair-wide attention with post-exp multiplicative
bias (exp(S+b) = exp(S)*exp(b) with a host-precomputed exp(bias) table that also
zeroes cross-window blocks and pad rows), no augmented-K matmuls, no head-realign
or v-shift DMAs. Activation-table thrash removed (DVE Newton rsqrt; chunk-pair
interleaving batches exp/gelu). Elementwise spread across ACT / DVE / Pool.
"""

import sys

sys.path.insert(0, "/opt/trn_rl_repo")

import numpy as np
import ml_dtypes

import concourse.bass as bass
import concourse.bacc as bacc
import concourse.tile as tile
import concourse.mybir as mybir
from concourse.bass_utils import run_bass_kernel_spmd

BF16 = ml_dtypes.bfloat16
FP8 = ml_dtypes.float8_e4m3
FP32 = mybir.dt.float32
BF16_DT = mybir.dt.bfloat16
FP8_DT = mybir.dt.float8e4
INT32 = mybir.dt.int32
W8SCALE = 64.0

# ---- static geometry ----
WH, WW = 7, 7
S = 49                     # valid tokens per window
C = 256                    # channels
NH = 8                     # heads
HD = 32                    # head dim
NWIN = 256                 # windows per batch image
B = 8                      # batch == number of cores
GRID = 16                  # 16x16 window grid
SCALE = HD ** -0.5
EPS = 1e-5
MASK_VAL = -30000.0

NPAIR = NWIN // 2          # 128 window pairs per core
PPC = 8                    # pairs per chunk
NCHUNK = NPAIR // PPC      # 16 chunks
TPP = 128                  # tile rows per pair (98 valid + 30 zero pad)
VPP = 2 * S                # 98 valid tokens per pair
TPC = PPC * TPP            # 1024 padded tokens per chunk
VPC = PPC * VPP            # 784 valid tokens per chunk
NTOK = NWIN * S            # 12544 valid tokens per core
NTOKP = NPAIR * TPP        # 16384 padded tokens per core

RSQRT_MAGIC = 0x5F3759DF

ActF = mybir.ActivationFunctionType
Alu = mybir.AluOpType


# --------------------------------------------------------------------------
# host-side preparation
# --------------------------------------------------------------------------

def _relative_position_index():
    ch, cw = np.arange(WH), np.arange(WW)
    coords = np.stack(np.meshgrid(ch, cw, indexing="ij")).reshape(2, -1)
    rel = coords[:, :, None] - coords[:, None, :]
    rel = rel.transpose(1, 2, 0).astype(np.int64)
    rel[..., 0] += WH - 1
    rel[..., 1] += WW - 1
    rel[..., 0] *= 2 * WW - 1
    return rel.sum(-1)                                    # (S, S)


def _window_mask_types():
    """Per-window mask type: 0 none, 1 bottom-row, 2 right-col, 3 corner."""
    h = w = GRID
    s1, s2 = WH - WH // 2, WW - WW // 2
    m = np.zeros((h, w, WH, WW, WH, WW), dtype=bool)
    m[-1, :, :s1, :, s1:, :] = True
    m[-1, :, s1:, :, :s1, :] = True
    m[:, -1, :, :s2, :, s2:] = True
    m[:, -1, :, s2:, :, :s2] = True
    m = m.reshape(h * w, S, S)
    types = np.zeros(NWIN, dtype=np.int64)
    rr, cc = np.divmod(np.arange(NWIN), GRID)
    types[(rr == GRID - 1) & (cc < GRID - 1)] = 1
    types[(rr < GRID - 1) & (cc == GRID - 1)] = 2
    types[(rr == GRID - 1) & (cc == GRID - 1)] = 3
    masks = np.zeros((4, S, S), dtype=np.float32)
    masks[1] = np.where(m[GRID * (GRID - 1)], MASK_VAL, 0.0)
    masks[2] = np.where(m[GRID - 1], MASK_VAL, 0.0)
    masks[3] = np.where(m[NWIN - 1], MASK_VAL, 0.0)
    return types, masks


def _pair_types():
    types, _ = _window_mask_types()
    combos = []
    ptype = np.zeros(NPAIR, dtype=np.int64)
    for j in range(NPAIR):
        c = (int(types[2 * j]), int(types[2 * j + 1]))
        if c not in combos:
            combos.append(c)
        ptype[j] = combos.index(c)
    assert len(combos) <= 4, combos
    while len(combos) < 4:
        combos.append((0, 0))
    return ptype, combos


_PTYPE, _PCOMBOS = _pair_types()


def _tile_kxoc(wT):
    """[K, OC] -> [128, K//128, OC] with K = 128*kt + p."""
    K, OC = wT.shape
    return np.ascontiguousarray(wT.reshape(K // 128, 128, OC).transpose(1, 0, 2))


def host_prep(inputs):
    x = np.asarray(inputs["x"], dtype=np.float32)          # (B, N, S, C)
    qkv_w = np.asarray(inputs["qkv_w"], dtype=np.float32)
    qkv_b = np.asarray(inputs["qkv_b"], dtype=np.float32)
    proj_w = np.asarray(inputs["proj_w"], dtype=np.float32)
    proj_b = np.asarray(inputs["proj_b"], dtype=np.float32)
    n1g = np.asarray(inputs["norm1_g"], dtype=np.float32)
    n1b = np.asarray(inputs["norm1_b"], dtype=np.float32)
    n2g = np.asarray(inputs["norm2_g"], dtype=np.float32)
    n2b = np.asarray(inputs["norm2_b"], dtype=np.float32)
    w1 = np.asarray(inputs["mlp_w1"], dtype=np.float32)
    b1 = np.asarray(inputs["mlp_b1"], dtype=np.float32)
    w2 = np.asarray(inputs["mlp_w2"], dtype=np.float32)
    b2 = np.asarray(inputs["mlp_b2"], dtype=np.float32)
    table = np.asarray(inputs["bias_table"], dtype=np.float32)

    # fold layernorm affine into the following matmuls
    qkv_w_f = qkv_w * n1g[None, :]
    qkv_b_f = qkv_b + qkv_w @ n1b
    w1_f = w1 * n2g[None, :]
    b1_f = b1 + w1 @ n2b

    wq = qkv_w_f[0:C] * SCALE
    bq = qkv_b_f[0:C] * SCALE
    wk = qkv_w_f[C:2 * C]
    bk = qkv_b_f[C:2 * C]
    wv = qkv_w_f[2 * C:3 * C]
    bv = qkv_b_f[2 * C:3 * C]

    common = {
        "wq": _tile_kxoc(wq.T).astype(BF16),
        "wk": _tile_kxoc(wk.T).astype(BF16),
        "wv": _tile_kxoc(wv.T).astype(BF16),
        "wp": _tile_kxoc(proj_w.T).astype(BF16),
        "w1": _tile_kxoc(w1_f.T * W8SCALE).astype(FP8),
        "w2": _tile_kxoc(w2.T * W8SCALE).astype(FP8),
    }

    # exp(bias + mask) multiplicative table: eb[t_row, ptype, h, q_row]
    # t_row/q_row = 49*r + local; zero on cross-window blocks and pad rows.
    rel = _relative_position_index()
    bias_sht = table[rel].transpose(2, 0, 1)               # [h, s, t]
    _, masks = _window_mask_types()                        # [4, s, t]
    eb = np.zeros((TPP, 4, NH, VPP), dtype=np.float32)
    for pt, (tA, tB) in enumerate(_PCOMBOS):
        for r, wt in ((0, tA), (1, tB)):
            blk = np.exp(bias_sht + masks[wt][None])       # [h, s, t]
            eb[S * r:S * r + S, pt, :, S * r:S * r + S] = blk.transpose(2, 0, 1)
    common["eb"] = eb.astype(BF16)

    extra = {
        "bq_nz": bool(np.any(bq != 0.0) or np.any(bk != 0.0)),
        "bv_nz": bool(np.any(bv != 0.0)),
        "pb_nz": bool(np.any(proj_b != 0.0)),
        "b1_nz": bool(np.any(b1_f != 0.0)),
        "b2_nz": bool(np.any(b2 != 0.0)),
    }
    if extra["bq_nz"]:
        common["bq"] = np.ascontiguousarray(bq.reshape(2, 128).T).astype(np.float32)
        common["bk"] = np.ascontiguousarray(bk.reshape(2, 128).T).astype(np.float32)
    if extra["b1_nz"]:
        common["b1"] = np.ascontiguousarray(b1_f.reshape(8, 128).T).astype(np.float32)
    if extra["bv_nz"]:
        common["bvbc"] = np.tile(bv[None, :], (128, 1)).astype(np.float32)
    if extra["pb_nz"]:
        common["pbbc"] = np.tile(proj_b[None, :], (128, 1)).astype(np.float32)
    if extra["b2_nz"]:
        common["b2bc"] = np.tile(b2[None, :], (128, 1)).astype(np.float32)

    in_maps = []
    for b in range(B):
        m = dict(common)
        xp = np.zeros((NPAIR, TPP, C), dtype=np.float32)
        xp[:, :VPP, :] = x[b].reshape(NPAIR, VPP, C)
        m["x"] = xp.reshape(NTOKP, C)
        in_maps.append(m)
    return in_maps, extra


# --------------------------------------------------------------------------
# kernel builder
# --------------------------------------------------------------------------

def build_program(n_pairs=NPAIR, flags=None):
    flags = flags or {}
    assert n_pairs % (2 * PPC) == 0
    n_chunks = n_pairs // PPC

    nc = bacc.Bacc("TRN2", target_bir_lowering=False, debug=False)

    ext = {}
    ext["x"] = nc.dram_tensor("x", [n_pairs * TPP, C], FP32, kind="ExternalInput")
    ext["out"] = nc.dram_tensor("out", [n_pairs * VPP, C], FP32, kind="ExternalOutput")
    ext["wq"] = nc.dram_tensor("wq", [128, 2, C], BF16_DT, kind="ExternalInput")
    ext["wk"] = nc.dram_tensor("wk", [128, 2, C], BF16_DT, kind="ExternalInput")
    ext["wv"] = nc.dram_tensor("wv", [128, 2, C], BF16_DT, kind="ExternalInput")
    ext["wp"] = nc.dram_tensor("wp", [128, 2, C], BF16_DT, kind="ExternalInput")
    ext["w1"] = nc.dram_tensor("w1", [128, 2, 4 * C], FP8_DT, kind="ExternalInput")
    ext["w2"] = nc.dram_tensor("w2", [128, 8, C], FP8_DT, kind="ExternalInput")
    ext["eb"] = nc.dram_tensor("eb", [TPP, 4, NH, VPP], BF16_DT, kind="ExternalInput")
    for name, shape, flg in (
        ("bq", [128, 2], "bq_nz"), ("bk", [128, 2], "bq_nz"),
        ("b1", [128, 8], "b1_nz"),
        ("bvbc", [128, C], "bv_nz"), ("pbbc", [128, C], "pb_nz"),
        ("b2bc", [128, C], "b2_nz"),
    ):
        ext[name] = (nc.dram_tensor(name, shape, FP32, kind="ExternalInput")
                     if flags.get(flg) else None)

    with tile.TileContext(nc) as tc:
        _body(tc, n_chunks, ext)

    nc.compile()
    return nc


def _body(tc, n_chunks, ext):
    nc = tc.nc
    import contextlib
    with contextlib.ExitStack() as ctx:
        const = ctx.enter_context(tc.tile_pool(name="const", bufs=1))
        cst = {}
        for name, shape, dt in (
            ("wq", [128, 2, C], BF16_DT), ("wk", [128, 2, C], BF16_DT),
            ("wv", [128, 2, C], BF16_DT), ("wp", [128, 2, C], BF16_DT),
            ("w1", [128, 2, 4 * C], FP8_DT), ("w2", [128, 8, C], FP8_DT),
            ("eb", [TPP, 4, NH, VPP], BF16_DT),
            ("bq", [128, 2], FP32), ("bk", [128, 2], FP32),
            ("b1", [128, 8], FP32),
            ("bvbc", [128, C], FP32), ("pbbc", [128, C], FP32),
            ("b2bc", [128, C], FP32),
        ):
            if ext.get(name) is None:
                cst[name] = None
                continue
            t = const.tile(shape, dt, tag=name, name=name)
            nc.sync.dma_start(out=t[:], in_=ext[name].ap())
            cst[name] = t
        ident = const.tile([128, 128], BF16_DT, tag="ident", name="ident")
        from concourse.masks import make_identity
        make_identity(nc, ident[:])
        cst["ident"] = ident

        pools = {}
        for name, bufs in (("xp", 4), ("xnp", 2), ("xnT", 2), ("attnT", 2),
                           ("xn2T", 2), ("qkp", 2), ("qk3p", 2), ("vsp", 2),
                           ("etp", 2), ("et2p", 2), ("atp", 2), ("x2p", 2),
                           ("hp", 2), ("statp", 2)):
            pools[name] = ctx.enter_context(tc.tile_pool(name=name, bufs=bufs))
        # PSUM budget (8 banks): ps_S 4 (one bank per concurrent PE row-tile),
        # ps_wide 2, ps_tok 2 (shared ring: v/psa/proj/mlp2/transpose drains)
        for name, bufs in (("ps_wide", 2), ("ps_tok", 2), ("ps_S", 1)):
            pools[name] = ctx.enter_context(
                tc.tile_pool(name=name, bufs=bufs, space="PSUM"))
        pools["ps_tr"] = pools["ps_tok"]
        pools["_ident"] = cst["ident"]

        import os
        n_phases = int(os.environ.get("PHASES", "4"))

        def _store_dbg(ci, t):
            dst = ext["out"][ci * VPC:, :]
            dst_ap = bass.AP(tensor=dst.tensor, offset=dst.offset,
                             ap=[[C, VPP], [VPP * C, PPC], [1, C]])
            nc.sync.dma_start(out=dst_ap, in_=t[0:VPP, :, :])

        st = {}
        for cp in range(n_chunks // 2):
            a, b = 2 * cp, 2 * cp + 1
            st[a] = _phase_in(tc, a, ext, cst, pools)
            st[b] = _phase_in(tc, b, ext, cst, pools)
            if n_phases < 2:
                _store_dbg(a, st[a]["x_t"])
                _store_dbg(b, st[b]["x_t"])
                del st[a], st[b]
                continue
            _phase_attn(tc, a, cst, pools, st[a])
            _phase_attn(tc, b, cst, pools, st[b])
            if n_phases < 3:
                _store_dbg(a, st[a]["x_t"])
                _store_dbg(b, st[b]["x_t"])
                del st[a], st[b]
                continue
            _phase_proj(tc, a, cst, pools, st[a])
            _phase_proj(tc, b, cst, pools, st[b])
            if n_phases < 4:
                _store_dbg(a, st[a]["x2_t"])
                _store_dbg(b, st[b]["x2_t"])
                del st[a], st[b]
                continue
            _phase_mlp(tc, a, ext, cst, pools, st[a])
            _phase_mlp(tc, b, ext, cst, pools, st[b])
            del st[a], st[b]


def _layernorm(nc, pools, x_t, xn_t):
    """x_t [128, PPC, 256] f32 -> xn_t bf16 ((x-mu)*rstd).

    Stats on DVE (bn_stats), rstd via quake-rsqrt + 2 Newton steps on DVE
    (avoids the Sqrt activation-table load), apply on Pool."""
    statp = pools["statp"]
    mv = statp.tile([128, PPC, 2], FP32, tag="mv", name="mv")
    for j in range(PPC):
        bnst = statp.tile([128, 6], FP32, tag="bnst", name="bnst")
        nc.vector.bn_stats(bnst[:], x_t[:, j, :])
        nc.vector.bn_aggr(mv[:, j, :], bnst[:])
    var = statp.tile([128, PPC], FP32, tag="var", name="var")
    rst = statp.tile([128, PPC], FP32, tag="rst", name="rst")
    tmp = statp.tile([128, PPC], FP32, tag="tmp", name="tmp")
    nc.vector.tensor_scalar(out=var[:], in0=mv[:, :, 1], scalar1=EPS,
                            scalar2=None, op0=Alu.add)
    nc.vector.tensor_scalar(out=rst[:].bitcast(INT32), in0=var[:].bitcast(INT32),
                            scalar1=1, scalar2=None, op0=Alu.logical_shift_right)
    nc.vector.tensor_scalar(out=rst[:].bitcast(INT32), in0=rst[:].bitcast(INT32),
                            scalar1=-1, scalar2=RSQRT_MAGIC,
                            op0=Alu.mult, op1=Alu.add)
    for _ in range(2):
        nc.vector.tensor_tensor(out=tmp[:], in0=rst[:], in1=rst[:], op=Alu.mult)
        nc.vector.tensor_tensor(out=tmp[:], in0=tmp[:], in1=var[:], op=Alu.mult)
        nc.vector.tensor_scalar(out=tmp[:], in0=tmp[:], scalar1=-0.5,
                                scalar2=1.5, op0=Alu.mult, op1=Alu.add)
        nc.vector.tensor_tensor(out=rst[:], in0=rst[:], in1=tmp[:], op=Alu.mult)
    for j in range(PPC):
        nc.vector.tensor_scalar(
            out=xn_t[:, j, :], in0=x_t[:, j, :],
            scalar1=mv[:, j, 0:1], scalar2=rst[:, j:j + 1],
            op0=Alu.subtract, op1=Alu.mult,
        )


def _transposes(nc, pools, src_t, dst_T, drain_eng):
    """src_t [128, PPC, 256] -> dst_T [128, 2, PPC, 128] via PE + drain."""
    ident = pools["_ident"]
    for j in range(PPC):
        ps = pools["ps_tr"].tile([128, 2, 128], BF16_DT, tag="tok", name="trps")
        for ch in range(2):
            nc.tensor.transpose(ps[:, ch, :], src_t[:, j, 128 * ch:128 * (ch + 1)],
                                ident[:, :])
        if hasattr(drain_eng, "tensor_copy"):
            drain_eng.tensor_copy(out=dst_T[:, :, j, :], in_=ps[:])
        else:
            drain_eng.copy(out=dst_T[:, :, j, :], in_=ps[:])


def _phase_in(tc, ci, ext, cst, pools):
    """Load x, LN1, transpose, QKV -> qT/kT/vp."""
    nc = tc.nc
    s = {}

    x_t = pools["xp"].tile([128, PPC, C], FP32, tag="xo", name="x_t")
    nc.sync.dma_start(
        out=x_t[:],
        in_=ext["x"][ci * TPC:(ci + 1) * TPC, :].rearrange("(j p) c -> p j c", p=TPP),
    )
    s["x_t"] = x_t

    xn_t = pools["xnp"].tile([128, PPC, C], BF16_DT, tag="xn", name="xn_t")
    _layernorm(nc, pools, x_t, xn_t)
    xnT = pools["xnT"].tile([128, 2, PPC, 128], BF16_DT, tag="xnT", name="xnT")
    _transposes(nc, pools, xn_t, xnT, nc.vector)

    # q, k: channel-major slabs; drain on ACT (identity/copy, bias optional)
    qT = pools["qkp"].tile([128, 2, PPC, 128], BF16_DT, tag="qT", name="qT")
    kT = pools["qkp"].tile([128, 2, PPC, 128], BF16_DT, tag="kT", name="kT")
    for s2 in range(2):
        jsl = slice(4 * s2, 4 * s2 + 4)
        for (dstT, wname, bname) in ((qT, "wq", "bq"), (kT, "wk", "bk")):
            w_sb = cst[wname]
            for octl in range(2):
                ps = pools["ps_wide"].tile([128, 4, 128], FP32, tag="wide",
                                           name="qkps")
                for kt in range(2):
                    nc.tensor.matmul(
                        ps[:],
                        lhsT=w_sb[:, kt, 128 * octl:128 * (octl + 1)],
                        rhs=xnT[:, kt, jsl, :],
                        start=(kt == 0), stop=(kt == 1),
                    )
                if cst[bname] is not None:
                    nc.scalar.activation(
                        dstT[:, octl, jsl, :], ps[:], ActF.Identity,
                        bias=cst[bname][:, octl:octl + 1])
                else:
                    nc.scalar.activation(dstT[:, octl, jsl, :], ps[:], ActF.Copy)
    s["qT"], s["kT"] = qT, kT

    # PE matmul row-tile base 96 is unsupported; realign the hm==3 head rows
    # (h = 3, 7) of q/k to partition base 0 via one small DMA each.
    qk3 = pools["qk3p"].tile([32, 2, 2, PPC, 128], BF16_DT, tag="qk3", name="qk3")
    nc.sync.dma_start(out=qk3[0:32, 0], in_=qT[96:128, :, :, :])
    nc.sync.dma_start(out=qk3[0:32, 1], in_=kT[96:128, :, :, :])
    s["qk3"] = qk3

    # v: token-major, drain straight into pair-local vp (no base shift)
    vp = pools["vsp"].tile([128, PPC, NH, HD + 1], BF16_DT, tag="vp", name="vp")
    for j in range(PPC):
        ps = pools["ps_tok"].tile([128, C], FP32, tag="tok", name="vps")
        for kt in range(2):
            nc.tensor.matmul(
                ps[:], lhsT=xnT[:, kt, j, :], rhs=cst["wv"][:, kt, :],
                start=(kt == 0), stop=(kt == 1),
            )
        nc.vector.memset(vp[:, j, :, 0:1], 1.0)
        if cst["bvbc"] is not None:
            nc.vector.tensor_add(ps[:], ps[:], cst["bvbc"][:])
        nc.scalar.copy(
            out=vp[:, j, :, 1:HD + 1],
            in_=ps[:].rearrange("p (h d) -> p h d", h=NH),
        )
    s["vp"] = vp
    return s


def _phase_attn(tc, ci, cst, pools, s):
    """S = K^T Q pair-wide, exp, *exp(bias), A = et2 @ [1|v]."""
    nc = tc.nc
    qT, kT, vp = s["qT"], s["kT"], s["vp"]
    attn_t = pools["atp"].tile([128, PPC, C], BF16_DT, tag="attn", name="attn_t")
    if ci < 2:
        # first use of each ring buffer: seed pad rows (never valid-read,
        # but must be finite/initialized for the pair transposes). 96-aligned
        # partition start; rows 96:98 are re-written by the attn drain below.
        nc.vector.memset(attn_t[96:128, :, :], 0.0)
    qk3 = s["qk3"]
    for j in range(PPC):
        pt = int(_PTYPE[ci * PPC + j])
        et = pools["etp"].tile([128, NH, VPP], BF16_DT, tag="et", name="et")
        for hh in range(2):
            # h4-stride = 2KB: each concurrent PE row-tile owns its own bank
            pss = pools["ps_S"].tile([128, 4, 512], FP32, tag="S", name="pss")
            for h4 in range(4):
                h = 4 * hh + h4
                octl, hm = h // 4, h % 4
                if hm == 3:
                    lhsT = qk3[0:32, 1, octl, j, :]
                    rhs = qk3[0:32, 0, octl, j, 0:VPP]
                    base = 0
                else:
                    base = 32 * hm
                    lhsT = kT[base:base + 32, octl, j, :]
                    rhs = qT[base:base + 32, octl, j, 0:VPP]
                nc.tensor.matmul(
                    pss[:, h4, 0:VPP], lhsT=lhsT, rhs=rhs,
                    start=True, stop=True,
                    tile_position=(base, 0),
                )
            nc.scalar.activation(et[:, 4 * hh:4 * hh + 4, :], pss[:, :, 0:VPP],
                                 ActF.Exp)
        et2 = pools["et2p"].tile([128, NH, VPP], BF16_DT, tag="et2", name="et2")
        nc.vector.tensor_tensor(
            out=et2[:], in0=et[:], in1=cst["eb"][:, pt, :, :], op=Alu.mult,
        )
        psa = pools["ps_tok"].tile([VPP, NH, HD + 1], FP32, tag="tok", name="psa")
        for h in range(NH):
            nc.tensor.matmul(
                psa[:, h, :], lhsT=et2[:, h, :], rhs=vp[:, j, h, :],
                start=True, stop=True,
            )
        rec = pools["statp"].tile([VPP, NH], FP32, tag="rec", name="rec")
        nc.vector.tensor_scalar_max(out=rec[:], in0=psa[:, :, 0], scalar1=1e-30)
        nc.vector.reciprocal(rec[:], rec[:])
        rec_b = bass.AP(tensor=rec[:].tensor, offset=rec[:].offset,
                        ap=list(rec[:].ap) + [[0, HD]])
        nc.vector.tensor_mul(
            attn_t[0:VPP, j, :].rearrange("p (h d) -> p h d", h=NH),
            psa[:, :, 1:HD + 1], rec_b,
        )
    s["attn_t"] = attn_t


def _phase_proj(tc, ci, cst, pools, s):
    """attn transpose, proj + resid1, LN2, transpose -> xn2T (fp8)."""
    nc = tc.nc
    attnT = pools["attnT"].tile([128, 2, PPC, 128], BF16_DT, tag="attnT",
                                name="attnT")
    _transposes(nc, pools, s["attn_t"], attnT, nc.vector)

    x2_t = pools["x2p"].tile([128, PPC, C], FP32, tag="x2", name="x2_t")
    for j in range(PPC):
        ps = pools["ps_tok"].tile([128, C], FP32, tag="tok", name="prps")
        for kt in range(2):
            nc.tensor.matmul(
                ps[:], lhsT=attnT[:, kt, j, :], rhs=cst["wp"][:, kt, :],
                start=(kt == 0), stop=(kt == 1),
            )
        if cst["pbbc"] is not None:
            nc.vector.tensor_add(ps[:], ps[:], cst["pbbc"][:])
        nc.vector.scalar_tensor_tensor(
            out=x2_t[:, j, :], in0=ps[:], scalar=1.0,
            in1=s["x_t"][:, j, :], op0=Alu.mult, op1=Alu.add,
        )
    s["x2_t"] = x2_t

    xn2_t = pools["xnp"].tile([128, PPC, C], BF16_DT, tag="xn", name="xn2_t")
    _layernorm(nc, pools, x2_t, xn2_t)
    xn2T = pools["xn2T"].tile([128, 2, PPC, 128], FP8_DT, tag="xn2T", name="xn2T")
    _transposes(nc, pools, xn2_t, xn2T, nc.scalar)
    s["xn2T"] = xn2T


def _phase_mlp(tc, ci, ext, cst, pools, s):
    """MLP (fp8 DoubleRow) + resid2, store."""
    nc = tc.nc
    xn2T = s["xn2T"]
    hT = pools["hp"].tile([128, 8, PPC, 128], FP8_DT, tag="hT", name="hT")
    if ci < 2:
        nc.vector.memset(hT[:, :, :, VPP:128], 0.0)
    for s2 in range(2):
        jsl = slice(4 * s2, 4 * s2 + 4)
        for m in range(8):
            ps = pools["ps_wide"].tile([128, 4, 128], FP32, tag="wide",
                                       name="m1ps")
            nc.tensor.matmul(
                ps[:], lhsT=cst["w1"][:, :, 128 * m:128 * (m + 1)],
                rhs=xn2T[:, :, jsl, :],
                start=True, stop=True,
                perf_mode=mybir.MatmulPerfMode.DoubleRow,
            )
            bias = (cst["b1"][:, m:m + 1] if cst["b1"] is not None else 0.0)
            import os
            gelu_f = ActF.Identity if os.environ.get("GELU_ID") else ActF.Gelu
            nc.scalar.activation(
                hT[:, m, jsl, 0:VPP], ps[:, :, 0:VPP],
                gelu_f, bias=bias, scale=1.0 / W8SCALE,
            )

    out_t = pools["xp"].tile([128, PPC, C], FP32, tag="xo", name="out_t")
    for j in range(PPC):
        ps = pools["ps_tok"].tile([128, C], FP32, tag="tok", name="m2ps")
        for k2 in range(4):
            nc.tensor.matmul(
                ps[:], lhsT=hT[:, 2 * k2:2 * k2 + 2, j, :],
                rhs=cst["w2"][:, 2 * k2:2 * k2 + 2, :],
                start=(k2 == 0), stop=(k2 == 3),
                perf_mode=mybir.MatmulPerfMode.DoubleRow,
            )
        if cst["b2bc"] is not None:
            nc.vector.tensor_add(ps[:], ps[:], cst["b2bc"][:])
        nc.vector.scalar_tensor_tensor(
            out=out_t[:, j, :], in0=ps[:], scalar=1.0 / W8SCALE,
            in1=s["x2_t"][:, j, :], op0=Alu.mult, op1=Alu.add,
        )

    # compact store: pair (ci*PPC + j) valid rows 0:VPP
    dst = ext["out"][ci * VPC:, :]
    dst_ap = bass.AP(
        tensor=dst.tensor, offset=dst.offset,
        ap=[[C, VPP], [VPP * C, PPC], [1, C]],
    )
    nc.sync.dma_start(out=dst_ap, in_=out_t[0:VPP, :, :])


# --------------------------------------------------------------------------
# entry point
# --------------------------------------------------------------------------

_CACHE = {}


def _get_program(key_flags):
    if key_flags not in _CACHE:
        _CACHE[key_flags] = build_program(NPAIR, flags=dict(key_flags))
    return _CACHE[key_flags]


def kernel(**inputs):
    in_maps, extra = host_prep(inputs)
    nc = _get_program(tuple(sorted(extra.items())))
    res = run_bass_kernel_spmd(nc, in_maps, core_ids=list(range(B)))
    out = np.stack([res.results[i]["out"] for i in range(B)], axis=0)
    return out.reshape(B, NWIN, S, C).astype(np.float32)


# revision 39
# speedup vs baseline: 2.1404x; 1.1519x over previous
"""Swin-style windowed-attention block on 8 TRN2 NeuronCores (data-parallel over batch).

v2: compact-pair layout (both windows' 49 valid tokens at rows 0:98 of a 128-row
pair tile, zero pad rows 98:128). Pair-wide attention with post-exp multiplicative
bias (exp(S+b) = exp(S)*exp(b) with a host-precomputed exp(bias) table that also
zeroes cross-window blocks and pad rows), no augmented-K matmuls, no head-realign
or v-shift DMAs. Activation-table thrash removed (DVE Newton rsqrt; chunk-pair
interleaving batches exp/gelu). Elementwise spread across ACT / DVE / Pool.
"""

import sys

sys.path.insert(0, "/opt/trn_rl_repo")

import numpy as np
import ml_dtypes

import concourse.bass as bass
import concourse.bacc as bacc
import concourse.tile as tile
import concourse.mybir as mybir
from concourse.bass_utils import run_bass_kernel_spmd

BF16 = ml_dtypes.bfloat16
FP8 = ml_dtypes.float8_e4m3
FP32 = mybir.dt.float32
BF16_DT = mybir.dt.bfloat16
FP8_DT = mybir.dt.float8e4
INT32 = mybir.dt.int32
W8SCALE = 64.0

# ---- static geometry ----
WH, WW = 7, 7
S = 49                     # valid tokens per window
C = 256                    # channels
NH = 8                     # heads
HD = 32                    # head dim
NWIN = 256                 # windows per batch image
B = 8                      # batch == number of cores
GRID = 16                  # 16x16 window grid
SCALE = HD ** -0.5
EPS = 1e-5
MASK_VAL = -30000.0

NPAIR = NWIN // 2          # 128 window pairs per core
PPC = 8                    # pairs per chunk
NCHUNK = NPAIR // PPC      # 16 chunks
TPP = 128                  # tile rows per pair (98 valid + 30 zero pad)
VPP = 2 * S                # 98 valid tokens per pair
TPC = PPC * TPP            # 1024 padded tokens per chunk
VPC = PPC * VPP            # 784 valid tokens per chunk
NTOK = NWIN * S            # 12544 valid tokens per core
NTOKP = NPAIR * TPP        # 16384 padded tokens per core

RSQRT_MAGIC = 0x5F3759DF

ActF = mybir.ActivationFunctionType
Alu = mybir.AluOpType


# --------------------------------------------------------------------------
# host-side preparation
# --------------------------------------------------------------------------

def _relative_position_index():
    ch, cw = np.arange(WH), np.arange(WW)
    coords = np.stack(np.meshgrid(ch, cw, indexing="ij")).reshape(2, -1)
    rel = coords[:, :, None] - coords[:, None, :]
    rel = rel.transpose(1, 2, 0).astype(np.int64)
    rel[..., 0] += WH - 1
    rel[..., 1] += WW - 1
    rel[..., 0] *= 2 * WW - 1
    return rel.sum(-1)                                    # (S, S)


def _window_mask_types():
    """Per-window mask type: 0 none, 1 bottom-row, 2 right-col, 3 corner."""
    h = w = GRID
    s1, s2 = WH - WH // 2, WW - WW // 2
    m = np.zeros((h, w, WH, WW, WH, WW), dtype=bool)
    m[-1, :, :s1, :, s1:, :] = True
    m[-1, :, s1:, :, :s1, :] = True
    m[:, -1, :, :s2, :, s2:] = True
    m[:, -1, :, s2:, :, :s2] = True
    m = m.reshape(h * w, S, S)
    types = np.zeros(NWIN, dtype=np.int64)
    rr, cc = np.divmod(np.arange(NWIN), GRID)
    types[(rr == GRID - 1) & (cc < GRID - 1)] = 1
    types[(rr < GRID - 1) & (cc == GRID - 1)] = 2
    types[(rr == GRID - 1) & (cc == GRID - 1)] = 3
    masks = np.zeros((4, S, S), dtype=np.float32)
    masks[1] = np.where(m[GRID * (GRID - 1)], MASK_VAL, 0.0)
    masks[2] = np.where(m[GRID - 1], MASK_VAL, 0.0)
    masks[3] = np.where(m[NWIN - 1], MASK_VAL, 0.0)
    return types, masks


def _pair_types():
    types, _ = _window_mask_types()
    combos = []
    ptype = np.zeros(NPAIR, dtype=np.int64)
    for j in range(NPAIR):
        c = (int(types[2 * j]), int(types[2 * j + 1]))
        if c not in combos:
            combos.append(c)
        ptype[j] = combos.index(c)
    assert len(combos) <= 4, combos
    while len(combos) < 4:
        combos.append((0, 0))
    return ptype, combos


_PTYPE, _PCOMBOS = _pair_types()


def _tile_kxoc(wT):
    """[K, OC] -> [128, K//128, OC] with K = 128*kt + p."""
    K, OC = wT.shape
    return np.ascontiguousarray(wT.reshape(K // 128, 128, OC).transpose(1, 0, 2))


def host_prep(inputs):
    x = np.asarray(inputs["x"], dtype=np.float32)          # (B, N, S, C)
    qkv_w = np.asarray(inputs["qkv_w"], dtype=np.float32)
    qkv_b = np.asarray(inputs["qkv_b"], dtype=np.float32)
    proj_w = np.asarray(inputs["proj_w"], dtype=np.float32)
    proj_b = np.asarray(inputs["proj_b"], dtype=np.float32)
    n1g = np.asarray(inputs["norm1_g"], dtype=np.float32)
    n1b = np.asarray(inputs["norm1_b"], dtype=np.float32)
    n2g = np.asarray(inputs["norm2_g"], dtype=np.float32)
    n2b = np.asarray(inputs["norm2_b"], dtype=np.float32)
    w1 = np.asarray(inputs["mlp_w1"], dtype=np.float32)
    b1 = np.asarray(inputs["mlp_b1"], dtype=np.float32)
    w2 = np.asarray(inputs["mlp_w2"], dtype=np.float32)
    b2 = np.asarray(inputs["mlp_b2"], dtype=np.float32)
    table = np.asarray(inputs["bias_table"], dtype=np.float32)

    # fold layernorm affine into the following matmuls
    qkv_w_f = qkv_w * n1g[None, :]
    qkv_b_f = qkv_b + qkv_w @ n1b
    w1_f = w1 * n2g[None, :]
    b1_f = b1 + w1 @ n2b

    wq = qkv_w_f[0:C] * SCALE
    bq = qkv_b_f[0:C] * SCALE
    wk = qkv_w_f[C:2 * C]
    bk = qkv_b_f[C:2 * C]
    wv = qkv_w_f[2 * C:3 * C]
    bv = qkv_b_f[2 * C:3 * C]

    common = {
        "wq": _tile_kxoc(wq.T).astype(BF16),
        "wk": _tile_kxoc(wk.T).astype(BF16),
        "wv": _tile_kxoc(wv.T).astype(BF16),
        "wp": _tile_kxoc(proj_w.T).astype(BF16),
        "w1": _tile_kxoc(w1_f.T * W8SCALE).astype(FP8),
        "w2": _tile_kxoc(w2.T * W8SCALE).astype(FP8),
    }

    # exp(bias + mask) multiplicative table: eb[t_row, ptype, h, q_row]
    # t_row/q_row = 49*r + local; zero on cross-window blocks and pad rows.
    rel = _relative_position_index()
    bias_sht = table[rel].transpose(2, 0, 1)               # [h, s, t]
    _, masks = _window_mask_types()                        # [4, s, t]
    eb = np.zeros((TPP, 4, NH, VPP), dtype=np.float32)
    for pt, (tA, tB) in enumerate(_PCOMBOS):
        for r, wt in ((0, tA), (1, tB)):
            blk = np.exp(bias_sht + masks[wt][None])       # [h, s, t]
            eb[S * r:S * r + S, pt, :, S * r:S * r + S] = blk.transpose(2, 0, 1)
    common["eb"] = eb.astype(BF16)

    extra = {
        "bq_nz": bool(np.any(bq != 0.0) or np.any(bk != 0.0)),
        "bv_nz": bool(np.any(bv != 0.0)),
        "pb_nz": bool(np.any(proj_b != 0.0)),
        "b1_nz": bool(np.any(b1_f != 0.0)),
        "b2_nz": bool(np.any(b2 != 0.0)),
    }
    if extra["bq_nz"]:
        common["bq"] = np.ascontiguousarray(bq.reshape(2, 128).T).astype(np.float32)
        common["bk"] = np.ascontiguousarray(bk.reshape(2, 128).T).astype(np.float32)
    if extra["b1_nz"]:
        common["b1"] = np.ascontiguousarray(b1_f.reshape(8, 128).T).astype(np.float32)
    if extra["bv_nz"]:
        common["bvbc"] = np.tile(bv[None, :], (128, 1)).astype(np.float32)
    if extra["pb_nz"]:
        common["pbbc"] = np.tile(proj_b[None, :], (128, 1)).astype(np.float32)
    if extra["b2_nz"]:
        common["b2bc"] = np.tile(b2[None, :], (128, 1)).astype(np.float32)

    in_maps = []
    for b in range(B):
        m = dict(common)
        xp = np.zeros((NPAIR, TPP, C), dtype=BF16)
        xp[:, :VPP, :] = x[b].reshape(NPAIR, VPP, C).astype(BF16)
        m["x"] = xp.reshape(NTOKP, C)
        in_maps.append(m)
    return in_maps, extra


# --------------------------------------------------------------------------
# kernel builder
# --------------------------------------------------------------------------

def build_program(n_pairs=NPAIR, flags=None):
    flags = flags or {}
    assert n_pairs % (2 * PPC) == 0
    n_chunks = n_pairs // PPC

    nc = bacc.Bacc("TRN2", target_bir_lowering=False, debug=False)

    ext = {}
    ext["x"] = nc.dram_tensor("x", [n_pairs * TPP, C], BF16_DT, kind="ExternalInput")
    ext["out"] = nc.dram_tensor("out", [n_pairs * VPP, C], FP32, kind="ExternalOutput")
    ext["wq"] = nc.dram_tensor("wq", [128, 2, C], BF16_DT, kind="ExternalInput")
    ext["wk"] = nc.dram_tensor("wk", [128, 2, C], BF16_DT, kind="ExternalInput")
    ext["wv"] = nc.dram_tensor("wv", [128, 2, C], BF16_DT, kind="ExternalInput")
    ext["wp"] = nc.dram_tensor("wp", [128, 2, C], BF16_DT, kind="ExternalInput")
    ext["w1"] = nc.dram_tensor("w1", [128, 2, 4 * C], FP8_DT, kind="ExternalInput")
    ext["w2"] = nc.dram_tensor("w2", [128, 8, C], FP8_DT, kind="ExternalInput")
    ext["eb"] = nc.dram_tensor("eb", [TPP, 4, NH, VPP], BF16_DT, kind="ExternalInput")
    for name, shape, flg in (
        ("bq", [128, 2], "bq_nz"), ("bk", [128, 2], "bq_nz"),
        ("b1", [128, 8], "b1_nz"),
        ("bvbc", [128, C], "bv_nz"), ("pbbc", [128, C], "pb_nz"),
        ("b2bc", [128, C], "b2_nz"),
    ):
        ext[name] = (nc.dram_tensor(name, shape, FP32, kind="ExternalInput")
                     if flags.get(flg) else None)

    with tile.TileContext(nc) as tc:
        _body(tc, n_chunks, ext)

    nc.compile()
    return nc


def _body(tc, n_chunks, ext):
    nc = tc.nc
    import contextlib
    with contextlib.ExitStack() as ctx:
        const = ctx.enter_context(tc.tile_pool(name="const", bufs=1))
        cst = {}
        for name, shape, dt in (
            ("wq", [128, 2, C], BF16_DT), ("wk", [128, 2, C], BF16_DT),
            ("wv", [128, 2, C], BF16_DT), ("wp", [128, 2, C], BF16_DT),
            ("w1", [128, 2, 4 * C], FP8_DT), ("w2", [128, 8, C], FP8_DT),
            ("eb", [TPP, 4, NH, VPP], BF16_DT),
            ("bq", [128, 2], FP32), ("bk", [128, 2], FP32),
            ("b1", [128, 8], FP32),
            ("bvbc", [128, C], FP32), ("pbbc", [128, C], FP32),
            ("b2bc", [128, C], FP32),
        ):
            if ext.get(name) is None:
                cst[name] = None
                continue
            t = const.tile(shape, dt, tag=name, name=name)
            nc.sync.dma_start(out=t[:], in_=ext[name].ap())
            cst[name] = t
        ident = const.tile([128, 128], BF16_DT, tag="ident", name="ident")
        from concourse.masks import make_identity
        make_identity(nc, ident[:])
        cst["ident"] = ident

        pools = {}
        for name, bufs in (("xp", 2), ("xbp", 2), ("xnp", 2), ("xnT", 2),
                           ("attnT", 2), ("xn2T", 2), ("qkp", 2), ("qk3p", 2),
                           ("vsp", 2), ("etp", 2), ("et2p", 2), ("atp", 2),
                           ("x2p", 2), ("hp", 2), ("statp", 2)):
            pools[name] = ctx.enter_context(tc.tile_pool(name=name, bufs=bufs))
        # PSUM budget (8 banks): ps_S 4 (one bank per concurrent PE row-tile),
        # ps_wide 2, ps_tok 2 (shared ring: v/psa/proj/mlp2/transpose drains)
        for name, bufs in (("ps_wide", 2), ("ps_tok", 2), ("ps_S", 1)):
            pools[name] = ctx.enter_context(
                tc.tile_pool(name=name, bufs=bufs, space="PSUM"))
        pools["ps_tr"] = pools["ps_tok"]
        pools["_ident"] = cst["ident"]

        import os
        n_phases = int(os.environ.get("PHASES", "4"))

        def _store_dbg(ci, t):
            dst = ext["out"][ci * VPC:, :]
            dst_ap = bass.AP(tensor=dst.tensor, offset=dst.offset,
                             ap=[[C, VPP], [VPP * C, PPC], [1, C]])
            nc.sync.dma_start(out=dst_ap, in_=t[0:VPP, :, :])

        st = {}
        for cp in range(n_chunks // 2):
            a, b = 2 * cp, 2 * cp + 1
            st[a] = _phase_in(tc, a, ext, cst, pools)
            st[b] = _phase_in(tc, b, ext, cst, pools)
            if n_phases < 2:
                _store_dbg(a, st[a]["x_t"])
                _store_dbg(b, st[b]["x_t"])
                del st[a], st[b]
                continue
            _phase_attn(tc, a, cst, pools, st[a])
            _phase_attn(tc, b, cst, pools, st[b])
            if n_phases < 3:
                _store_dbg(a, st[a]["x_t"])
                _store_dbg(b, st[b]["x_t"])
                del st[a], st[b]
                continue
            _phase_proj(tc, a, cst, pools, st[a])
            _phase_proj(tc, b, cst, pools, st[b])
            if n_phases < 4:
                _store_dbg(a, st[a]["x2_t"])
                _store_dbg(b, st[b]["x2_t"])
                del st[a], st[b]
                continue
            _phase_mlp(tc, a, ext, cst, pools, st[a])
            _phase_mlp(tc, b, ext, cst, pools, st[b])
            del st[a], st[b]


def _layernorm(nc, pools, x_t, xn_t):
    """x_t [128, PPC, 256] bf16 -> xn_t bf16 ((x-mu)*rstd).

    Batched stats on DVE (bn_stats 2 pairs/instr), rstd via quake-rsqrt +
    2 Newton steps on DVE (avoids the Sqrt activation-table load), apply on
    DVE (2x/4x with bf16 operands)."""
    statp = pools["statp"]
    mv = statp.tile([128, PPC, 2], FP32, tag="mv", name="mv")
    for j in range(PPC):
        bnst = statp.tile([128, 6], FP32, tag="bnst", name="bnst")
        nc.vector.bn_stats(bnst[:], x_t[:, j, :])
        nc.vector.bn_aggr(mv[:, j, :], bnst[:])
    var = statp.tile([128, PPC], FP32, tag="var", name="var")
    rst = statp.tile([128, PPC], FP32, tag="rst", name="rst")
    tmp = statp.tile([128, PPC], FP32, tag="tmp", name="tmp")
    nc.vector.tensor_scalar(out=var[:], in0=mv[:, :, 1], scalar1=EPS,
                            scalar2=None, op0=Alu.add)
    nc.vector.tensor_scalar(out=rst[:].bitcast(INT32), in0=var[:].bitcast(INT32),
                            scalar1=1, scalar2=None, op0=Alu.logical_shift_right)
    nc.vector.tensor_scalar(out=rst[:].bitcast(INT32), in0=rst[:].bitcast(INT32),
                            scalar1=-1, scalar2=RSQRT_MAGIC,
                            op0=Alu.mult, op1=Alu.add)
    for _ in range(2):
        nc.vector.tensor_tensor(out=tmp[:], in0=rst[:], in1=rst[:], op=Alu.mult)
        nc.vector.tensor_tensor(out=tmp[:], in0=tmp[:], in1=var[:], op=Alu.mult)
        nc.vector.tensor_scalar(out=tmp[:], in0=tmp[:], scalar1=-0.5,
                                scalar2=1.5, op0=Alu.mult, op1=Alu.add)
        nc.vector.tensor_tensor(out=rst[:], in0=rst[:], in1=tmp[:], op=Alu.mult)
    for j in range(PPC):
        nc.vector.tensor_scalar(
            out=xn_t[:, j, :], in0=x_t[:, j, :],
            scalar1=mv[:, j, 0:1], scalar2=rst[:, j:j + 1],
            op0=Alu.subtract, op1=Alu.mult,
        )


def _transposes(nc, pools, src_t, dst_T, drain_eng):
    """src_t [128, PPC, 256] -> dst_T [128, 2, PPC, 128] via PE + drain."""
    ident = pools["_ident"]
    for j in range(PPC):
        ps = pools["ps_tr"].tile([128, 2, 128], BF16_DT, tag="tok", name="trps")
        for ch in range(2):
            nc.tensor.transpose(ps[:, ch, :], src_t[:, j, 128 * ch:128 * (ch + 1)],
                                ident[:, :])
        if hasattr(drain_eng, "tensor_copy"):
            drain_eng.tensor_copy(out=dst_T[:, :, j, :], in_=ps[:])
        else:
            drain_eng.copy(out=dst_T[:, :, j, :], in_=ps[:])


def _phase_in(tc, ci, ext, cst, pools):
    """Load x, LN1, transpose, QKV -> qT/kT/vp."""
    nc = tc.nc
    s = {}

    x_t = pools["xbp"].tile([128, PPC, C], BF16_DT, tag="xb", name="x_t")
    nc.sync.dma_start(
        out=x_t[:],
        in_=ext["x"][ci * TPC:(ci + 1) * TPC, :].rearrange("(j p) c -> p j c", p=TPP),
    )
    s["x_t"] = x_t

    xn_t = pools["xnp"].tile([128, PPC, C], BF16_DT, tag="xn", name="xn_t")
    _layernorm(nc, pools, x_t, xn_t)
    xnT = pools["xnT"].tile([128, 2, PPC, 128], BF16_DT, tag="xnT", name="xnT")
    _transposes(nc, pools, xn_t, xnT, nc.vector)

    # q, k: channel-major slabs; drain on ACT (identity/copy, bias optional)
    qT = pools["qkp"].tile([128, 2, PPC, 128], BF16_DT, tag="qT", name="qT")
    kT = pools["qkp"].tile([128, 2, PPC, 128], BF16_DT, tag="kT", name="kT")
    for s2 in range(2):
        jsl = slice(4 * s2, 4 * s2 + 4)
        for (dstT, wname, bname) in ((qT, "wq", "bq"), (kT, "wk", "bk")):
            w_sb = cst[wname]
            for octl in range(2):
                ps = pools["ps_wide"].tile([128, 4, 128], FP32, tag="wide",
                                           name="qkps")
                for kt in range(2):
                    nc.tensor.matmul(
                        ps[:],
                        lhsT=w_sb[:, kt, 128 * octl:128 * (octl + 1)],
                        rhs=xnT[:, kt, jsl, :],
                        start=(kt == 0), stop=(kt == 1),
                    )
                if cst[bname] is not None:
                    nc.scalar.activation(
                        dstT[:, octl, jsl, :], ps[:], ActF.Identity,
                        bias=cst[bname][:, octl:octl + 1])
                else:
                    nc.scalar.activation(dstT[:, octl, jsl, :], ps[:], ActF.Copy)
    s["qT"], s["kT"] = qT, kT

    # PE matmul row-tile base 96 is unsupported; realign the hm==3 head rows
    # (h = 3, 7) of q/k to partition base 0 via one small DMA each.
    qk3 = pools["qk3p"].tile([32, 2, 2, PPC, 128], BF16_DT, tag="qk3", name="qk3")
    nc.sync.dma_start(out=qk3[0:32, 0], in_=qT[96:128, :, :, :])
    nc.sync.dma_start(out=qk3[0:32, 1], in_=kT[96:128, :, :, :])
    s["qk3"] = qk3

    # v: token-major, drain straight into pair-local vp (no base shift)
    vp = pools["vsp"].tile([128, PPC, NH, HD + 1], BF16_DT, tag="vp", name="vp")
    for j in range(PPC):
        ps = pools["ps_tok"].tile([128, C], FP32, tag="tok", name="vps")
        for kt in range(2):
            nc.tensor.matmul(
                ps[:], lhsT=xnT[:, kt, j, :], rhs=cst["wv"][:, kt, :],
                start=(kt == 0), stop=(kt == 1),
            )
        nc.vector.memset(vp[:, j, :, 0:1], 1.0)
        if cst["bvbc"] is not None:
            nc.vector.tensor_add(ps[:], ps[:], cst["bvbc"][:])
        nc.scalar.copy(
            out=vp[:, j, :, 1:HD + 1],
            in_=ps[:].rearrange("p (h d) -> p h d", h=NH),
        )
    s["vp"] = vp
    return s


def _phase_attn(tc, ci, cst, pools, s):
    """S = K^T Q pair-wide, exp, *exp(bias), A = et2 @ [1|v]."""
    nc = tc.nc
    qT, kT, vp = s["qT"], s["kT"], s["vp"]
    attn_t = pools["atp"].tile([128, PPC, C], BF16_DT, tag="attn", name="attn_t")
    if ci < 2:
        # first use of each ring buffer: seed pad rows (never valid-read,
        # but must be finite/initialized for the pair transposes). 96-aligned
        # partition start; rows 96:98 are re-written by the attn drain below.
        nc.vector.memset(attn_t[96:128, :, :], 0.0)
    qk3 = s["qk3"]
    JB = 4                                      # pairs per exp batch
    for jg in range(PPC // JB):
        js = list(range(JB * jg, JB * jg + JB))
        # et holds a 4-pair batch: [t, h, jj, q]
        et = pools["etp"].tile([128, NH, JB, VPP], BF16_DT, tag="et", name="et")
        for hh in range(2):
            # h4-stride = 2KB: each concurrent PE row-tile owns its own bank;
            # the JB pairs pack at 128-col offsets within each bank.
            pss = pools["ps_S"].tile([128, 4, JB, 128], FP32, tag="S", name="pss")
            for jj, j in enumerate(js):
                for h4 in range(4):
                    h = 4 * hh + h4
                    octl, hm = h // 4, h % 4
                    if hm == 3:
                        lhsT = qk3[0:32, 1, octl, j, :]
                        rhs = qk3[0:32, 0, octl, j, 0:VPP]
                        base = 0
                    else:
                        base = 32 * hm
                        lhsT = kT[base:base + 32, octl, j, :]
                        rhs = qT[base:base + 32, octl, j, 0:VPP]
                    nc.tensor.matmul(
                        pss[:, h4, jj, 0:VPP], lhsT=lhsT, rhs=rhs,
                        start=True, stop=True,
                        tile_position=(base, 0),
                    )
            nc.scalar.activation(et[:, 4 * hh:4 * hh + 4, :, :],
                                 pss[:, :, :, 0:VPP], ActF.Exp)
        # eb multiply, batched over runs of equal pair-type
        et2 = pools["et2p"].tile([128, NH, JB, VPP], BF16_DT, tag="et2",
                                 name="et2")
        runs = []
        for jj, j in enumerate(js):
            pt = int(_PTYPE[ci * PPC + j])
            if runs and runs[-1][0] == pt:
                runs[-1][2] = jj + 1
            else:
                runs.append([pt, jj, jj + 1])
        for pt, j0, j1 in runs:
            e3 = cst["eb"][:, pt, :, :]
            eb_b = bass.AP(tensor=e3.tensor, offset=e3.offset,
                           ap=[e3.ap[0], e3.ap[1], [0, j1 - j0], e3.ap[2]])
            nc.vector.tensor_tensor(
                out=et2[:, :, j0:j1, :], in0=et[:, :, j0:j1, :],
                in1=eb_b, op=Alu.mult,
            )
        for jj, j in enumerate(js):
            psa = pools["ps_tok"].tile([VPP, NH, HD + 1], FP32, tag="tok",
                                       name="psa")
            for h in range(NH):
                nc.tensor.matmul(
                    psa[:, h, :], lhsT=et2[:, h, jj, :], rhs=vp[:, j, h, :],
                    start=True, stop=True,
                )
            rec = pools["statp"].tile([VPP, NH], FP32, tag="rec", name="rec")
            nc.vector.tensor_scalar_max(out=rec[:], in0=psa[:, :, 0],
                                        scalar1=1e-30)
            nc.vector.reciprocal(rec[:], rec[:])
            rec_b = bass.AP(tensor=rec[:].tensor, offset=rec[:].offset,
                            ap=list(rec[:].ap) + [[0, HD]])
            nc.vector.tensor_mul(
                attn_t[0:VPP, j, :].rearrange("p (h d) -> p h d", h=NH),
                psa[:, :, 1:HD + 1], rec_b,
            )
    s["attn_t"] = attn_t


def _phase_proj(tc, ci, cst, pools, s):
    """attn transpose, proj + resid1, LN2, transpose -> xn2T (fp8)."""
    nc = tc.nc
    attnT = pools["attnT"].tile([128, 2, PPC, 128], BF16_DT, tag="attnT",
                                name="attnT")
    _transposes(nc, pools, s["attn_t"], attnT, nc.vector)

    x2_t = pools["x2p"].tile([128, PPC, C], BF16_DT, tag="x2", name="x2_t")
    for j in range(PPC):
        ps = pools["ps_tok"].tile([128, C], FP32, tag="tok", name="prps")
        for kt in range(2):
            nc.tensor.matmul(
                ps[:], lhsT=attnT[:, kt, j, :], rhs=cst["wp"][:, kt, :],
                start=(kt == 0), stop=(kt == 1),
            )
        if cst["pbbc"] is not None:
            nc.vector.tensor_add(ps[:], ps[:], cst["pbbc"][:])
        nc.vector.scalar_tensor_tensor(
            out=x2_t[:, j, :], in0=ps[:], scalar=1.0,
            in1=s["x_t"][:, j, :], op0=Alu.mult, op1=Alu.add,
        )
    s["x2_t"] = x2_t

    xn2_t = pools["xnp"].tile([128, PPC, C], BF16_DT, tag="xn", name="xn2_t")
    _layernorm(nc, pools, x2_t, xn2_t)
    xn2T = pools["xn2T"].tile([128, 2, PPC, 128], FP8_DT, tag="xn2T", name="xn2T")
    _transposes(nc, pools, xn2_t, xn2T, nc.scalar)
    s["xn2T"] = xn2T


def _phase_mlp(tc, ci, ext, cst, pools, s):
    """MLP (fp8 DoubleRow) + resid2, store."""
    nc = tc.nc
    xn2T = s["xn2T"]
    hT = pools["hp"].tile([128, 8, PPC, 128], FP8_DT, tag="hT", name="hT")
    if ci < 2:
        nc.vector.memset(hT[:, :, :, VPP:128], 0.0)
    for s2 in range(2):
        jsl = slice(4 * s2, 4 * s2 + 4)
        for m in range(8):
            ps = pools["ps_wide"].tile([128, 4, 128], FP32, tag="wide",
                                       name="m1ps")
            nc.tensor.matmul(
                ps[:], lhsT=cst["w1"][:, :, 128 * m:128 * (m + 1)],
                rhs=xn2T[:, :, jsl, :],
                start=True, stop=True,
                perf_mode=mybir.MatmulPerfMode.DoubleRow,
            )
            bias = (cst["b1"][:, m:m + 1] if cst["b1"] is not None else 0.0)
            import os
            gelu_f = ActF.Identity if os.environ.get("GELU_ID") else ActF.Gelu
            nc.scalar.activation(
                hT[:, m, jsl, 0:VPP], ps[:, :, 0:VPP],
                gelu_f, bias=bias, scale=1.0 / W8SCALE,
            )

    out_t = pools["xp"].tile([128, PPC, C], FP32, tag="xo", name="out_t")
    for j in range(PPC):
        ps = pools["ps_tok"].tile([128, C], FP32, tag="tok", name="m2ps")
        for k2 in range(4):
            nc.tensor.matmul(
                ps[:], lhsT=hT[:, 2 * k2:2 * k2 + 2, j, :],
                rhs=cst["w2"][:, 2 * k2:2 * k2 + 2, :],
                start=(k2 == 0), stop=(k2 == 3),
                perf_mode=mybir.MatmulPerfMode.DoubleRow,
            )
        if cst["b2bc"] is not None:
            nc.vector.tensor_add(ps[:], ps[:], cst["b2bc"][:])
        nc.vector.scalar_tensor_tensor(
            out=out_t[:, j, :], in0=ps[:], scalar=1.0 / W8SCALE,
            in1=s["x2_t"][:, j, :], op0=Alu.mult, op1=Alu.add,
        )

    # compact store: pair (ci*PPC + j) valid rows 0:VPP
    dst = ext["out"][ci * VPC:, :]
    dst_ap = bass.AP(
        tensor=dst.tensor, offset=dst.offset,
        ap=[[C, VPP], [VPP * C, PPC], [1, C]],
    )
    nc.sync.dma_start(out=dst_ap, in_=out_t[0:VPP, :, :])


# --------------------------------------------------------------------------
# entry point
# --------------------------------------------------------------------------

_CACHE = {}


def _get_program(key_flags):
    if key_flags not in _CACHE:
        _CACHE[key_flags] = build_program(NPAIR, flags=dict(key_flags))
    return _CACHE[key_flags]


def kernel(**inputs):
    in_maps, extra = host_prep(inputs)
    nc = _get_program(tuple(sorted(extra.items())))
    res = run_bass_kernel_spmd(nc, in_maps, core_ids=list(range(B)))
    out = np.stack([res.results[i]["out"] for i in range(B)], axis=0)
    return out.reshape(B, NWIN, S, C).astype(np.float32)


# revision 45
# speedup vs baseline: 2.1938x; 1.0249x over previous
"""Swin-style windowed-attention block on 8 TRN2 NeuronCores (data-parallel over batch).

v2: compact-pair layout (both windows' 49 valid tokens at rows 0:98 of a 128-row
pair tile, zero pad rows 98:128). Pair-wide attention with post-exp multiplicative
bias (exp(S+b) = exp(S)*exp(b) with a host-precomputed exp(bias) table that also
zeroes cross-window blocks and pad rows), no augmented-K matmuls, no head-realign
or v-shift DMAs. Activation-table thrash removed (DVE Newton rsqrt; chunk-pair
interleaving batches exp/gelu). Elementwise spread across ACT / DVE / Pool.
"""

import sys

sys.path.insert(0, "/opt/trn_rl_repo")

import numpy as np
import ml_dtypes

import concourse.bass as bass
import concourse.bacc as bacc
import concourse.tile as tile
import concourse.mybir as mybir
from concourse.bass_utils import run_bass_kernel_spmd

BF16 = ml_dtypes.bfloat16
FP8 = ml_dtypes.float8_e4m3
FP32 = mybir.dt.float32
BF16_DT = mybir.dt.bfloat16
FP8_DT = mybir.dt.float8e4
INT32 = mybir.dt.int32
W8SCALE = 64.0

# ---- static geometry ----
WH, WW = 7, 7
S = 49                     # valid tokens per window
C = 256                    # channels
NH = 8                     # heads
HD = 32                    # head dim
NWIN = 256                 # windows per batch image
B = 8                      # batch == number of cores
GRID = 16                  # 16x16 window grid
SCALE = HD ** -0.5
EPS = 1e-5
MASK_VAL = -30000.0

NPAIR = NWIN // 2          # 128 window pairs per core
PPC = 8                    # pairs per chunk
NCHUNK = NPAIR // PPC      # 16 chunks
TPP = 128                  # tile rows per pair (98 valid + 30 zero pad)
VPP = 2 * S                # 98 valid tokens per pair
TPC = PPC * TPP            # 1024 padded tokens per chunk
VPC = PPC * VPP            # 784 valid tokens per chunk
NTOK = NWIN * S            # 12544 valid tokens per core
NTOKP = NPAIR * TPP        # 16384 padded tokens per core

RSQRT_MAGIC = 0x5F3759DF

ActF = mybir.ActivationFunctionType
Alu = mybir.AluOpType


# --------------------------------------------------------------------------
# host-side preparation
# --------------------------------------------------------------------------

def _relative_position_index():
    ch, cw = np.arange(WH), np.arange(WW)
    coords = np.stack(np.meshgrid(ch, cw, indexing="ij")).reshape(2, -1)
    rel = coords[:, :, None] - coords[:, None, :]
    rel = rel.transpose(1, 2, 0).astype(np.int64)
    rel[..., 0] += WH - 1
    rel[..., 1] += WW - 1
    rel[..., 0] *= 2 * WW - 1
    return rel.sum(-1)                                    # (S, S)


def _window_mask_types():
    """Per-window mask type: 0 none, 1 bottom-row, 2 right-col, 3 corner."""
    h = w = GRID
    s1, s2 = WH - WH // 2, WW - WW // 2
    m = np.zeros((h, w, WH, WW, WH, WW), dtype=bool)
    m[-1, :, :s1, :, s1:, :] = True
    m[-1, :, s1:, :, :s1, :] = True
    m[:, -1, :, :s2, :, s2:] = True
    m[:, -1, :, s2:, :, :s2] = True
    m = m.reshape(h * w, S, S)
    types = np.zeros(NWIN, dtype=np.int64)
    rr, cc = np.divmod(np.arange(NWIN), GRID)
    types[(rr == GRID - 1) & (cc < GRID - 1)] = 1
    types[(rr < GRID - 1) & (cc == GRID - 1)] = 2
    types[(rr == GRID - 1) & (cc == GRID - 1)] = 3
    masks = np.zeros((4, S, S), dtype=np.float32)
    masks[1] = np.where(m[GRID * (GRID - 1)], MASK_VAL, 0.0)
    masks[2] = np.where(m[GRID - 1], MASK_VAL, 0.0)
    masks[3] = np.where(m[NWIN - 1], MASK_VAL, 0.0)
    return types, masks


def _pair_types():
    types, _ = _window_mask_types()
    combos = []
    ptype = np.zeros(NPAIR, dtype=np.int64)
    for j in range(NPAIR):
        c = (int(types[2 * j]), int(types[2 * j + 1]))
        if c not in combos:
            combos.append(c)
        ptype[j] = combos.index(c)
    assert len(combos) <= 4, combos
    while len(combos) < 4:
        combos.append((0, 0))
    return ptype, combos


_PTYPE, _PCOMBOS = _pair_types()


def _tile_kxoc(wT):
    """[K, OC] -> [128, K//128, OC] with K = 128*kt + p."""
    K, OC = wT.shape
    return np.ascontiguousarray(wT.reshape(K // 128, 128, OC).transpose(1, 0, 2))


def host_prep(inputs):
    x = np.asarray(inputs["x"], dtype=np.float32)          # (B, N, S, C)
    qkv_w = np.asarray(inputs["qkv_w"], dtype=np.float32)
    qkv_b = np.asarray(inputs["qkv_b"], dtype=np.float32)
    proj_w = np.asarray(inputs["proj_w"], dtype=np.float32)
    proj_b = np.asarray(inputs["proj_b"], dtype=np.float32)
    n1g = np.asarray(inputs["norm1_g"], dtype=np.float32)
    n1b = np.asarray(inputs["norm1_b"], dtype=np.float32)
    n2g = np.asarray(inputs["norm2_g"], dtype=np.float32)
    n2b = np.asarray(inputs["norm2_b"], dtype=np.float32)
    w1 = np.asarray(inputs["mlp_w1"], dtype=np.float32)
    b1 = np.asarray(inputs["mlp_b1"], dtype=np.float32)
    w2 = np.asarray(inputs["mlp_w2"], dtype=np.float32)
    b2 = np.asarray(inputs["mlp_b2"], dtype=np.float32)
    table = np.asarray(inputs["bias_table"], dtype=np.float32)

    # fold layernorm affine into the following matmuls
    qkv_w_f = qkv_w * n1g[None, :]
    qkv_b_f = qkv_b + qkv_w @ n1b
    w1_f = w1 * n2g[None, :]
    b1_f = b1 + w1 @ n2b

    wq = qkv_w_f[0:C] * SCALE
    bq = qkv_b_f[0:C] * SCALE
    wk = qkv_w_f[C:2 * C]
    bk = qkv_b_f[C:2 * C]
    wv = qkv_w_f[2 * C:3 * C]
    bv = qkv_b_f[2 * C:3 * C]

    common = {
        "wq": _tile_kxoc(wq.T).astype(BF16),
        "wk": _tile_kxoc(wk.T).astype(BF16),
        "wv": _tile_kxoc(wv.T).astype(BF16),
        "wp": _tile_kxoc(proj_w.T).astype(BF16),
        "w1": _tile_kxoc(w1_f.T * W8SCALE).astype(FP8),
        "w2": _tile_kxoc(w2.T * W8SCALE).astype(FP8),
    }

    # exp(bias + mask) multiplicative table: eb[t_row, ptype, h, q_row]
    # t_row/q_row = 49*r + local; zero on cross-window blocks and pad rows.
    rel = _relative_position_index()
    bias_sht = table[rel].transpose(2, 0, 1)               # [h, s, t]
    _, masks = _window_mask_types()                        # [4, s, t]
    eb = np.zeros((TPP, 4, NH, VPP), dtype=np.float32)
    for pt, (tA, tB) in enumerate(_PCOMBOS):
        for r, wt in ((0, tA), (1, tB)):
            blk = np.exp(bias_sht + masks[wt][None])       # [h, s, t]
            eb[S * r:S * r + S, pt, :, S * r:S * r + S] = blk.transpose(2, 0, 1)
    common["eb"] = eb.astype(BF16)

    extra = {
        "bq_nz": bool(np.any(bq != 0.0) or np.any(bk != 0.0)),
        "bv_nz": bool(np.any(bv != 0.0)),
        "pb_nz": bool(np.any(proj_b != 0.0)),
        "b1_nz": bool(np.any(b1_f != 0.0)),
        "b2_nz": bool(np.any(b2 != 0.0)),
    }
    if extra["bq_nz"]:
        common["bq"] = np.ascontiguousarray(bq.reshape(2, 128).T).astype(np.float32)
        common["bk"] = np.ascontiguousarray(bk.reshape(2, 128).T).astype(np.float32)
    if extra["b1_nz"]:
        common["b1"] = np.ascontiguousarray(b1_f.reshape(8, 128).T).astype(np.float32)
    if extra["bv_nz"]:
        common["bvbc"] = np.tile(bv[None, :], (128, 1)).astype(np.float32)
    if extra["pb_nz"]:
        common["pbbc"] = np.tile(proj_b[None, :], (128, 1)).astype(np.float32)
    if extra["b2_nz"]:
        common["b2bc"] = np.tile(b2[None, :], (128, 1)).astype(np.float32)

    in_maps = []
    for b in range(B):
        m = dict(common)
        xp = np.zeros((NPAIR, TPP, C), dtype=BF16)
        xp[:, :VPP, :] = x[b].reshape(NPAIR, VPP, C).astype(BF16)
        m["x"] = xp.reshape(NTOKP, C)
        in_maps.append(m)
    return in_maps, extra


# --------------------------------------------------------------------------
# kernel builder
# --------------------------------------------------------------------------

def build_program(n_pairs=NPAIR, flags=None):
    flags = flags or {}
    assert n_pairs % (2 * PPC) == 0
    n_chunks = n_pairs // PPC

    nc = bacc.Bacc("TRN2", target_bir_lowering=False, debug=False)

    ext = {}
    ext["x"] = nc.dram_tensor("x", [n_pairs * TPP, C], BF16_DT, kind="ExternalInput")
    ext["out"] = nc.dram_tensor("out", [n_pairs * VPP, C], FP32, kind="ExternalOutput")
    ext["wq"] = nc.dram_tensor("wq", [128, 2, C], BF16_DT, kind="ExternalInput")
    ext["wk"] = nc.dram_tensor("wk", [128, 2, C], BF16_DT, kind="ExternalInput")
    ext["wv"] = nc.dram_tensor("wv", [128, 2, C], BF16_DT, kind="ExternalInput")
    ext["wp"] = nc.dram_tensor("wp", [128, 2, C], BF16_DT, kind="ExternalInput")
    ext["w1"] = nc.dram_tensor("w1", [128, 2, 4 * C], FP8_DT, kind="ExternalInput")
    ext["w2"] = nc.dram_tensor("w2", [128, 8, C], FP8_DT, kind="ExternalInput")
    ext["eb"] = nc.dram_tensor("eb", [TPP, 4, NH, VPP], BF16_DT, kind="ExternalInput")
    for name, shape, flg in (
        ("bq", [128, 2], "bq_nz"), ("bk", [128, 2], "bq_nz"),
        ("b1", [128, 8], "b1_nz"),
        ("bvbc", [128, C], "bv_nz"), ("pbbc", [128, C], "pb_nz"),
        ("b2bc", [128, C], "b2_nz"),
    ):
        ext[name] = (nc.dram_tensor(name, shape, FP32, kind="ExternalInput")
                     if flags.get(flg) else None)

    with tile.TileContext(nc) as tc:
        _body(tc, n_chunks, ext)

    nc.compile()
    return nc


def _body(tc, n_chunks, ext):
    nc = tc.nc
    import contextlib
    with contextlib.ExitStack() as ctx:
        const = ctx.enter_context(tc.tile_pool(name="const", bufs=1))
        cst = {}
        for name, shape, dt in (
            ("wq", [128, 2, C], BF16_DT), ("wk", [128, 2, C], BF16_DT),
            ("wv", [128, 2, C], BF16_DT), ("wp", [128, 2, C], BF16_DT),
            ("w1", [128, 2, 4 * C], FP8_DT), ("w2", [128, 8, C], FP8_DT),
            ("eb", [TPP, 4, NH, VPP], BF16_DT),
            ("bq", [128, 2], FP32), ("bk", [128, 2], FP32),
            ("b1", [128, 8], FP32),
            ("bvbc", [128, C], FP32), ("pbbc", [128, C], FP32),
            ("b2bc", [128, C], FP32),
        ):
            if ext.get(name) is None:
                cst[name] = None
                continue
            t = const.tile(shape, dt, tag=name, name=name)
            nc.sync.dma_start(out=t[:], in_=ext[name].ap())
            cst[name] = t
        ident = const.tile([128, 128], BF16_DT, tag="ident", name="ident")
        from concourse.masks import make_identity
        make_identity(nc, ident[:])
        cst["ident"] = ident

        pools = {}
        for name, bufs in (("xp", 2), ("xbp", 2), ("xnp", 2), ("xnT", 2),
                           ("attnT", 2), ("xn2T", 2), ("qkp", 2), ("qk3p", 2),
                           ("vsp", 2), ("etp", 2), ("et2p", 2), ("atp", 2),
                           ("x2p", 2), ("hp", 2), ("statp", 2)):
            pools[name] = ctx.enter_context(tc.tile_pool(name=name, bufs=bufs))
        # PSUM budget (8 banks): ps_S 4 (one bank per concurrent PE row-tile),
        # ps_wide 2, ps_tok 2 (shared ring: v/psa/proj/mlp2/transpose drains)
        for name, bufs in (("ps_wide", 2), ("ps_tok", 2), ("ps_S", 1)):
            pools[name] = ctx.enter_context(
                tc.tile_pool(name=name, bufs=bufs, space="PSUM"))
        pools["ps_tr"] = pools["ps_wide"]
        pools["_ident"] = cst["ident"]

        import os
        n_phases = int(os.environ.get("PHASES", "4"))

        def _store_dbg(ci, t):
            dst = ext["out"][ci * VPC:, :]
            dst_ap = bass.AP(tensor=dst.tensor, offset=dst.offset,
                             ap=[[C, VPP], [VPP * C, PPC], [1, C]])
            nc.sync.dma_start(out=dst_ap, in_=t[0:VPP, :, :])

        st = {}
        for cp in range(n_chunks // 2):
            a, b = 2 * cp, 2 * cp + 1
            st[a] = _phase_in(tc, a, ext, cst, pools)
            st[b] = _phase_in(tc, b, ext, cst, pools)
            if n_phases < 2:
                _store_dbg(a, st[a]["x_t"])
                _store_dbg(b, st[b]["x_t"])
                del st[a], st[b]
                continue
            _phase_attn(tc, a, cst, pools, st[a])
            _phase_attn(tc, b, cst, pools, st[b])
            if n_phases < 3:
                _store_dbg(a, st[a]["x_t"])
                _store_dbg(b, st[b]["x_t"])
                del st[a], st[b]
                continue
            _phase_proj(tc, a, cst, pools, st[a])
            _phase_proj(tc, b, cst, pools, st[b])
            if n_phases < 4:
                _store_dbg(a, st[a]["x2_t"])
                _store_dbg(b, st[b]["x2_t"])
                del st[a], st[b]
                continue
            _phase_mlp(tc, a, ext, cst, pools, st[a])
            _phase_mlp(tc, b, ext, cst, pools, st[b])
            del st[a], st[b]


def _layernorm(nc, pools, x_t, xn_t):
    """x_t [128, PPC, 256] bf16 -> xn_t bf16 ((x-mu)*rstd).

    Batched stats on DVE (bn_stats 2 pairs/instr), rstd via quake-rsqrt +
    2 Newton steps on DVE (avoids the Sqrt activation-table load), apply on
    DVE (2x/4x with bf16 operands)."""
    statp = pools["statp"]
    mv = statp.tile([128, PPC, 2], FP32, tag="mv", name="mv")
    for j in range(PPC):
        bnst = statp.tile([128, 6], FP32, tag="bnst", name="bnst")
        nc.vector.bn_stats(bnst[:], x_t[:, j, :])
        nc.vector.bn_aggr(mv[:, j, :], bnst[:])
    var = statp.tile([128, PPC], FP32, tag="var", name="var")
    rst = statp.tile([128, PPC], FP32, tag="rst", name="rst")
    tmp = statp.tile([128, PPC], FP32, tag="tmp", name="tmp")
    nc.vector.tensor_scalar(out=var[:], in0=mv[:, :, 1], scalar1=EPS,
                            scalar2=None, op0=Alu.add)
    nc.vector.tensor_scalar(out=rst[:].bitcast(INT32), in0=var[:].bitcast(INT32),
                            scalar1=1, scalar2=None, op0=Alu.logical_shift_right)
    nc.vector.tensor_scalar(out=rst[:].bitcast(INT32), in0=rst[:].bitcast(INT32),
                            scalar1=-1, scalar2=RSQRT_MAGIC,
                            op0=Alu.mult, op1=Alu.add)
    for _ in range(2):
        nc.vector.tensor_tensor(out=tmp[:], in0=rst[:], in1=rst[:], op=Alu.mult)
        nc.vector.tensor_tensor(out=tmp[:], in0=tmp[:], in1=var[:], op=Alu.mult)
        nc.vector.tensor_scalar(out=tmp[:], in0=tmp[:], scalar1=-0.5,
                                scalar2=1.5, op0=Alu.mult, op1=Alu.add)
        nc.vector.tensor_tensor(out=rst[:], in0=rst[:], in1=tmp[:], op=Alu.mult)
    for j in range(PPC):
        nc.vector.tensor_scalar(
            out=xn_t[:, j, :], in0=x_t[:, j, :],
            scalar1=mv[:, j, 0:1], scalar2=rst[:, j:j + 1],
            op0=Alu.subtract, op1=Alu.mult,
        )


def _transposes(nc, pools, src_t, dst_T, drain_eng):
    """src_t [128, PPC, 256] -> dst_T [128, 2, PPC, 128] via PE + drain.

    Two pairs share one psum tile and one drain."""
    ident = pools["_ident"]
    for j in range(0, PPC, 2):
        ps = pools["ps_tr"].tile([128, 2, 2, 128], BF16_DT, tag="wide",
                                 name="trps")
        for jj in range(2):
            for ch in range(2):
                nc.tensor.transpose(
                    ps[:, jj, ch, :],
                    src_t[:, j + jj, 128 * ch:128 * (ch + 1)], ident[:, :])
        src = ps[:].rearrange("p a b t -> p b a t")
        if hasattr(drain_eng, "tensor_copy"):
            drain_eng.tensor_copy(out=dst_T[:, :, j:j + 2, :], in_=src)
        else:
            drain_eng.copy(out=dst_T[:, :, j:j + 2, :], in_=src)


def _phase_in(tc, ci, ext, cst, pools):
    """Load x, LN1, transpose, QKV -> qT/kT/vp."""
    nc = tc.nc
    s = {}

    x_t = pools["xbp"].tile([128, PPC, C], BF16_DT, tag="xb", name="x_t")
    nc.sync.dma_start(
        out=x_t[:],
        in_=ext["x"][ci * TPC:(ci + 1) * TPC, :].rearrange("(j p) c -> p j c", p=TPP),
    )
    s["x_t"] = x_t

    xn_t = pools["xnp"].tile([128, PPC, C], BF16_DT, tag="xn", name="xn_t")
    _layernorm(nc, pools, x_t, xn_t)
    xnT = pools["xnT"].tile([128, 2, PPC, 128], BF16_DT, tag="xnT", name="xnT")
    _transposes(nc, pools, xn_t, xnT, nc.vector)

    # q, k: channel-major slabs; drain on ACT (identity/copy, bias optional)
    qT = pools["qkp"].tile([128, 2, PPC, 128], BF16_DT, tag="qT", name="qT")
    kT = pools["qkp"].tile([128, 2, PPC, 128], BF16_DT, tag="kT", name="kT")
    for s2 in range(2):
        jsl = slice(4 * s2, 4 * s2 + 4)
        for (dstT, wname, bname) in ((qT, "wq", "bq"), (kT, "wk", "bk")):
            w_sb = cst[wname]
            for octl in range(2):
                ps = pools["ps_wide"].tile([128, 4, 128], FP32, tag="wide",
                                           name="qkps")
                for kt in range(2):
                    nc.tensor.matmul(
                        ps[:],
                        lhsT=w_sb[:, kt, 128 * octl:128 * (octl + 1)],
                        rhs=xnT[:, kt, jsl, :],
                        start=(kt == 0), stop=(kt == 1),
                    )
                if cst[bname] is not None:
                    nc.scalar.activation(
                        dstT[:, octl, jsl, :], ps[:], ActF.Identity,
                        bias=cst[bname][:, octl:octl + 1])
                else:
                    nc.scalar.activation(dstT[:, octl, jsl, :], ps[:], ActF.Copy)
    s["qT"], s["kT"] = qT, kT

    # PE matmul row-tile base 96 is unsupported; realign the hm==3 head rows
    # (h = 3, 7) of q/k to partition base 0 via one small DMA each.
    qk3 = pools["qk3p"].tile([32, 2, 2, PPC, 128], BF16_DT, tag="qk3", name="qk3")
    nc.sync.dma_start(out=qk3[0:32, 0], in_=qT[96:128, :, :, :])
    nc.sync.dma_start(out=qk3[0:32, 1], in_=kT[96:128, :, :, :])
    s["qk3"] = qk3

    # v: token-major, drain straight into pair-local vp (no base shift);
    # two pairs share one psum tile and one drain
    vp = pools["vsp"].tile([128, PPC, NH, HD + 1], BF16_DT, tag="vp", name="vp")
    for j in range(0, PPC, 2):
        ps = pools["ps_tok"].tile([128, 2, C], FP32, tag="tok", name="vps")
        for jj in range(2):
            for kt in range(2):
                nc.tensor.matmul(
                    ps[:, jj, :], lhsT=xnT[:, kt, j + jj, :],
                    rhs=cst["wv"][:, kt, :],
                    start=(kt == 0), stop=(kt == 1),
                )
        nc.vector.memset(vp[:, j:j + 2, :, 0:1], 1.0)
        if cst["bvbc"] is not None:
            for jj in range(2):
                nc.vector.tensor_add(ps[:, jj, :], ps[:, jj, :], cst["bvbc"][:])
        nc.scalar.copy(
            out=vp[:, j:j + 2, :, 1:HD + 1],
            in_=ps[:].rearrange("p a (h d) -> p a h d", h=NH),
        )
    s["vp"] = vp
    return s


def _phase_attn(tc, ci, cst, pools, s):
    """S = K^T Q pair-wide, exp, *exp(bias), A = et2 @ [1|v]."""
    nc = tc.nc
    qT, kT, vp = s["qT"], s["kT"], s["vp"]
    attn_t = pools["atp"].tile([128, PPC, C], BF16_DT, tag="attn", name="attn_t")
    if ci < 2:
        # first use of each ring buffer: seed pad rows (never valid-read,
        # but must be finite/initialized for the pair transposes). 96-aligned
        # partition start; rows 96:98 are re-written by the attn drain below.
        nc.vector.memset(attn_t[96:128, :, :], 0.0)
    qk3 = s["qk3"]
    JB = 2                                      # pairs per exp batch
    for jg in range(PPC // JB):
        js = list(range(JB * jg, JB * jg + JB))
        # et holds the batch: [t, h, jj, q]
        et = pools["etp"].tile([128, NH, JB, VPP], BF16_DT, tag="et", name="et")
        # all 8 heads x JB pairs in one psum tile: [h4(bank), hh, jj, col];
        # h4-stride = 2KB so each concurrent PE row-tile owns its own bank
        # (same-bank writes h and h+4 run on the same row-tile => serialized).
        pss = pools["ps_S"].tile([128, 4, 2, JB, 128], FP32, tag="S", name="pss")
        for jj, j in enumerate(js):
            for h in range(NH):
                hh, hm = h // 4, h % 4
                octl = h // 4
                if hm == 3:
                    lhsT = qk3[0:32, 1, octl, j, :]
                    rhs = qk3[0:32, 0, octl, j, 0:VPP]
                    base = 0
                else:
                    base = 32 * hm
                    lhsT = kT[base:base + 32, octl, j, :]
                    rhs = qT[base:base + 32, octl, j, 0:VPP]
                nc.tensor.matmul(
                    pss[:, hm, hh, jj, 0:VPP], lhsT=lhsT, rhs=rhs,
                    start=True, stop=True,
                    tile_position=(base, 0),
                )
        e = et[:]
        et_ap = bass.AP(tensor=e.tensor, offset=e.offset,
                        ap=[e.ap[0], [JB * VPP, 4], [4 * JB * VPP, 2],
                            [VPP, JB], [1, VPP]])
        nc.scalar.activation(et_ap, pss[:, :, :, :, 0:VPP], ActF.Exp)
        # eb multiply, batched over runs of equal pair-type
        et2 = pools["et2p"].tile([128, NH, JB, VPP], BF16_DT, tag="et2",
                                 name="et2")
        runs = []
        for jj, j in enumerate(js):
            pt = int(_PTYPE[ci * PPC + j])
            if runs and runs[-1][0] == pt:
                runs[-1][2] = jj + 1
            else:
                runs.append([pt, jj, jj + 1])
        for pt, j0, j1 in runs:
            e3 = cst["eb"][:, pt, :, :]
            eb_b = bass.AP(tensor=e3.tensor, offset=e3.offset,
                           ap=[e3.ap[0], e3.ap[1], [0, j1 - j0], e3.ap[2]])
            nc.vector.tensor_tensor(
                out=et2[:, :, j0:j1, :], in0=et[:, :, j0:j1, :],
                in1=eb_b, op=Alu.mult,
            )
        for jj, j in enumerate(js):
            psa = pools["ps_tok"].tile([VPP, NH, HD + 1], FP32, tag="tok",
                                       name="psa")
            for h in range(NH):
                nc.tensor.matmul(
                    psa[:, h, :], lhsT=et2[:, h, jj, :], rhs=vp[:, j, h, :],
                    start=True, stop=True,
                )
            rec = pools["statp"].tile([VPP, NH], FP32, tag="rec", name="rec")
            nc.vector.tensor_scalar_max(out=rec[:], in0=psa[:, :, 0],
                                        scalar1=1e-30)
            nc.vector.reciprocal(rec[:], rec[:])
            rec_b = bass.AP(tensor=rec[:].tensor, offset=rec[:].offset,
                            ap=list(rec[:].ap) + [[0, HD]])
            nc.vector.tensor_mul(
                attn_t[0:VPP, j, :].rearrange("p (h d) -> p h d", h=NH),
                psa[:, :, 1:HD + 1], rec_b,
            )
    s["attn_t"] = attn_t


def _phase_proj(tc, ci, cst, pools, s):
    """attn transpose, proj + resid1, LN2, transpose -> xn2T (fp8)."""
    nc = tc.nc
    attnT = pools["attnT"].tile([128, 2, PPC, 128], BF16_DT, tag="attnT",
                                name="attnT")
    _transposes(nc, pools, s["attn_t"], attnT, nc.vector)

    x2_t = pools["x2p"].tile([128, PPC, C], BF16_DT, tag="x2", name="x2_t")
    for j in range(0, PPC, 2):
        ps = pools["ps_tok"].tile([128, 2, C], FP32, tag="tok", name="prps")
        for jj in range(2):
            for kt in range(2):
                nc.tensor.matmul(
                    ps[:, jj, :], lhsT=attnT[:, kt, j + jj, :],
                    rhs=cst["wp"][:, kt, :],
                    start=(kt == 0), stop=(kt == 1),
                )
            if cst["pbbc"] is not None:
                nc.vector.tensor_add(ps[:, jj, :], ps[:, jj, :], cst["pbbc"][:])
        nc.vector.scalar_tensor_tensor(
            out=x2_t[:, j:j + 2, :], in0=ps[:], scalar=1.0,
            in1=s["x_t"][:, j:j + 2, :], op0=Alu.mult, op1=Alu.add,
        )
    s["x2_t"] = x2_t

    xn2_t = pools["xnp"].tile([128, PPC, C], BF16_DT, tag="xn", name="xn2_t")
    _layernorm(nc, pools, x2_t, xn2_t)
    xn2T = pools["xn2T"].tile([128, 2, PPC, 128], FP8_DT, tag="xn2T", name="xn2T")
    _transposes(nc, pools, xn2_t, xn2T, nc.scalar)
    s["xn2T"] = xn2T


def _phase_mlp(tc, ci, ext, cst, pools, s):
    """MLP (fp8 DoubleRow) + resid2, store."""
    nc = tc.nc
    xn2T = s["xn2T"]
    hT = pools["hp"].tile([128, 8, PPC, 128], FP8_DT, tag="hT", name="hT")
    if ci < 2:
        nc.vector.memset(hT[:, :, :, VPP:128], 0.0)
    for s2 in range(2):
        jsl = slice(4 * s2, 4 * s2 + 4)
        for m in range(8):
            ps = pools["ps_wide"].tile([128, 4, 128], FP32, tag="wide",
                                       name="m1ps")
            nc.tensor.matmul(
                ps[:], lhsT=cst["w1"][:, :, 128 * m:128 * (m + 1)],
                rhs=xn2T[:, :, jsl, :],
                start=True, stop=True,
                perf_mode=mybir.MatmulPerfMode.DoubleRow,
            )
            bias = (cst["b1"][:, m:m + 1] if cst["b1"] is not None else 0.0)
            import os
            gelu_f = ActF.Identity if os.environ.get("GELU_ID") else ActF.Gelu
            nc.scalar.activation(
                hT[:, m, jsl, 0:VPP], ps[:, :, 0:VPP],
                gelu_f, bias=bias, scale=1.0 / W8SCALE,
            )

    out_t = pools["xp"].tile([128, PPC, C], FP32, tag="xo", name="out_t")
    for j in range(0, PPC, 2):
        ps = pools["ps_tok"].tile([128, 2, C], FP32, tag="tok", name="m2ps")
        for jj in range(2):
            for k2 in range(4):
                nc.tensor.matmul(
                    ps[:, jj, :], lhsT=hT[:, 2 * k2:2 * k2 + 2, j + jj, :],
                    rhs=cst["w2"][:, 2 * k2:2 * k2 + 2, :],
                    start=(k2 == 0), stop=(k2 == 3),
                    perf_mode=mybir.MatmulPerfMode.DoubleRow,
                )
            if cst["b2bc"] is not None:
                nc.vector.tensor_add(ps[:, jj, :], ps[:, jj, :], cst["b2bc"][:])
        nc.vector.scalar_tensor_tensor(
            out=out_t[:, j:j + 2, :], in0=ps[:], scalar=1.0 / W8SCALE,
            in1=s["x2_t"][:, j:j + 2, :], op0=Alu.mult, op1=Alu.add,
        )

    # compact store: pair (ci*PPC + j) valid rows 0:VPP
    dst = ext["out"][ci * VPC:, :]
    dst_ap = bass.AP(
        tensor=dst.tensor, offset=dst.offset,
        ap=[[C, VPP], [VPP * C, PPC], [1, C]],
    )
    nc.sync.dma_start(out=dst_ap, in_=out_t[0:VPP, :, :])


# --------------------------------------------------------------------------
# entry point
# --------------------------------------------------------------------------

_CACHE = {}


def _get_program(key_flags):
    if key_flags not in _CACHE:
        _CACHE[key_flags] = build_program(NPAIR, flags=dict(key_flags))
    return _CACHE[key_flags]


def kernel(**inputs):
    in_maps, extra = host_prep(inputs)
    nc = _get_program(tuple(sorted(extra.items())))
    res = run_bass_kernel_spmd(nc, in_maps, core_ids=list(range(B)))
    out = np.stack([res.results[i]["out"] for i in range(B)], axis=0)
    return out.reshape(B, NWIN, S, C).astype(np.float32)


# revision 50
# speedup vs baseline: 2.2270x; 1.0151x over previous
"""Swin-style windowed-attention block on 8 TRN2 NeuronCores (data-parallel over batch).

v2: compact-pair layout (both windows' 49 valid tokens at rows 0:98 of a 128-row
pair tile, zero pad rows 98:128). Pair-wide attention with post-exp multiplicative
bias (exp(S+b) = exp(S)*exp(b) with a host-precomputed exp(bias) table that also
zeroes cross-window blocks and pad rows), no augmented-K matmuls, no head-realign
or v-shift DMAs. Activation-table thrash removed (DVE Newton rsqrt; chunk-pair
interleaving batches exp/gelu). Elementwise spread across ACT / DVE / Pool.
"""

import sys

sys.path.insert(0, "/opt/trn_rl_repo")

import numpy as np
import ml_dtypes

import concourse.bass as bass
import concourse.bacc as bacc
import concourse.tile as tile
import concourse.mybir as mybir
from concourse.bass_utils import run_bass_kernel_spmd

BF16 = ml_dtypes.bfloat16
FP8 = ml_dtypes.float8_e4m3
FP32 = mybir.dt.float32
BF16_DT = mybir.dt.bfloat16
FP8_DT = mybir.dt.float8e4
INT32 = mybir.dt.int32
W8SCALE = 64.0

# ---- static geometry ----
WH, WW = 7, 7
S = 49                     # valid tokens per window
C = 256                    # channels
NH = 8                     # heads
HD = 32                    # head dim
NWIN = 256                 # windows per batch image
B = 8                      # batch == number of cores
GRID = 16                  # 16x16 window grid
SCALE = HD ** -0.5
EPS = 1e-5
MASK_VAL = -30000.0

NPAIR = NWIN // 2          # 128 window pairs per core
PPC = 8                    # pairs per chunk
NCHUNK = NPAIR // PPC      # 16 chunks
TPP = 128                  # tile rows per pair (98 valid + 30 zero pad)
VPP = 2 * S                # 98 valid tokens per pair
TPC = PPC * TPP            # 1024 padded tokens per chunk
VPC = PPC * VPP            # 784 valid tokens per chunk
NTOK = NWIN * S            # 12544 valid tokens per core
NTOKP = NPAIR * TPP        # 16384 padded tokens per core

RSQRT_MAGIC = 0x5F3759DF

ActF = mybir.ActivationFunctionType
Alu = mybir.AluOpType


# --------------------------------------------------------------------------
# host-side preparation
# --------------------------------------------------------------------------

def _relative_position_index():
    ch, cw = np.arange(WH), np.arange(WW)
    coords = np.stack(np.meshgrid(ch, cw, indexing="ij")).reshape(2, -1)
    rel = coords[:, :, None] - coords[:, None, :]
    rel = rel.transpose(1, 2, 0).astype(np.int64)
    rel[..., 0] += WH - 1
    rel[..., 1] += WW - 1
    rel[..., 0] *= 2 * WW - 1
    return rel.sum(-1)                                    # (S, S)


def _window_mask_types():
    """Per-window mask type: 0 none, 1 bottom-row, 2 right-col, 3 corner."""
    h = w = GRID
    s1, s2 = WH - WH // 2, WW - WW // 2
    m = np.zeros((h, w, WH, WW, WH, WW), dtype=bool)
    m[-1, :, :s1, :, s1:, :] = True
    m[-1, :, s1:, :, :s1, :] = True
    m[:, -1, :, :s2, :, s2:] = True
    m[:, -1, :, s2:, :, :s2] = True
    m = m.reshape(h * w, S, S)
    types = np.zeros(NWIN, dtype=np.int64)
    rr, cc = np.divmod(np.arange(NWIN), GRID)
    types[(rr == GRID - 1) & (cc < GRID - 1)] = 1
    types[(rr < GRID - 1) & (cc == GRID - 1)] = 2
    types[(rr == GRID - 1) & (cc == GRID - 1)] = 3
    masks = np.zeros((4, S, S), dtype=np.float32)
    masks[1] = np.where(m[GRID * (GRID - 1)], MASK_VAL, 0.0)
    masks[2] = np.where(m[GRID - 1], MASK_VAL, 0.0)
    masks[3] = np.where(m[NWIN - 1], MASK_VAL, 0.0)
    return types, masks


def _pair_types():
    types, _ = _window_mask_types()
    combos = []
    ptype = np.zeros(NPAIR, dtype=np.int64)
    for j in range(NPAIR):
        c = (int(types[2 * j]), int(types[2 * j + 1]))
        if c not in combos:
            combos.append(c)
        ptype[j] = combos.index(c)
    assert len(combos) <= 4, combos
    while len(combos) < 4:
        combos.append((0, 0))
    return ptype, combos


_PTYPE, _PCOMBOS = _pair_types()


def _tile_kxoc(wT):
    """[K, OC] -> [128, K//128, OC] with K = 128*kt + p."""
    K, OC = wT.shape
    return np.ascontiguousarray(wT.reshape(K // 128, 128, OC).transpose(1, 0, 2))


def host_prep(inputs):
    x = np.asarray(inputs["x"], dtype=np.float32)          # (B, N, S, C)
    qkv_w = np.asarray(inputs["qkv_w"], dtype=np.float32)
    qkv_b = np.asarray(inputs["qkv_b"], dtype=np.float32)
    proj_w = np.asarray(inputs["proj_w"], dtype=np.float32)
    proj_b = np.asarray(inputs["proj_b"], dtype=np.float32)
    n1g = np.asarray(inputs["norm1_g"], dtype=np.float32)
    n1b = np.asarray(inputs["norm1_b"], dtype=np.float32)
    n2g = np.asarray(inputs["norm2_g"], dtype=np.float32)
    n2b = np.asarray(inputs["norm2_b"], dtype=np.float32)
    w1 = np.asarray(inputs["mlp_w1"], dtype=np.float32)
    b1 = np.asarray(inputs["mlp_b1"], dtype=np.float32)
    w2 = np.asarray(inputs["mlp_w2"], dtype=np.float32)
    b2 = np.asarray(inputs["mlp_b2"], dtype=np.float32)
    table = np.asarray(inputs["bias_table"], dtype=np.float32)

    # fold layernorm affine into the following matmuls
    qkv_w_f = qkv_w * n1g[None, :]
    qkv_b_f = qkv_b + qkv_w @ n1b
    w1_f = w1 * n2g[None, :]
    b1_f = b1 + w1 @ n2b

    wq = qkv_w_f[0:C] * SCALE
    bq = qkv_b_f[0:C] * SCALE
    wk = qkv_w_f[C:2 * C]
    bk = qkv_b_f[C:2 * C]
    wv = qkv_w_f[2 * C:3 * C]
    bv = qkv_b_f[2 * C:3 * C]

    common = {
        "wq": _tile_kxoc(wq.T).astype(BF16),
        "wk": _tile_kxoc(wk.T).astype(BF16),
        "wv": _tile_kxoc(wv.T).astype(BF16),
        "wp": _tile_kxoc(proj_w.T).astype(BF16),
        "w1": _tile_kxoc(w1_f.T * W8SCALE).astype(FP8),
        "w2": _tile_kxoc(w2.T * W8SCALE).astype(FP8),
    }

    # exp(bias + mask) multiplicative table: eb[t_row, ptype, h, q_row]
    # t_row/q_row = 49*r + local; zero on cross-window blocks and pad rows.
    rel = _relative_position_index()
    bias_sht = table[rel].transpose(2, 0, 1)               # [h, s, t]
    _, masks = _window_mask_types()                        # [4, s, t]
    eb = np.zeros((TPP, 4, NH, VPP), dtype=np.float32)
    for pt, (tA, tB) in enumerate(_PCOMBOS):
        for r, wt in ((0, tA), (1, tB)):
            blk = np.exp(bias_sht + masks[wt][None])       # [h, s, t]
            eb[S * r:S * r + S, pt, :, S * r:S * r + S] = blk.transpose(2, 0, 1)
    # h-axis reordered to (h4, hh)-major to match the S-psum bank layout
    perm = [4 * hh + h4 for h4 in range(4) for hh in range(2)]
    common["eb"] = eb[:, :, perm, :].astype(BF16)

    extra = {
        "bq_nz": bool(np.any(bq != 0.0) or np.any(bk != 0.0)),
        "bv_nz": bool(np.any(bv != 0.0)),
        "pb_nz": bool(np.any(proj_b != 0.0)),
        "b1_nz": bool(np.any(b1_f != 0.0)),
        "b2_nz": bool(np.any(b2 != 0.0)),
    }
    if extra["bq_nz"]:
        common["bq"] = np.ascontiguousarray(bq.reshape(2, 128).T).astype(np.float32)
        common["bk"] = np.ascontiguousarray(bk.reshape(2, 128).T).astype(np.float32)
    if extra["b1_nz"]:
        common["b1"] = np.ascontiguousarray(b1_f.reshape(8, 128).T).astype(np.float32)
    if extra["bv_nz"]:
        common["bvbc"] = np.tile(bv[None, :], (128, 1)).astype(np.float32)
    if extra["pb_nz"]:
        common["pbbc"] = np.tile(proj_b[None, :], (128, 1)).astype(np.float32)
    if extra["b2_nz"]:
        common["b2bc"] = np.tile(b2[None, :], (128, 1)).astype(np.float32)

    in_maps = []
    for b in range(B):
        m = dict(common)
        xp = np.zeros((NPAIR, TPP, C), dtype=BF16)
        xp[:, :VPP, :] = x[b].reshape(NPAIR, VPP, C).astype(BF16)
        m["x"] = xp.reshape(NTOKP, C)
        in_maps.append(m)
    return in_maps, extra


# --------------------------------------------------------------------------
# kernel builder
# --------------------------------------------------------------------------

def build_program(n_pairs=NPAIR, flags=None):
    flags = flags or {}
    assert n_pairs % (2 * PPC) == 0
    n_chunks = n_pairs // PPC

    nc = bacc.Bacc("TRN2", target_bir_lowering=False, debug=False)

    ext = {}
    ext["x"] = nc.dram_tensor("x", [n_pairs * TPP, C], BF16_DT, kind="ExternalInput")
    ext["out"] = nc.dram_tensor("out", [n_pairs * VPP, C], FP32, kind="ExternalOutput")
    ext["wq"] = nc.dram_tensor("wq", [128, 2, C], BF16_DT, kind="ExternalInput")
    ext["wk"] = nc.dram_tensor("wk", [128, 2, C], BF16_DT, kind="ExternalInput")
    ext["wv"] = nc.dram_tensor("wv", [128, 2, C], BF16_DT, kind="ExternalInput")
    ext["wp"] = nc.dram_tensor("wp", [128, 2, C], BF16_DT, kind="ExternalInput")
    ext["w1"] = nc.dram_tensor("w1", [128, 2, 4 * C], FP8_DT, kind="ExternalInput")
    ext["w2"] = nc.dram_tensor("w2", [128, 8, C], FP8_DT, kind="ExternalInput")
    ext["eb"] = nc.dram_tensor("eb", [TPP, 4, NH, VPP], BF16_DT, kind="ExternalInput")
    for name, shape, flg in (
        ("bq", [128, 2], "bq_nz"), ("bk", [128, 2], "bq_nz"),
        ("b1", [128, 8], "b1_nz"),
        ("bvbc", [128, C], "bv_nz"), ("pbbc", [128, C], "pb_nz"),
        ("b2bc", [128, C], "b2_nz"),
    ):
        ext[name] = (nc.dram_tensor(name, shape, FP32, kind="ExternalInput")
                     if flags.get(flg) else None)

    with tile.TileContext(nc) as tc:
        _body(tc, n_chunks, ext)

    nc.compile()
    return nc


def _body(tc, n_chunks, ext):
    nc = tc.nc
    import contextlib
    with contextlib.ExitStack() as ctx:
        const = ctx.enter_context(tc.tile_pool(name="const", bufs=1))
        cst = {}
        for name, shape, dt in (
            ("wq", [128, 2, C], BF16_DT), ("wk", [128, 2, C], BF16_DT),
            ("wv", [128, 2, C], BF16_DT), ("wp", [128, 2, C], BF16_DT),
            ("w1", [128, 2, 4 * C], FP8_DT), ("w2", [128, 8, C], FP8_DT),
            ("eb", [TPP, 4, NH, VPP], BF16_DT),
            ("bq", [128, 2], FP32), ("bk", [128, 2], FP32),
            ("b1", [128, 8], FP32),
            ("bvbc", [128, C], FP32), ("pbbc", [128, C], FP32),
            ("b2bc", [128, C], FP32),
        ):
            if ext.get(name) is None:
                cst[name] = None
                continue
            t = const.tile(shape, dt, tag=name, name=name)
            nc.sync.dma_start(out=t[:], in_=ext[name].ap())
            cst[name] = t
        ident = const.tile([128, 128], BF16_DT, tag="ident", name="ident")
        from concourse.masks import make_identity
        make_identity(nc, ident[:])
        cst["ident"] = ident

        pools = {}
        for name, bufs in (("xp", 2), ("xbp", 2), ("xnp", 2), ("xnT", 2),
                           ("attnT", 2), ("xn2T", 2), ("qkp", 2), ("qk3p", 2),
                           ("vsp", 2), ("etp", 2), ("et2p", 2), ("atp", 2),
                           ("x2p", 2), ("hp", 2), ("statp", 2)):
            pools[name] = ctx.enter_context(tc.tile_pool(name=name, bufs=bufs))
        # PSUM budget (8 banks): ps_S 4 (one bank per concurrent PE row-tile),
        # ps_wide 2, ps_tok 2 (shared ring: v/psa/proj/mlp2/transpose drains)
        for name, bufs in (("ps_wide", 2), ("ps_tok", 2), ("ps_S", 1)):
            pools[name] = ctx.enter_context(
                tc.tile_pool(name=name, bufs=bufs, space="PSUM"))
        pools["ps_tr"] = pools["ps_wide"]
        pools["_ident"] = cst["ident"]

        import os
        n_phases = int(os.environ.get("PHASES", "4"))

        def _store_dbg(ci, t):
            dst = ext["out"][ci * VPC:, :]
            dst_ap = bass.AP(tensor=dst.tensor, offset=dst.offset,
                             ap=[[C, VPP], [VPP * C, PPC], [1, C]])
            nc.sync.dma_start(out=dst_ap, in_=t[0:VPP, :, :])

        st = {}
        for cp in range(n_chunks // 2):
            a, b = 2 * cp, 2 * cp + 1
            st[a] = _phase_in(tc, a, ext, cst, pools)
            st[b] = _phase_in(tc, b, ext, cst, pools)
            if n_phases < 2:
                _store_dbg(a, st[a]["x_t"])
                _store_dbg(b, st[b]["x_t"])
                del st[a], st[b]
                continue
            _phase_attn(tc, a, cst, pools, st[a])
            _phase_attn(tc, b, cst, pools, st[b])
            if n_phases < 3:
                _store_dbg(a, st[a]["x_t"])
                _store_dbg(b, st[b]["x_t"])
                del st[a], st[b]
                continue
            _phase_proj(tc, a, cst, pools, st[a])
            _phase_proj(tc, b, cst, pools, st[b])
            if n_phases < 4:
                _store_dbg(a, st[a]["x2_t"])
                _store_dbg(b, st[b]["x2_t"])
                del st[a], st[b]
                continue
            _phase_mlp(tc, a, ext, cst, pools, st[a])
            _phase_mlp(tc, b, ext, cst, pools, st[b])
            del st[a], st[b]


def _layernorm(nc, pools, x_t, xn_t):
    """x_t [128, PPC, 256] bf16 -> xn_t bf16 ((x-mu)*rstd).

    Batched stats on DVE (bn_stats 2 pairs/instr), rstd via quake-rsqrt +
    2 Newton steps on DVE (avoids the Sqrt activation-table load), apply on
    DVE (2x/4x with bf16 operands)."""
    statp = pools["statp"]
    mv = statp.tile([128, PPC, 2], FP32, tag="mv", name="mv")
    for j in range(PPC):
        bnst = statp.tile([128, 6], FP32, tag="bnst", name="bnst")
        nc.vector.bn_stats(bnst[:], x_t[:, j, :])
        nc.vector.bn_aggr(mv[:, j, :], bnst[:])
    var = statp.tile([128, PPC], FP32, tag="var", name="var")
    rst = statp.tile([128, PPC], FP32, tag="rst", name="rst")
    tmp = statp.tile([128, PPC], FP32, tag="tmp", name="tmp")
    nc.vector.tensor_scalar(out=var[:], in0=mv[:, :, 1], scalar1=EPS,
                            scalar2=None, op0=Alu.add)
    nc.vector.tensor_scalar(out=rst[:].bitcast(INT32), in0=var[:].bitcast(INT32),
                            scalar1=1, scalar2=None, op0=Alu.logical_shift_right)
    nc.vector.tensor_scalar(out=rst[:].bitcast(INT32), in0=rst[:].bitcast(INT32),
                            scalar1=-1, scalar2=RSQRT_MAGIC,
                            op0=Alu.mult, op1=Alu.add)
    for _ in range(1):
        nc.vector.tensor_tensor(out=tmp[:], in0=rst[:], in1=rst[:], op=Alu.mult)
        nc.vector.tensor_tensor(out=tmp[:], in0=tmp[:], in1=var[:], op=Alu.mult)
        nc.vector.tensor_scalar(out=tmp[:], in0=tmp[:], scalar1=-0.5,
                                scalar2=1.5, op0=Alu.mult, op1=Alu.add)
        nc.vector.tensor_tensor(out=rst[:], in0=rst[:], in1=tmp[:], op=Alu.mult)
    for j in range(PPC):
        nc.vector.tensor_scalar(
            out=xn_t[:, j, :], in0=x_t[:, j, :],
            scalar1=mv[:, j, 0:1], scalar2=rst[:, j:j + 1],
            op0=Alu.subtract, op1=Alu.mult,
        )


def _transposes(nc, pools, src_t, dst_T, drain_eng):
    """src_t [128, PPC, 256] -> dst_T [128, 2, PPC, 128] via PE + drain.

    Two pairs share one psum tile and one drain."""
    ident = pools["_ident"]
    for j in range(0, PPC, 2):
        ps = pools["ps_tr"].tile([128, 2, 2, 128], BF16_DT, tag="wide",
                                 name="trps")
        for jj in range(2):
            for ch in range(2):
                nc.tensor.transpose(
                    ps[:, jj, ch, :],
                    src_t[:, j + jj, 128 * ch:128 * (ch + 1)], ident[:, :])
        src = ps[:].rearrange("p a b t -> p b a t")
        if hasattr(drain_eng, "tensor_copy"):
            drain_eng.tensor_copy(out=dst_T[:, :, j:j + 2, :], in_=src)
        else:
            drain_eng.copy(out=dst_T[:, :, j:j + 2, :], in_=src)


def _phase_in(tc, ci, ext, cst, pools):
    """Load x, LN1, transpose, QKV -> qT/kT/vp."""
    nc = tc.nc
    s = {}

    x_t = pools["xbp"].tile([128, PPC, C], BF16_DT, tag="xb", name="x_t")
    nc.sync.dma_start(
        out=x_t[:],
        in_=ext["x"][ci * TPC:(ci + 1) * TPC, :].rearrange("(j p) c -> p j c", p=TPP),
    )
    s["x_t"] = x_t

    xn_t = pools["xnp"].tile([128, PPC, C], BF16_DT, tag="xn", name="xn_t")
    _layernorm(nc, pools, x_t, xn_t)
    xnT = pools["xnT"].tile([128, 2, PPC, 128], BF16_DT, tag="xnT", name="xnT")
    _transposes(nc, pools, xn_t, xnT, nc.vector)

    # q, k: channel-major slabs; drain on ACT (identity/copy, bias optional)
    qT = pools["qkp"].tile([128, 2, PPC, 128], BF16_DT, tag="qT", name="qT")
    kT = pools["qkp"].tile([128, 2, PPC, 128], BF16_DT, tag="kT", name="kT")
    for s2 in range(2):
        jsl = slice(4 * s2, 4 * s2 + 4)
        for (dstT, wname, bname) in ((qT, "wq", "bq"), (kT, "wk", "bk")):
            w_sb = cst[wname]
            for octl in range(2):
                ps = pools["ps_wide"].tile([128, 4, 128], FP32, tag="wide",
                                           name="qkps")
                for kt in range(2):
                    nc.tensor.matmul(
                        ps[:],
                        lhsT=w_sb[:, kt, 128 * octl:128 * (octl + 1)],
                        rhs=xnT[:, kt, jsl, :],
                        start=(kt == 0), stop=(kt == 1),
                    )
                if cst[bname] is not None:
                    nc.scalar.activation(
                        dstT[:, octl, jsl, :], ps[:], ActF.Identity,
                        bias=cst[bname][:, octl:octl + 1])
                else:
                    nc.scalar.activation(dstT[:, octl, jsl, :], ps[:], ActF.Copy)
    s["qT"], s["kT"] = qT, kT

    # PE matmul row-tile base 96 is unsupported; realign the hm==3 head rows
    # (h = 3, 7) of q/k to partition base 0 via one small DMA each.
    qk3 = pools["qk3p"].tile([32, 2, 2, PPC, 128], BF16_DT, tag="qk3", name="qk3")
    nc.sync.dma_start(out=qk3[0:32, 0], in_=qT[96:128, :, :, :])
    nc.sync.dma_start(out=qk3[0:32, 1], in_=kT[96:128, :, :, :])
    s["qk3"] = qk3

    # v: token-major, drain straight into pair-local vp (no base shift);
    # two pairs share one psum tile and one drain
    vp = pools["vsp"].tile([128, PPC, NH, HD + 1], BF16_DT, tag="vp", name="vp")
    for j in range(0, PPC, 2):
        ps = pools["ps_tok"].tile([128, 2, C], FP32, tag="tok", name="vps")
        for jj in range(2):
            for kt in range(2):
                nc.tensor.matmul(
                    ps[:, jj, :], lhsT=xnT[:, kt, j + jj, :],
                    rhs=cst["wv"][:, kt, :],
                    start=(kt == 0), stop=(kt == 1),
                )
        nc.vector.memset(vp[:, j:j + 2, :, 0:1], 1.0)
        if cst["bvbc"] is not None:
            for jj in range(2):
                nc.vector.tensor_add(ps[:, jj, :], ps[:, jj, :], cst["bvbc"][:])
        nc.scalar.copy(
            out=vp[:, j:j + 2, :, 1:HD + 1],
            in_=ps[:].rearrange("p a (h d) -> p a h d", h=NH),
        )
    s["vp"] = vp
    return s


def _phase_attn(tc, ci, cst, pools, s):
    """S = K^T Q pair-wide, exp, *exp(bias), A = et2 @ [1|v]."""
    nc = tc.nc
    qT, kT, vp = s["qT"], s["kT"], s["vp"]
    attn_t = pools["atp"].tile([128, PPC, C], BF16_DT, tag="attn", name="attn_t")
    if ci < 2:
        # first use of each ring buffer: seed pad rows (never valid-read,
        # but must be finite/initialized for the pair transposes). 96-aligned
        # partition start; rows 96:98 are re-written by the attn drain below.
        nc.vector.memset(attn_t[96:128, :, :], 0.0)
    qk3 = s["qk3"]
    JB = 2                                      # pairs per exp batch
    for jg in range(PPC // JB):
        js = list(range(JB * jg, JB * jg + JB))
        # et holds the batch, h-axis (h4, hh)-major to match the psum layout
        et = pools["etp"].tile([128, NH, JB, VPP], BF16_DT, tag="et", name="et")
        # all 8 heads x JB pairs in one psum tile: [h4(bank), hh, jj, col];
        # h4-stride = 2KB so each concurrent PE row-tile owns its own bank
        # (same-bank writes h and h+4 run on the same row-tile => serialized).
        pss = pools["ps_S"].tile([128, 4, 2, JB, 128], FP32, tag="S", name="pss")
        for jj, j in enumerate(js):
            for h in range(NH):
                hh, hm = h // 4, h % 4
                octl = h // 4
                if hm == 3:
                    lhsT = qk3[0:32, 1, octl, j, :]
                    rhs = qk3[0:32, 0, octl, j, 0:VPP]
                    base = 0
                else:
                    base = 32 * hm
                    lhsT = kT[base:base + 32, octl, j, :]
                    rhs = qT[base:base + 32, octl, j, 0:VPP]
                nc.tensor.matmul(
                    pss[:, hm, hh, jj, 0:VPP], lhsT=lhsT, rhs=rhs,
                    start=True, stop=True,
                    tile_position=(base, 0),
                )
        nc.scalar.activation(
            et[:].rearrange("p (a b) j q -> p a b j q", a=4),
            pss[:, :, :, :, 0:VPP], ActF.Exp)
        # eb multiply, batched over runs of equal pair-type
        et2 = pools["et2p"].tile([128, NH, JB, VPP], BF16_DT, tag="et2",
                                 name="et2")
        runs = []
        for jj, j in enumerate(js):
            pt = int(_PTYPE[ci * PPC + j])
            if runs and runs[-1][0] == pt:
                runs[-1][2] = jj + 1
            else:
                runs.append([pt, jj, jj + 1])
        for pt, j0, j1 in runs:
            e3 = cst["eb"][:, pt, :, :]
            eb_b = bass.AP(tensor=e3.tensor, offset=e3.offset,
                           ap=[e3.ap[0], e3.ap[1], [0, j1 - j0], e3.ap[2]])
            nc.vector.tensor_tensor(
                out=et2[:, :, j0:j1, :], in0=et[:, :, j0:j1, :],
                in1=eb_b, op=Alu.mult,
            )
        for jj, j in enumerate(js):
            psa = pools["ps_tok"].tile([VPP, NH, HD + 1], FP32, tag="tok",
                                       name="psa")
            for h in range(NH):
                nc.tensor.matmul(
                    psa[:, h, :], lhsT=et2[:, 2 * (h % 4) + h // 4, jj, :],
                    rhs=vp[:, j, h, :],
                    start=True, stop=True,
                )
            rec = pools["statp"].tile([VPP, NH], FP32, tag="rec", name="rec")
            nc.vector.tensor_scalar_max(out=rec[:], in0=psa[:, :, 0],
                                        scalar1=1e-30)
            nc.vector.reciprocal(rec[:], rec[:])
            rec_b = bass.AP(tensor=rec[:].tensor, offset=rec[:].offset,
                            ap=list(rec[:].ap) + [[0, HD]])
            nc.vector.tensor_mul(
                attn_t[0:VPP, j, :].rearrange("p (h d) -> p h d", h=NH),
                psa[:, :, 1:HD + 1], rec_b,
            )
    s["attn_t"] = attn_t


def _phase_proj(tc, ci, cst, pools, s):
    """attn transpose, proj + resid1, LN2, transpose -> xn2T (fp8)."""
    nc = tc.nc
    attnT = pools["attnT"].tile([128, 2, PPC, 128], BF16_DT, tag="attnT",
                                name="attnT")
    _transposes(nc, pools, s["attn_t"], attnT, nc.vector)

    x2_t = pools["x2p"].tile([128, PPC, C], BF16_DT, tag="x2", name="x2_t")
    for j in range(0, PPC, 2):
        ps = pools["ps_tok"].tile([128, 2, C], FP32, tag="tok", name="prps")
        for jj in range(2):
            for kt in range(2):
                nc.tensor.matmul(
                    ps[:, jj, :], lhsT=attnT[:, kt, j + jj, :],
                    rhs=cst["wp"][:, kt, :],
                    start=(kt == 0), stop=(kt == 1),
                )
            if cst["pbbc"] is not None:
                nc.vector.tensor_add(ps[:, jj, :], ps[:, jj, :], cst["pbbc"][:])
        nc.vector.scalar_tensor_tensor(
            out=x2_t[:, j:j + 2, :], in0=ps[:], scalar=1.0,
            in1=s["x_t"][:, j:j + 2, :], op0=Alu.mult, op1=Alu.add,
        )
    s["x2_t"] = x2_t

    xn2_t = pools["xnp"].tile([128, PPC, C], BF16_DT, tag="xn", name="xn2_t")
    _layernorm(nc, pools, x2_t, xn2_t)
    xn2T = pools["xn2T"].tile([128, 2, PPC, 128], FP8_DT, tag="xn2T", name="xn2T")
    _transposes(nc, pools, xn2_t, xn2T, nc.scalar)
    s["xn2T"] = xn2T


def _phase_mlp(tc, ci, ext, cst, pools, s):
    """MLP (fp8 DoubleRow) + resid2, store."""
    nc = tc.nc
    xn2T = s["xn2T"]
    hT = pools["hp"].tile([128, 8, PPC, 128], FP8_DT, tag="hT", name="hT")
    if ci < 2:
        nc.vector.memset(hT[:, :, :, VPP:128], 0.0)
    for s2 in range(2):
        jsl = slice(4 * s2, 4 * s2 + 4)
        for m in range(8):
            ps = pools["ps_wide"].tile([128, 4, 128], FP32, tag="wide",
                                       name="m1ps")
            nc.tensor.matmul(
                ps[:], lhsT=cst["w1"][:, :, 128 * m:128 * (m + 1)],
                rhs=xn2T[:, :, jsl, :],
                start=True, stop=True,
                perf_mode=mybir.MatmulPerfMode.DoubleRow,
            )
            bias = (cst["b1"][:, m:m + 1] if cst["b1"] is not None else 0.0)
            import os
            gelu_f = ActF.Identity if os.environ.get("GELU_ID") else ActF.Gelu
            nc.scalar.activation(
                hT[:, m, jsl, 0:VPP], ps[:, :, 0:VPP],
                gelu_f, bias=bias, scale=1.0 / W8SCALE,
            )

    out_t = pools["xp"].tile([128, PPC, C], FP32, tag="xo", name="out_t")
    for j in range(0, PPC, 2):
        ps = pools["ps_tok"].tile([128, 2, C], FP32, tag="tok", name="m2ps")
        for jj in range(2):
            for k2 in range(4):
                nc.tensor.matmul(
                    ps[:, jj, :], lhsT=hT[:, 2 * k2:2 * k2 + 2, j + jj, :],
                    rhs=cst["w2"][:, 2 * k2:2 * k2 + 2, :],
                    start=(k2 == 0), stop=(k2 == 3),
                    perf_mode=mybir.MatmulPerfMode.DoubleRow,
                )
            if cst["b2bc"] is not None:
                nc.vector.tensor_add(ps[:, jj, :], ps[:, jj, :], cst["b2bc"][:])
        nc.vector.scalar_tensor_tensor(
            out=out_t[:, j:j + 2, :], in0=ps[:], scalar=1.0 / W8SCALE,
            in1=s["x2_t"][:, j:j + 2, :], op0=Alu.mult, op1=Alu.add,
        )

    # compact store: pair (ci*PPC + j) valid rows 0:VPP
    dst = ext["out"][ci * VPC:, :]
    dst_ap = bass.AP(
        tensor=dst.tensor, offset=dst.offset,
        ap=[[C, VPP], [VPP * C, PPC], [1, C]],
    )
    nc.sync.dma_start(out=dst_ap, in_=out_t[0:VPP, :, :])


# --------------------------------------------------------------------------
# entry point
# --------------------------------------------------------------------------

_CACHE = {}


def _get_program(key_flags):
    if key_flags not in _CACHE:
        _CACHE[key_flags] = build_program(NPAIR, flags=dict(key_flags))
    return _CACHE[key_flags]


def kernel(**inputs):
    in_maps, extra = host_prep(inputs)
    nc = _get_program(tuple(sorted(extra.items())))
    res = run_bass_kernel_spmd(nc, in_maps, core_ids=list(range(B)))
    out = np.stack([res.results[i]["out"] for i in range(B)], axis=0)
    return out.reshape(B, NWIN, S, C).astype(np.float32)


# revision 51
# speedup vs baseline: 2.2306x; 1.0016x over previous
"""Swin-style windowed-attention block on 8 TRN2 NeuronCores (data-parallel over batch).

v2: compact-pair layout (both windows' 49 valid tokens at rows 0:98 of a 128-row
pair tile, zero pad rows 98:128). Pair-wide attention with post-exp multiplicative
bias (exp(S+b) = exp(S)*exp(b) with a host-precomputed exp(bias) table that also
zeroes cross-window blocks and pad rows), no augmented-K matmuls, no head-realign
or v-shift DMAs. Activation-table thrash removed (DVE Newton rsqrt; chunk-pair
interleaving batches exp/gelu). Elementwise spread across ACT / DVE / Pool.
"""

import sys

sys.path.insert(0, "/opt/trn_rl_repo")

import numpy as np
import ml_dtypes

import concourse.bass as bass
import concourse.bacc as bacc
import concourse.tile as tile
import concourse.mybir as mybir
from concourse.bass_utils import run_bass_kernel_spmd

BF16 = ml_dtypes.bfloat16
FP8 = ml_dtypes.float8_e4m3
FP32 = mybir.dt.float32
BF16_DT = mybir.dt.bfloat16
FP8_DT = mybir.dt.float8e4
INT32 = mybir.dt.int32
W8SCALE = 64.0

# ---- static geometry ----
WH, WW = 7, 7
S = 49                     # valid tokens per window
C = 256                    # channels
NH = 8                     # heads
HD = 32                    # head dim
NWIN = 256                 # windows per batch image
B = 8                      # batch == number of cores
GRID = 16                  # 16x16 window grid
SCALE = HD ** -0.5
EPS = 1e-5
MASK_VAL = -30000.0

NPAIR = NWIN // 2          # 128 window pairs per core
PPC = 8                    # pairs per chunk
NCHUNK = NPAIR // PPC      # 16 chunks
TPP = 128                  # tile rows per pair (98 valid + 30 zero pad)
VPP = 2 * S                # 98 valid tokens per pair
TPC = PPC * TPP            # 1024 padded tokens per chunk
VPC = PPC * VPP            # 784 valid tokens per chunk
NTOK = NWIN * S            # 12544 valid tokens per core
NTOKP = NPAIR * TPP        # 16384 padded tokens per core

RSQRT_MAGIC = 0x5F3759DF

ActF = mybir.ActivationFunctionType
Alu = mybir.AluOpType


# --------------------------------------------------------------------------
# host-side preparation
# --------------------------------------------------------------------------

def _relative_position_index():
    ch, cw = np.arange(WH), np.arange(WW)
    coords = np.stack(np.meshgrid(ch, cw, indexing="ij")).reshape(2, -1)
    rel = coords[:, :, None] - coords[:, None, :]
    rel = rel.transpose(1, 2, 0).astype(np.int64)
    rel[..., 0] += WH - 1
    rel[..., 1] += WW - 1
    rel[..., 0] *= 2 * WW - 1
    return rel.sum(-1)                                    # (S, S)


def _window_mask_types():
    """Per-window mask type: 0 none, 1 bottom-row, 2 right-col, 3 corner."""
    h = w = GRID
    s1, s2 = WH - WH // 2, WW - WW // 2
    m = np.zeros((h, w, WH, WW, WH, WW), dtype=bool)
    m[-1, :, :s1, :, s1:, :] = True
    m[-1, :, s1:, :, :s1, :] = True
    m[:, -1, :, :s2, :, s2:] = True
    m[:, -1, :, s2:, :, :s2] = True
    m = m.reshape(h * w, S, S)
    types = np.zeros(NWIN, dtype=np.int64)
    rr, cc = np.divmod(np.arange(NWIN), GRID)
    types[(rr == GRID - 1) & (cc < GRID - 1)] = 1
    types[(rr < GRID - 1) & (cc == GRID - 1)] = 2
    types[(rr == GRID - 1) & (cc == GRID - 1)] = 3
    masks = np.zeros((4, S, S), dtype=np.float32)
    masks[1] = np.where(m[GRID * (GRID - 1)], MASK_VAL, 0.0)
    masks[2] = np.where(m[GRID - 1], MASK_VAL, 0.0)
    masks[3] = np.where(m[NWIN - 1], MASK_VAL, 0.0)
    return types, masks


def _pair_types():
    types, _ = _window_mask_types()
    combos = []
    ptype = np.zeros(NPAIR, dtype=np.int64)
    for j in range(NPAIR):
        c = (int(types[2 * j]), int(types[2 * j + 1]))
        if c not in combos:
            combos.append(c)
        ptype[j] = combos.index(c)
    assert len(combos) <= 4, combos
    while len(combos) < 4:
        combos.append((0, 0))
    return ptype, combos


_PTYPE, _PCOMBOS = _pair_types()


def _tile_kxoc(wT):
    """[K, OC] -> [128, K//128, OC] with K = 128*kt + p."""
    K, OC = wT.shape
    return np.ascontiguousarray(wT.reshape(K // 128, 128, OC).transpose(1, 0, 2))


def host_prep(inputs):
    x = np.asarray(inputs["x"], dtype=np.float32)          # (B, N, S, C)
    qkv_w = np.asarray(inputs["qkv_w"], dtype=np.float32)
    qkv_b = np.asarray(inputs["qkv_b"], dtype=np.float32)
    proj_w = np.asarray(inputs["proj_w"], dtype=np.float32)
    proj_b = np.asarray(inputs["proj_b"], dtype=np.float32)
    n1g = np.asarray(inputs["norm1_g"], dtype=np.float32)
    n1b = np.asarray(inputs["norm1_b"], dtype=np.float32)
    n2g = np.asarray(inputs["norm2_g"], dtype=np.float32)
    n2b = np.asarray(inputs["norm2_b"], dtype=np.float32)
    w1 = np.asarray(inputs["mlp_w1"], dtype=np.float32)
    b1 = np.asarray(inputs["mlp_b1"], dtype=np.float32)
    w2 = np.asarray(inputs["mlp_w2"], dtype=np.float32)
    b2 = np.asarray(inputs["mlp_b2"], dtype=np.float32)
    table = np.asarray(inputs["bias_table"], dtype=np.float32)

    # fold layernorm affine into the following matmuls
    qkv_w_f = qkv_w * n1g[None, :]
    qkv_b_f = qkv_b + qkv_w @ n1b
    w1_f = w1 * n2g[None, :]
    b1_f = b1 + w1 @ n2b

    wq = qkv_w_f[0:C] * SCALE
    bq = qkv_b_f[0:C] * SCALE
    wk = qkv_w_f[C:2 * C]
    bk = qkv_b_f[C:2 * C]
    wv = qkv_w_f[2 * C:3 * C]
    bv = qkv_b_f[2 * C:3 * C]

    common = {
        "wq": _tile_kxoc(wq.T).astype(BF16),
        "wk": _tile_kxoc(wk.T).astype(BF16),
        "wv": _tile_kxoc(wv.T).astype(BF16),
        "wp": _tile_kxoc(proj_w.T).astype(BF16),
        "w1": _tile_kxoc(w1_f.T * W8SCALE).astype(FP8),
        "w2": _tile_kxoc(w2.T * W8SCALE).astype(FP8),
    }

    # exp(bias + mask) multiplicative table: eb[t_row, ptype, h, q_row]
    # t_row/q_row = 49*r + local; zero on cross-window blocks and pad rows.
    rel = _relative_position_index()
    bias_sht = table[rel].transpose(2, 0, 1)               # [h, s, t]
    _, masks = _window_mask_types()                        # [4, s, t]
    eb = np.zeros((TPP, 4, NH, VPP), dtype=np.float32)
    for pt, (tA, tB) in enumerate(_PCOMBOS):
        for r, wt in ((0, tA), (1, tB)):
            blk = np.exp(bias_sht + masks[wt][None])       # [h, s, t]
            eb[S * r:S * r + S, pt, :, S * r:S * r + S] = blk.transpose(2, 0, 1)
    # h-axis reordered to (h4, hh)-major to match the S-psum bank layout
    perm = [4 * hh + h4 for h4 in range(4) for hh in range(2)]
    common["eb"] = eb[:, :, perm, :].astype(BF16)

    extra = {
        "bq_nz": bool(np.any(bq != 0.0) or np.any(bk != 0.0)),
        "bv_nz": bool(np.any(bv != 0.0)),
        "pb_nz": bool(np.any(proj_b != 0.0)),
        "b1_nz": bool(np.any(b1_f != 0.0)),
        "b2_nz": bool(np.any(b2 != 0.0)),
    }
    if extra["bq_nz"]:
        common["bq"] = np.ascontiguousarray(bq.reshape(2, 128).T).astype(np.float32)
        common["bk"] = np.ascontiguousarray(bk.reshape(2, 128).T).astype(np.float32)
    if extra["b1_nz"]:
        common["b1"] = np.ascontiguousarray(b1_f.reshape(8, 128).T).astype(np.float32)
    if extra["bv_nz"]:
        common["bvbc"] = np.tile(bv[None, :], (128, 1)).astype(np.float32)
    if extra["pb_nz"]:
        common["pbbc"] = np.tile(proj_b[None, :], (128, 1)).astype(np.float32)
    if extra["b2_nz"]:
        common["b2bc"] = np.tile(b2[None, :], (128, 1)).astype(np.float32)

    in_maps = []
    for b in range(B):
        m = dict(common)
        xp = np.zeros((NPAIR, TPP, C), dtype=BF16)
        xp[:, :VPP, :] = x[b].reshape(NPAIR, VPP, C).astype(BF16)
        m["x"] = xp.reshape(NTOKP, C)
        in_maps.append(m)
    return in_maps, extra


# --------------------------------------------------------------------------
# kernel builder
# --------------------------------------------------------------------------

def build_program(n_pairs=NPAIR, flags=None):
    flags = flags or {}
    assert n_pairs % (2 * PPC) == 0
    n_chunks = n_pairs // PPC

    nc = bacc.Bacc("TRN2", target_bir_lowering=False, debug=False)

    ext = {}
    ext["x"] = nc.dram_tensor("x", [n_pairs * TPP, C], BF16_DT, kind="ExternalInput")
    ext["out"] = nc.dram_tensor("out", [n_pairs * VPP, C], FP32, kind="ExternalOutput")
    ext["wq"] = nc.dram_tensor("wq", [128, 2, C], BF16_DT, kind="ExternalInput")
    ext["wk"] = nc.dram_tensor("wk", [128, 2, C], BF16_DT, kind="ExternalInput")
    ext["wv"] = nc.dram_tensor("wv", [128, 2, C], BF16_DT, kind="ExternalInput")
    ext["wp"] = nc.dram_tensor("wp", [128, 2, C], BF16_DT, kind="ExternalInput")
    ext["w1"] = nc.dram_tensor("w1", [128, 2, 4 * C], FP8_DT, kind="ExternalInput")
    ext["w2"] = nc.dram_tensor("w2", [128, 8, C], FP8_DT, kind="ExternalInput")
    ext["eb"] = nc.dram_tensor("eb", [TPP, 4, NH, VPP], BF16_DT, kind="ExternalInput")
    for name, shape, flg in (
        ("bq", [128, 2], "bq_nz"), ("bk", [128, 2], "bq_nz"),
        ("b1", [128, 8], "b1_nz"),
        ("bvbc", [128, C], "bv_nz"), ("pbbc", [128, C], "pb_nz"),
        ("b2bc", [128, C], "b2_nz"),
    ):
        ext[name] = (nc.dram_tensor(name, shape, FP32, kind="ExternalInput")
                     if flags.get(flg) else None)

    with tile.TileContext(nc) as tc:
        _body(tc, n_chunks, ext)

    nc.compile()
    return nc


def _body(tc, n_chunks, ext):
    nc = tc.nc
    import contextlib
    with contextlib.ExitStack() as ctx:
        const = ctx.enter_context(tc.tile_pool(name="const", bufs=1))
        cst = {}
        for name, shape, dt in (
            ("wq", [128, 2, C], BF16_DT), ("wk", [128, 2, C], BF16_DT),
            ("wv", [128, 2, C], BF16_DT), ("wp", [128, 2, C], BF16_DT),
            ("w1", [128, 2, 4 * C], FP8_DT), ("w2", [128, 8, C], FP8_DT),
            ("eb", [TPP, 4, NH, VPP], BF16_DT),
            ("bq", [128, 2], FP32), ("bk", [128, 2], FP32),
            ("b1", [128, 8], FP32),
            ("bvbc", [128, C], FP32), ("pbbc", [128, C], FP32),
            ("b2bc", [128, C], FP32),
        ):
            if ext.get(name) is None:
                cst[name] = None
                continue
            t = const.tile(shape, dt, tag=name, name=name)
            nc.sync.dma_start(out=t[:], in_=ext[name].ap())
            cst[name] = t
        ident = const.tile([128, 128], BF16_DT, tag="ident", name="ident")
        from concourse.masks import make_identity
        make_identity(nc, ident[:])
        cst["ident"] = ident

        pools = {}
        for name, bufs in (("xp", 2), ("xbp", 2), ("xnp", 2), ("xnT", 2),
                           ("attnT", 2), ("xn2T", 2), ("qkp", 2), ("qk3p", 2),
                           ("vsp", 2), ("etp", 2), ("et2p", 2), ("atp", 2),
                           ("x2p", 2), ("hp", 2), ("statp", 2)):
            pools[name] = ctx.enter_context(tc.tile_pool(name=name, bufs=bufs))
        # PSUM budget (8 banks): ps_S 4 (one bank per concurrent PE row-tile),
        # ps_wide 2, ps_tok 2 (shared ring: v/psa/proj/mlp2/transpose drains)
        for name, bufs in (("ps_wide", 2), ("ps_tok", 2), ("ps_S", 1)):
            pools[name] = ctx.enter_context(
                tc.tile_pool(name=name, bufs=bufs, space="PSUM"))
        pools["ps_tr"] = pools["ps_wide"]
        pools["_ident"] = cst["ident"]

        import os
        n_phases = int(os.environ.get("PHASES", "4"))

        def _store_dbg(ci, t):
            dst = ext["out"][ci * VPC:, :]
            dst_ap = bass.AP(tensor=dst.tensor, offset=dst.offset,
                             ap=[[C, VPP], [VPP * C, PPC], [1, C]])
            nc.sync.dma_start(out=dst_ap, in_=t[0:VPP, :, :])

        st = {}
        for cp in range(n_chunks // 2):
            a, b = 2 * cp, 2 * cp + 1
            st[a] = _phase_in(tc, a, ext, cst, pools)
            st[b] = _phase_in(tc, b, ext, cst, pools)
            if n_phases < 2:
                _store_dbg(a, st[a]["x_t"])
                _store_dbg(b, st[b]["x_t"])
                del st[a], st[b]
                continue
            _phase_attn(tc, a, cst, pools, st[a])
            _phase_attn(tc, b, cst, pools, st[b])
            if n_phases < 3:
                _store_dbg(a, st[a]["x_t"])
                _store_dbg(b, st[b]["x_t"])
                del st[a], st[b]
                continue
            _phase_proj(tc, a, cst, pools, st[a])
            _phase_proj(tc, b, cst, pools, st[b])
            if n_phases < 4:
                _store_dbg(a, st[a]["x2_t"])
                _store_dbg(b, st[b]["x2_t"])
                del st[a], st[b]
                continue
            _phase_mlp(tc, a, ext, cst, pools, st[a])
            _phase_mlp(tc, b, ext, cst, pools, st[b])
            del st[a], st[b]


def _layernorm(nc, pools, x_t, xn_t):
    """x_t [128, PPC, 256] bf16 -> xn_t bf16 ((x-mu)*rstd).

    Batched stats on DVE (bn_stats 2 pairs/instr), rstd via quake-rsqrt +
    2 Newton steps on DVE (avoids the Sqrt activation-table load), apply on
    DVE (2x/4x with bf16 operands)."""
    statp = pools["statp"]
    mv = statp.tile([128, PPC, 2], FP32, tag="mv", name="mv")
    for j in range(PPC):
        bnst = statp.tile([128, 6], FP32, tag="bnst", name="bnst")
        nc.vector.bn_stats(bnst[:], x_t[:, j, :])
        nc.vector.bn_aggr(mv[:, j, :], bnst[:])
    var = statp.tile([128, PPC], FP32, tag="var", name="var")
    rst = statp.tile([128, PPC], FP32, tag="rst", name="rst")
    tmp = statp.tile([128, PPC], FP32, tag="tmp", name="tmp")
    nc.vector.tensor_scalar(out=var[:], in0=mv[:, :, 1], scalar1=EPS,
                            scalar2=None, op0=Alu.add)
    nc.vector.tensor_scalar(out=rst[:].bitcast(INT32), in0=var[:].bitcast(INT32),
                            scalar1=1, scalar2=None, op0=Alu.logical_shift_right)
    nc.vector.tensor_scalar(out=rst[:].bitcast(INT32), in0=rst[:].bitcast(INT32),
                            scalar1=-1, scalar2=RSQRT_MAGIC,
                            op0=Alu.mult, op1=Alu.add)
    for _ in range(1):
        nc.vector.tensor_tensor(out=tmp[:], in0=rst[:], in1=rst[:], op=Alu.mult)
        nc.vector.tensor_tensor(out=tmp[:], in0=tmp[:], in1=var[:], op=Alu.mult)
        nc.vector.tensor_scalar(out=tmp[:], in0=tmp[:], scalar1=-0.5,
                                scalar2=1.5, op0=Alu.mult, op1=Alu.add)
        nc.vector.tensor_tensor(out=rst[:], in0=rst[:], in1=tmp[:], op=Alu.mult)
    for j in range(PPC):
        nc.vector.tensor_scalar(
            out=xn_t[:, j, :], in0=x_t[:, j, :],
            scalar1=mv[:, j, 0:1], scalar2=rst[:, j:j + 1],
            op0=Alu.subtract, op1=Alu.mult,
        )


def _transposes(nc, pools, src_t, dst_T, drain_eng):
    """src_t [128, PPC, 256] -> dst_T [128, 2, PPC, 128] via PE + drain.

    Two pairs share one psum tile and one drain."""
    ident = pools["_ident"]
    for j in range(0, PPC, 2):
        ps = pools["ps_tr"].tile([128, 2, 2, 128], BF16_DT, tag="wide",
                                 name="trps")
        for jj in range(2):
            for ch in range(2):
                nc.tensor.transpose(
                    ps[:, jj, ch, :],
                    src_t[:, j + jj, 128 * ch:128 * (ch + 1)], ident[:, :])
        src = ps[:].rearrange("p a b t -> p b a t")
        if hasattr(drain_eng, "tensor_copy"):
            drain_eng.tensor_copy(out=dst_T[:, :, j:j + 2, :], in_=src)
        else:
            drain_eng.copy(out=dst_T[:, :, j:j + 2, :], in_=src)


def _phase_in(tc, ci, ext, cst, pools):
    """Load x, LN1, transpose, QKV -> qT/kT/vp."""
    nc = tc.nc
    s = {}

    x_t = pools["xbp"].tile([128, PPC, C], BF16_DT, tag="xb", name="x_t")
    nc.sync.dma_start(
        out=x_t[:],
        in_=ext["x"][ci * TPC:(ci + 1) * TPC, :].rearrange("(j p) c -> p j c", p=TPP),
    )
    s["x_t"] = x_t

    xn_t = pools["xnp"].tile([128, PPC, C], BF16_DT, tag="xn", name="xn_t")
    _layernorm(nc, pools, x_t, xn_t)
    xnT = pools["xnT"].tile([128, 2, PPC, 128], BF16_DT, tag="xnT", name="xnT")
    _transposes(nc, pools, xn_t, xnT, nc.vector)

    # q, k: channel-major slabs; drain on ACT (identity/copy, bias optional)
    qT = pools["qkp"].tile([128, 2, PPC, 128], BF16_DT, tag="qT", name="qT")
    kT = pools["qkp"].tile([128, 2, PPC, 128], BF16_DT, tag="kT", name="kT")
    for s2 in range(2):
        jsl = slice(4 * s2, 4 * s2 + 4)
        for (dstT, wname, bname) in ((qT, "wq", "bq"), (kT, "wk", "bk")):
            w_sb = cst[wname]
            for octl in range(2):
                ps = pools["ps_wide"].tile([128, 4, 128], FP32, tag="wide",
                                           name="qkps")
                for kt in range(2):
                    nc.tensor.matmul(
                        ps[:],
                        lhsT=w_sb[:, kt, 128 * octl:128 * (octl + 1)],
                        rhs=xnT[:, kt, jsl, :],
                        start=(kt == 0), stop=(kt == 1),
                    )
                if cst[bname] is not None:
                    nc.scalar.activation(
                        dstT[:, octl, jsl, :], ps[:], ActF.Identity,
                        bias=cst[bname][:, octl:octl + 1])
                else:
                    nc.scalar.activation(dstT[:, octl, jsl, :], ps[:], ActF.Copy)
    s["qT"], s["kT"] = qT, kT

    # PE matmul row-tile base 96 is unsupported; realign the hm==3 head rows
    # (h = 3, 7) of q/k to partition base 0 via one small DMA each.
    qk3 = pools["qk3p"].tile([32, 2, 2, PPC, 128], BF16_DT, tag="qk3", name="qk3")
    nc.sync.dma_start(out=qk3[0:32, 0], in_=qT[96:128, :, :, :])
    nc.sync.dma_start(out=qk3[0:32, 1], in_=kT[96:128, :, :, :])
    s["qk3"] = qk3

    # v: token-major, drain straight into pair-local vp (no base shift);
    # two pairs share one psum tile and one drain
    vp = pools["vsp"].tile([128, PPC, NH, HD + 1], BF16_DT, tag="vp", name="vp")
    for j in range(0, PPC, 2):
        ps = pools["ps_tok"].tile([128, 2, C], FP32, tag="tok", name="vps")
        for jj in range(2):
            for kt in range(2):
                nc.tensor.matmul(
                    ps[:, jj, :], lhsT=xnT[:, kt, j + jj, :],
                    rhs=cst["wv"][:, kt, :],
                    start=(kt == 0), stop=(kt == 1),
                )
        nc.vector.memset(vp[:, j:j + 2, :, 0:1], 1.0)
        if cst["bvbc"] is not None:
            for jj in range(2):
                nc.vector.tensor_add(ps[:, jj, :], ps[:, jj, :], cst["bvbc"][:])
        nc.scalar.copy(
            out=vp[:, j:j + 2, :, 1:HD + 1],
            in_=ps[:].rearrange("p a (h d) -> p a h d", h=NH),
        )
    s["vp"] = vp
    return s


def _phase_attn(tc, ci, cst, pools, s):
    """S = K^T Q pair-wide, exp, *exp(bias), A = et2 @ [1|v]."""
    nc = tc.nc
    qT, kT, vp = s["qT"], s["kT"], s["vp"]
    attn_t = pools["atp"].tile([128, PPC, C], BF16_DT, tag="attn", name="attn_t")
    if ci < 2:
        # first use of each ring buffer: seed pad rows (never valid-read,
        # but must be finite/initialized for the pair transposes). 96-aligned
        # partition start; rows 96:98 are re-written by the attn drain below.
        nc.vector.memset(attn_t[96:128, :, :], 0.0)
    qk3 = s["qk3"]
    JB = 2                                      # pairs per exp batch
    for jg in range(PPC // JB):
        js = list(range(JB * jg, JB * jg + JB))
        # et holds the batch, h-axis (h4, hh)-major to match the psum layout
        et = pools["etp"].tile([128, NH, JB, VPP], BF16_DT, tag="et", name="et")
        # all 8 heads x JB pairs in one psum tile: [h4(bank), hh, jj, col];
        # h4-stride = 2KB so each concurrent PE row-tile owns its own bank
        # (same-bank writes h and h+4 run on the same row-tile => serialized).
        pss = pools["ps_S"].tile([128, 4, 2 * JB, 128], FP32, tag="S", name="pss")
        for jj, j in enumerate(js):
            for h in range(NH):
                hh, hm = h // 4, h % 4
                octl = h // 4
                if hm == 3:
                    lhsT = qk3[0:32, 1, octl, j, :]
                    rhs = qk3[0:32, 0, octl, j, 0:VPP]
                    base = 0
                else:
                    base = 32 * hm
                    lhsT = kT[base:base + 32, octl, j, :]
                    rhs = qT[base:base + 32, octl, j, 0:VPP]
                nc.tensor.matmul(
                    pss[:, hm, JB * hh + jj, 0:VPP], lhsT=lhsT, rhs=rhs,
                    start=True, stop=True,
                    tile_position=(base, 0),
                )
        nc.scalar.activation(
            et[:].rearrange("p (a b) j q -> p a (b j) q", a=4),
            pss[:, :, :, 0:VPP], ActF.Exp)
        # eb multiply, batched over runs of equal pair-type
        et2 = pools["et2p"].tile([128, NH, JB, VPP], BF16_DT, tag="et2",
                                 name="et2")
        runs = []
        for jj, j in enumerate(js):
            pt = int(_PTYPE[ci * PPC + j])
            if runs and runs[-1][0] == pt:
                runs[-1][2] = jj + 1
            else:
                runs.append([pt, jj, jj + 1])
        for pt, j0, j1 in runs:
            e3 = cst["eb"][:, pt, :, :]
            eb_b = bass.AP(tensor=e3.tensor, offset=e3.offset,
                           ap=[e3.ap[0], e3.ap[1], [0, j1 - j0], e3.ap[2]])
            nc.vector.tensor_tensor(
                out=et2[:, :, j0:j1, :], in0=et[:, :, j0:j1, :],
                in1=eb_b, op=Alu.mult,
            )
        for jj, j in enumerate(js):
            psa = pools["ps_tok"].tile([VPP, NH, HD + 1], FP32, tag="tok",
                                       name="psa")
            for h in range(NH):
                nc.tensor.matmul(
                    psa[:, h, :], lhsT=et2[:, 2 * (h % 4) + h // 4, jj, :],
                    rhs=vp[:, j, h, :],
                    start=True, stop=True,
                )
            rec = pools["statp"].tile([VPP, NH], FP32, tag="rec", name="rec")
            nc.vector.tensor_scalar_max(out=rec[:], in0=psa[:, :, 0],
                                        scalar1=1e-30)
            nc.vector.reciprocal(rec[:], rec[:])
            rec_b = bass.AP(tensor=rec[:].tensor, offset=rec[:].offset,
                            ap=list(rec[:].ap) + [[0, HD]])
            nc.vector.tensor_mul(
                attn_t[0:VPP, j, :].rearrange("p (h d) -> p h d", h=NH),
                psa[:, :, 1:HD + 1], rec_b,
            )
    s["attn_t"] = attn_t


def _phase_proj(tc, ci, cst, pools, s):
    """attn transpose, proj + resid1, LN2, transpose -> xn2T (fp8)."""
    nc = tc.nc
    attnT = pools["attnT"].tile([128, 2, PPC, 128], BF16_DT, tag="attnT",
                                name="attnT")
    _transposes(nc, pools, s["attn_t"], attnT, nc.vector)

    x2_t = pools["x2p"].tile([128, PPC, C], BF16_DT, tag="x2", name="x2_t")
    for j in range(0, PPC, 2):
        ps = pools["ps_tok"].tile([128, 2, C], FP32, tag="tok", name="prps")
        for jj in range(2):
            for kt in range(2):
                nc.tensor.matmul(
                    ps[:, jj, :], lhsT=attnT[:, kt, j + jj, :],
                    rhs=cst["wp"][:, kt, :],
                    start=(kt == 0), stop=(kt == 1),
                )
            if cst["pbbc"] is not None:
                nc.vector.tensor_add(ps[:, jj, :], ps[:, jj, :], cst["pbbc"][:])
        nc.vector.scalar_tensor_tensor(
            out=x2_t[:, j:j + 2, :], in0=ps[:], scalar=1.0,
            in1=s["x_t"][:, j:j + 2, :], op0=Alu.mult, op1=Alu.add,
        )
    s["x2_t"] = x2_t

    xn2_t = pools["xnp"].tile([128, PPC, C], BF16_DT, tag="xn", name="xn2_t")
    _layernorm(nc, pools, x2_t, xn2_t)
    xn2T = pools["xn2T"].tile([128, 2, PPC, 128], FP8_DT, tag="xn2T", name="xn2T")
    _transposes(nc, pools, xn2_t, xn2T, nc.scalar)
    s["xn2T"] = xn2T


def _phase_mlp(tc, ci, ext, cst, pools, s):
    """MLP (fp8 DoubleRow) + resid2, store."""
    nc = tc.nc
    xn2T = s["xn2T"]
    hT = pools["hp"].tile([128, 8, PPC, 128], FP8_DT, tag="hT", name="hT")
    if ci < 2:
        nc.vector.memset(hT[:, :, :, VPP:128], 0.0)
    for s2 in range(2):
        jsl = slice(4 * s2, 4 * s2 + 4)
        for m in range(8):
            ps = pools["ps_wide"].tile([128, 4, 128], FP32, tag="wide",
                                       name="m1ps")
            nc.tensor.matmul(
                ps[:], lhsT=cst["w1"][:, :, 128 * m:128 * (m + 1)],
                rhs=xn2T[:, :, jsl, :],
                start=True, stop=True,
                perf_mode=mybir.MatmulPerfMode.DoubleRow,
            )
            bias = (cst["b1"][:, m:m + 1] if cst["b1"] is not None else 0.0)
            import os
            gelu_f = ActF.Identity if os.environ.get("GELU_ID") else ActF.Gelu
            nc.scalar.activation(
                hT[:, m, jsl, 0:VPP], ps[:, :, 0:VPP],
                gelu_f, bias=bias, scale=1.0 / W8SCALE,
            )

    out_t = pools["xp"].tile([128, PPC, C], FP32, tag="xo", name="out_t")
    for j in range(0, PPC, 2):
        ps = pools["ps_tok"].tile([128, 2, C], FP32, tag="tok", name="m2ps")
        for jj in range(2):
            for k2 in range(4):
                nc.tensor.matmul(
                    ps[:, jj, :], lhsT=hT[:, 2 * k2:2 * k2 + 2, j + jj, :],
                    rhs=cst["w2"][:, 2 * k2:2 * k2 + 2, :],
                    start=(k2 == 0), stop=(k2 == 3),
                    perf_mode=mybir.MatmulPerfMode.DoubleRow,
                )
            if cst["b2bc"] is not None:
                nc.vector.tensor_add(ps[:, jj, :], ps[:, jj, :], cst["b2bc"][:])
        nc.vector.scalar_tensor_tensor(
            out=out_t[:, j:j + 2, :], in0=ps[:], scalar=1.0 / W8SCALE,
            in1=s["x2_t"][:, j:j + 2, :], op0=Alu.mult, op1=Alu.add,
        )

    # compact store: pair (ci*PPC + j) valid rows 0:VPP
    dst = ext["out"][ci * VPC:, :]
    dst_ap = bass.AP(
        tensor=dst.tensor, offset=dst.offset,
        ap=[[C, VPP], [VPP * C, PPC], [1, C]],
    )
    nc.sync.dma_start(out=dst_ap, in_=out_t[0:VPP, :, :])


# --------------------------------------------------------------------------
# entry point
# --------------------------------------------------------------------------

_CACHE = {}


def _get_program(key_flags):
    if key_flags not in _CACHE:
        _CACHE[key_flags] = build_program(NPAIR, flags=dict(key_flags))
    return _CACHE[key_flags]


def kernel(**inputs):
    in_maps, extra = host_prep(inputs)
    nc = _get_program(tuple(sorted(extra.items())))
    res = run_bass_kernel_spmd(nc, in_maps, core_ids=list(range(B)))
    out = np.stack([res.results[i]["out"] for i in range(B)], axis=0)
    return out.reshape(B, NWIN, S, C).astype(np.float32)


# revision 53
# speedup vs baseline: 2.2317x; 1.0005x over previous
"""Swin-style windowed-attention block on 8 TRN2 NeuronCores (data-parallel over batch).

v2: compact-pair layout (both windows' 49 valid tokens at rows 0:98 of a 128-row
pair tile, zero pad rows 98:128). Pair-wide attention with post-exp multiplicative
bias (exp(S+b) = exp(S)*exp(b) with a host-precomputed exp(bias) table that also
zeroes cross-window blocks and pad rows), no augmented-K matmuls, no head-realign
or v-shift DMAs. Activation-table thrash removed (DVE Newton rsqrt; chunk-pair
interleaving batches exp/gelu). Elementwise spread across ACT / DVE / Pool.
"""

import sys

sys.path.insert(0, "/opt/trn_rl_repo")

import numpy as np
import ml_dtypes

import concourse.bass as bass
import concourse.bacc as bacc
import concourse.tile as tile
import concourse.mybir as mybir
from concourse.bass_utils import run_bass_kernel_spmd

BF16 = ml_dtypes.bfloat16
FP8 = ml_dtypes.float8_e4m3
FP32 = mybir.dt.float32
BF16_DT = mybir.dt.bfloat16
FP8_DT = mybir.dt.float8e4
INT32 = mybir.dt.int32
W8SCALE = 64.0

# ---- static geometry ----
WH, WW = 7, 7
S = 49                     # valid tokens per window
C = 256                    # channels
NH = 8                     # heads
HD = 32                    # head dim
NWIN = 256                 # windows per batch image
B = 8                      # batch == number of cores
GRID = 16                  # 16x16 window grid
SCALE = HD ** -0.5
EPS = 1e-5
MASK_VAL = -30000.0

NPAIR = NWIN // 2          # 128 window pairs per core
PPC = 8                    # pairs per chunk
NCHUNK = NPAIR // PPC      # 16 chunks
TPP = 128                  # tile rows per pair (98 valid + 30 zero pad)
VPP = 2 * S                # 98 valid tokens per pair
TPC = PPC * TPP            # 1024 padded tokens per chunk
VPC = PPC * VPP            # 784 valid tokens per chunk
NTOK = NWIN * S            # 12544 valid tokens per core
NTOKP = NPAIR * TPP        # 16384 padded tokens per core

RSQRT_MAGIC = 0x5F3759DF

ActF = mybir.ActivationFunctionType
Alu = mybir.AluOpType


# --------------------------------------------------------------------------
# host-side preparation
# --------------------------------------------------------------------------

def _relative_position_index():
    ch, cw = np.arange(WH), np.arange(WW)
    coords = np.stack(np.meshgrid(ch, cw, indexing="ij")).reshape(2, -1)
    rel = coords[:, :, None] - coords[:, None, :]
    rel = rel.transpose(1, 2, 0).astype(np.int64)
    rel[..., 0] += WH - 1
    rel[..., 1] += WW - 1
    rel[..., 0] *= 2 * WW - 1
    return rel.sum(-1)                                    # (S, S)


def _window_mask_types():
    """Per-window mask type: 0 none, 1 bottom-row, 2 right-col, 3 corner."""
    h = w = GRID
    s1, s2 = WH - WH // 2, WW - WW // 2
    m = np.zeros((h, w, WH, WW, WH, WW), dtype=bool)
    m[-1, :, :s1, :, s1:, :] = True
    m[-1, :, s1:, :, :s1, :] = True
    m[:, -1, :, :s2, :, s2:] = True
    m[:, -1, :, s2:, :, :s2] = True
    m = m.reshape(h * w, S, S)
    types = np.zeros(NWIN, dtype=np.int64)
    rr, cc = np.divmod(np.arange(NWIN), GRID)
    types[(rr == GRID - 1) & (cc < GRID - 1)] = 1
    types[(rr < GRID - 1) & (cc == GRID - 1)] = 2
    types[(rr == GRID - 1) & (cc == GRID - 1)] = 3
    masks = np.zeros((4, S, S), dtype=np.float32)
    masks[1] = np.where(m[GRID * (GRID - 1)], MASK_VAL, 0.0)
    masks[2] = np.where(m[GRID - 1], MASK_VAL, 0.0)
    masks[3] = np.where(m[NWIN - 1], MASK_VAL, 0.0)
    return types, masks


def _pair_types():
    types, _ = _window_mask_types()
    combos = []
    ptype = np.zeros(NPAIR, dtype=np.int64)
    for j in range(NPAIR):
        c = (int(types[2 * j]), int(types[2 * j + 1]))
        if c not in combos:
            combos.append(c)
        ptype[j] = combos.index(c)
    assert len(combos) <= 4, combos
    while len(combos) < 4:
        combos.append((0, 0))
    return ptype, combos


_PTYPE, _PCOMBOS = _pair_types()


def _tile_kxoc(wT):
    """[K, OC] -> [128, K//128, OC] with K = 128*kt + p."""
    K, OC = wT.shape
    return np.ascontiguousarray(wT.reshape(K // 128, 128, OC).transpose(1, 0, 2))


def host_prep(inputs):
    x = np.asarray(inputs["x"], dtype=np.float32)          # (B, N, S, C)
    qkv_w = np.asarray(inputs["qkv_w"], dtype=np.float32)
    qkv_b = np.asarray(inputs["qkv_b"], dtype=np.float32)
    proj_w = np.asarray(inputs["proj_w"], dtype=np.float32)
    proj_b = np.asarray(inputs["proj_b"], dtype=np.float32)
    n1g = np.asarray(inputs["norm1_g"], dtype=np.float32)
    n1b = np.asarray(inputs["norm1_b"], dtype=np.float32)
    n2g = np.asarray(inputs["norm2_g"], dtype=np.float32)
    n2b = np.asarray(inputs["norm2_b"], dtype=np.float32)
    w1 = np.asarray(inputs["mlp_w1"], dtype=np.float32)
    b1 = np.asarray(inputs["mlp_b1"], dtype=np.float32)
    w2 = np.asarray(inputs["mlp_w2"], dtype=np.float32)
    b2 = np.asarray(inputs["mlp_b2"], dtype=np.float32)
    table = np.asarray(inputs["bias_table"], dtype=np.float32)

    # fold layernorm affine into the following matmuls
    qkv_w_f = qkv_w * n1g[None, :]
    qkv_b_f = qkv_b + qkv_w @ n1b
    w1_f = w1 * n2g[None, :]
    b1_f = b1 + w1 @ n2b

    wq = qkv_w_f[0:C] * SCALE
    bq = qkv_b_f[0:C] * SCALE
    wk = qkv_w_f[C:2 * C]
    bk = qkv_b_f[C:2 * C]
    wv = qkv_w_f[2 * C:3 * C]
    bv = qkv_b_f[2 * C:3 * C]

    common = {
        "wq": _tile_kxoc(wq.T).astype(BF16),
        "wk": _tile_kxoc(wk.T).astype(BF16),
        "wv": _tile_kxoc(wv.T).astype(BF16),
        "wp": _tile_kxoc(proj_w.T).astype(BF16),
        "w1": _tile_kxoc(w1_f.T * W8SCALE).astype(FP8),
        "w2": _tile_kxoc(w2.T * W8SCALE).astype(FP8),
    }

    # exp(bias + mask) multiplicative table: eb[t_row, ptype, h, q_row]
    # t_row/q_row = 49*r + local; zero on cross-window blocks and pad rows.
    rel = _relative_position_index()
    bias_sht = table[rel].transpose(2, 0, 1)               # [h, s, t]
    _, masks = _window_mask_types()                        # [4, s, t]
    eb = np.zeros((TPP, 4, NH, VPP), dtype=np.float32)
    for pt, (tA, tB) in enumerate(_PCOMBOS):
        for r, wt in ((0, tA), (1, tB)):
            blk = np.exp(bias_sht + masks[wt][None])       # [h, s, t]
            eb[S * r:S * r + S, pt, :, S * r:S * r + S] = blk.transpose(2, 0, 1)
    # h-axis reordered to (h4, hh)-major to match the S-psum bank layout
    perm = [4 * hh + h4 for h4 in range(4) for hh in range(2)]
    common["eb"] = eb[:, :, perm, :].astype(BF16)

    extra = {
        "bq_nz": bool(np.any(bq != 0.0) or np.any(bk != 0.0)),
        "bv_nz": bool(np.any(bv != 0.0)),
        "pb_nz": bool(np.any(proj_b != 0.0)),
        "b1_nz": bool(np.any(b1_f != 0.0)),
        "b2_nz": bool(np.any(b2 != 0.0)),
    }
    if extra["bq_nz"]:
        common["bq"] = np.ascontiguousarray(bq.reshape(2, 128).T).astype(np.float32)
        common["bk"] = np.ascontiguousarray(bk.reshape(2, 128).T).astype(np.float32)
    if extra["b1_nz"]:
        common["b1"] = np.ascontiguousarray(b1_f.reshape(8, 128).T).astype(np.float32)
    if extra["bv_nz"]:
        common["bvbc"] = np.tile(bv[None, :], (128, 1)).astype(np.float32)
    if extra["pb_nz"]:
        common["pbbc"] = np.tile(proj_b[None, :], (128, 1)).astype(np.float32)
    if extra["b2_nz"]:
        common["b2bc"] = np.tile(b2[None, :], (128, 1)).astype(np.float32)

    in_maps = []
    for b in range(B):
        m = dict(common)
        xp = np.zeros((NPAIR, TPP, C), dtype=BF16)
        xp[:, :VPP, :] = x[b].reshape(NPAIR, VPP, C).astype(BF16)
        m["x"] = xp.reshape(NTOKP, C)
        in_maps.append(m)
    return in_maps, extra


# --------------------------------------------------------------------------
# kernel builder
# --------------------------------------------------------------------------

def build_program(n_pairs=NPAIR, flags=None):
    flags = flags or {}
    assert n_pairs % (2 * PPC) == 0
    n_chunks = n_pairs // PPC

    nc = bacc.Bacc("TRN2", target_bir_lowering=False, debug=False)

    ext = {}
    ext["x"] = nc.dram_tensor("x", [n_pairs * TPP, C], BF16_DT, kind="ExternalInput")
    ext["out"] = nc.dram_tensor("out", [n_pairs * VPP, C], FP32, kind="ExternalOutput")
    ext["wq"] = nc.dram_tensor("wq", [128, 2, C], BF16_DT, kind="ExternalInput")
    ext["wk"] = nc.dram_tensor("wk", [128, 2, C], BF16_DT, kind="ExternalInput")
    ext["wv"] = nc.dram_tensor("wv", [128, 2, C], BF16_DT, kind="ExternalInput")
    ext["wp"] = nc.dram_tensor("wp", [128, 2, C], BF16_DT, kind="ExternalInput")
    ext["w1"] = nc.dram_tensor("w1", [128, 2, 4 * C], FP8_DT, kind="ExternalInput")
    ext["w2"] = nc.dram_tensor("w2", [128, 8, C], FP8_DT, kind="ExternalInput")
    ext["eb"] = nc.dram_tensor("eb", [TPP, 4, NH, VPP], BF16_DT, kind="ExternalInput")
    for name, shape, flg in (
        ("bq", [128, 2], "bq_nz"), ("bk", [128, 2], "bq_nz"),
        ("b1", [128, 8], "b1_nz"),
        ("bvbc", [128, C], "bv_nz"), ("pbbc", [128, C], "pb_nz"),
        ("b2bc", [128, C], "b2_nz"),
    ):
        ext[name] = (nc.dram_tensor(name, shape, FP32, kind="ExternalInput")
                     if flags.get(flg) else None)

    with tile.TileContext(nc) as tc:
        _body(tc, n_chunks, ext)

    nc.compile()
    return nc


def _body(tc, n_chunks, ext):
    nc = tc.nc
    import contextlib
    with contextlib.ExitStack() as ctx:
        const = ctx.enter_context(tc.tile_pool(name="const", bufs=1))
        cst = {}
        for name, shape, dt in (
            ("wq", [128, 2, C], BF16_DT), ("wk", [128, 2, C], BF16_DT),
            ("wv", [128, 2, C], BF16_DT), ("wp", [128, 2, C], BF16_DT),
            ("w1", [128, 2, 4 * C], FP8_DT), ("w2", [128, 8, C], FP8_DT),
            ("eb", [TPP, 4, NH, VPP], BF16_DT),
            ("bq", [128, 2], FP32), ("bk", [128, 2], FP32),
            ("b1", [128, 8], FP32),
            ("bvbc", [128, C], FP32), ("pbbc", [128, C], FP32),
            ("b2bc", [128, C], FP32),
        ):
            if ext.get(name) is None:
                cst[name] = None
                continue
            t = const.tile(shape, dt, tag=name, name=name)
            nc.sync.dma_start(out=t[:], in_=ext[name].ap())
            cst[name] = t
        ident = const.tile([128, 128], BF16_DT, tag="ident", name="ident")
        from concourse.masks import make_identity
        make_identity(nc, ident[:])
        cst["ident"] = ident

        pools = {}
        for name, bufs in (("xp", 2), ("xbp", 2), ("xnp", 2), ("xnT", 2),
                           ("attnT", 2), ("xn2T", 4), ("qkp", 2), ("qk3p", 2),
                           ("vsp", 2), ("etp", 2), ("et2p", 2), ("atp", 2),
                           ("x2p", 4), ("hp", 2), ("statp", 2)):
            pools[name] = ctx.enter_context(tc.tile_pool(name=name, bufs=bufs))
        # PSUM budget (8 banks): ps_S 4 (one bank per concurrent PE row-tile),
        # ps_wide 2, ps_tok 2 (shared ring: v/psa/proj/mlp2/transpose drains)
        for name, bufs in (("ps_wide", 2), ("ps_tok", 2), ("ps_S", 1)):
            pools[name] = ctx.enter_context(
                tc.tile_pool(name=name, bufs=bufs, space="PSUM"))
        pools["ps_tr"] = pools["ps_wide"]
        pools["_ident"] = cst["ident"]

        import os
        n_phases = int(os.environ.get("PHASES", "4"))

        def _store_dbg(ci, t):
            dst = ext["out"][ci * VPC:, :]
            dst_ap = bass.AP(tensor=dst.tensor, offset=dst.offset,
                             ap=[[C, VPP], [VPP * C, PPC], [1, C]])
            nc.sync.dma_start(out=dst_ap, in_=t[0:VPP, :, :])

        # software pipeline: the MLP of pair-group N is emitted after the
        # in/attn/proj of pair-group N+1, so the PE always has matmul work
        # while the DVE runs the next group's LN chains.
        st = {}
        prev = None
        for cp in range(n_chunks // 2):
            a, b = 2 * cp, 2 * cp + 1
            st[a] = _phase_in(tc, a, ext, cst, pools)
            st[b] = _phase_in(tc, b, ext, cst, pools)
            if n_phases < 2:
                _store_dbg(a, st[a]["x_t"])
                _store_dbg(b, st[b]["x_t"])
                del st[a], st[b]
                continue
            _phase_attn(tc, a, cst, pools, st[a])
            _phase_attn(tc, b, cst, pools, st[b])
            if n_phases < 3:
                _store_dbg(a, st[a]["x_t"])
                _store_dbg(b, st[b]["x_t"])
                del st[a], st[b]
                continue
            _phase_proj(tc, a, cst, pools, st[a])
            _phase_proj(tc, b, cst, pools, st[b])
            if n_phases < 4:
                _store_dbg(a, st[a]["x2_t"])
                _store_dbg(b, st[b]["x2_t"])
                del st[a], st[b]
                continue
            if prev is not None:
                for p in prev:
                    _phase_mlp(tc, p, ext, cst, pools, st[p])
                    del st[p]
            prev = (a, b)
        if n_phases >= 4 and prev is not None:
            for p in prev:
                _phase_mlp(tc, p, ext, cst, pools, st[p])
                del st[p]


def _layernorm(nc, pools, x_t, xn_t):
    """x_t [128, PPC, 256] bf16 -> xn_t bf16 ((x-mu)*rstd).

    Batched stats on DVE (bn_stats 2 pairs/instr), rstd via quake-rsqrt +
    2 Newton steps on DVE (avoids the Sqrt activation-table load), apply on
    DVE (2x/4x with bf16 operands)."""
    statp = pools["statp"]
    mv = statp.tile([128, PPC, 2], FP32, tag="mv", name="mv")
    for j in range(PPC):
        bnst = statp.tile([128, 6], FP32, tag="bnst", name="bnst")
        nc.vector.bn_stats(bnst[:], x_t[:, j, :])
        nc.vector.bn_aggr(mv[:, j, :], bnst[:])
    var = statp.tile([128, PPC], FP32, tag="var", name="var")
    rst = statp.tile([128, PPC], FP32, tag="rst", name="rst")
    tmp = statp.tile([128, PPC], FP32, tag="tmp", name="tmp")
    nc.vector.tensor_scalar(out=var[:], in0=mv[:, :, 1], scalar1=EPS,
                            scalar2=None, op0=Alu.add)
    nc.vector.tensor_scalar(out=rst[:].bitcast(INT32), in0=var[:].bitcast(INT32),
                            scalar1=1, scalar2=None, op0=Alu.logical_shift_right)
    nc.vector.tensor_scalar(out=rst[:].bitcast(INT32), in0=rst[:].bitcast(INT32),
                            scalar1=-1, scalar2=RSQRT_MAGIC,
                            op0=Alu.mult, op1=Alu.add)
    for _ in range(1):
        nc.vector.tensor_tensor(out=tmp[:], in0=rst[:], in1=rst[:], op=Alu.mult)
        nc.vector.tensor_tensor(out=tmp[:], in0=tmp[:], in1=var[:], op=Alu.mult)
        nc.vector.tensor_scalar(out=tmp[:], in0=tmp[:], scalar1=-0.5,
                                scalar2=1.5, op0=Alu.mult, op1=Alu.add)
        nc.vector.tensor_tensor(out=rst[:], in0=rst[:], in1=tmp[:], op=Alu.mult)
    for j in range(PPC):
        nc.vector.tensor_scalar(
            out=xn_t[:, j, :], in0=x_t[:, j, :],
            scalar1=mv[:, j, 0:1], scalar2=rst[:, j:j + 1],
            op0=Alu.subtract, op1=Alu.mult,
        )


def _transposes(nc, pools, src_t, dst_T, drain_eng):
    """src_t [128, PPC, 256] -> dst_T [128, 2, PPC, 128] via PE + drain.

    Two pairs share one psum tile and one drain."""
    ident = pools["_ident"]
    for j in range(0, PPC, 2):
        ps = pools["ps_tr"].tile([128, 2, 2, 128], BF16_DT, tag="wide",
                                 name="trps")
        for jj in range(2):
            for ch in range(2):
                nc.tensor.transpose(
                    ps[:, jj, ch, :],
                    src_t[:, j + jj, 128 * ch:128 * (ch + 1)], ident[:, :])
        src = ps[:].rearrange("p a b t -> p b a t")
        if hasattr(drain_eng, "tensor_copy"):
            drain_eng.tensor_copy(out=dst_T[:, :, j:j + 2, :], in_=src)
        else:
            drain_eng.copy(out=dst_T[:, :, j:j + 2, :], in_=src)


def _phase_in(tc, ci, ext, cst, pools):
    """Load x, LN1, transpose, QKV -> qT/kT/vp."""
    nc = tc.nc
    s = {}

    x_t = pools["xbp"].tile([128, PPC, C], BF16_DT, tag="xb", name="x_t")
    nc.sync.dma_start(
        out=x_t[:],
        in_=ext["x"][ci * TPC:(ci + 1) * TPC, :].rearrange("(j p) c -> p j c", p=TPP),
    )
    s["x_t"] = x_t

    xn_t = pools["xnp"].tile([128, PPC, C], BF16_DT, tag="xn", name="xn_t")
    _layernorm(nc, pools, x_t, xn_t)
    xnT = pools["xnT"].tile([128, 2, PPC, 128], BF16_DT, tag="xnT", name="xnT")
    _transposes(nc, pools, xn_t, xnT, nc.vector)

    # q, k: channel-major slabs; drain on ACT (identity/copy, bias optional)
    qT = pools["qkp"].tile([128, 2, PPC, 128], BF16_DT, tag="qT", name="qT")
    kT = pools["qkp"].tile([128, 2, PPC, 128], BF16_DT, tag="kT", name="kT")
    for s2 in range(2):
        jsl = slice(4 * s2, 4 * s2 + 4)
        for (dstT, wname, bname) in ((qT, "wq", "bq"), (kT, "wk", "bk")):
            w_sb = cst[wname]
            for octl in range(2):
                ps = pools["ps_wide"].tile([128, 4, 128], FP32, tag="wide",
                                           name="qkps")
                for kt in range(2):
                    nc.tensor.matmul(
                        ps[:],
                        lhsT=w_sb[:, kt, 128 * octl:128 * (octl + 1)],
                        rhs=xnT[:, kt, jsl, :],
                        start=(kt == 0), stop=(kt == 1),
                    )
                if cst[bname] is not None:
                    nc.scalar.activation(
                        dstT[:, octl, jsl, :], ps[:], ActF.Identity,
                        bias=cst[bname][:, octl:octl + 1])
                else:
                    nc.scalar.activation(dstT[:, octl, jsl, :], ps[:], ActF.Copy)
    s["qT"], s["kT"] = qT, kT

    # PE matmul row-tile base 96 is unsupported; realign the hm==3 head rows
    # (h = 3, 7) of q/k to partition base 0 via one small DMA each.
    qk3 = pools["qk3p"].tile([32, 2, 2, PPC, 128], BF16_DT, tag="qk3", name="qk3")
    nc.sync.dma_start(out=qk3[0:32, 0], in_=qT[96:128, :, :, :])
    nc.sync.dma_start(out=qk3[0:32, 1], in_=kT[96:128, :, :, :])
    s["qk3"] = qk3

    # v: token-major, drain straight into pair-local vp (no base shift);
    # two pairs share one psum tile and one drain
    vp = pools["vsp"].tile([128, PPC, NH, HD + 1], BF16_DT, tag="vp", name="vp")
    for j in range(0, PPC, 2):
        ps = pools["ps_tok"].tile([128, 2, C], FP32, tag="tok", name="vps")
        for jj in range(2):
            for kt in range(2):
                nc.tensor.matmul(
                    ps[:, jj, :], lhsT=xnT[:, kt, j + jj, :],
                    rhs=cst["wv"][:, kt, :],
                    start=(kt == 0), stop=(kt == 1),
                )
        nc.vector.memset(vp[:, j:j + 2, :, 0:1], 1.0)
        if cst["bvbc"] is not None:
            for jj in range(2):
                nc.vector.tensor_add(ps[:, jj, :], ps[:, jj, :], cst["bvbc"][:])
        nc.scalar.copy(
            out=vp[:, j:j + 2, :, 1:HD + 1],
            in_=ps[:].rearrange("p a (h d) -> p a h d", h=NH),
        )
    s["vp"] = vp
    return s


def _phase_attn(tc, ci, cst, pools, s):
    """S = K^T Q pair-wide, exp, *exp(bias), A = et2 @ [1|v]."""
    nc = tc.nc
    qT, kT, vp = s["qT"], s["kT"], s["vp"]
    attn_t = pools["atp"].tile([128, PPC, C], BF16_DT, tag="attn", name="attn_t")
    if ci < 2:
        # first use of each ring buffer: seed pad rows (never valid-read,
        # but must be finite/initialized for the pair transposes). 96-aligned
        # partition start; rows 96:98 are re-written by the attn drain below.
        nc.vector.memset(attn_t[96:128, :, :], 0.0)
    qk3 = s["qk3"]
    JB = 2                                      # pairs per exp batch
    for jg in range(PPC // JB):
        js = list(range(JB * jg, JB * jg + JB))
        # et holds the batch, h-axis (h4, hh)-major to match the psum layout
        et = pools["etp"].tile([128, NH, JB, VPP], BF16_DT, tag="et", name="et")
        # all 8 heads x JB pairs in one psum tile: [h4(bank), hh, jj, col];
        # h4-stride = 2KB so each concurrent PE row-tile owns its own bank
        # (same-bank writes h and h+4 run on the same row-tile => serialized).
        pss = pools["ps_S"].tile([128, 4, 2 * JB, 128], FP32, tag="S", name="pss")
        for jj, j in enumerate(js):
            for h in range(NH):
                hh, hm = h // 4, h % 4
                octl = h // 4
                if hm == 3:
                    lhsT = qk3[0:32, 1, octl, j, :]
                    rhs = qk3[0:32, 0, octl, j, 0:VPP]
                    base = 0
                else:
                    base = 32 * hm
                    lhsT = kT[base:base + 32, octl, j, :]
                    rhs = qT[base:base + 32, octl, j, 0:VPP]
                nc.tensor.matmul(
                    pss[:, hm, JB * hh + jj, 0:VPP], lhsT=lhsT, rhs=rhs,
                    start=True, stop=True,
                    tile_position=(base, 0),
                )
        nc.scalar.activation(
            et[:].rearrange("p (a b) j q -> p a (b j) q", a=4),
            pss[:, :, :, 0:VPP], ActF.Exp)
        # eb multiply, batched over runs of equal pair-type
        et2 = pools["et2p"].tile([128, NH, JB, VPP], BF16_DT, tag="et2",
                                 name="et2")
        runs = []
        for jj, j in enumerate(js):
            pt = int(_PTYPE[ci * PPC + j])
            if runs and runs[-1][0] == pt:
                runs[-1][2] = jj + 1
            else:
                runs.append([pt, jj, jj + 1])
        for pt, j0, j1 in runs:
            e3 = cst["eb"][:, pt, :, :]
            eb_b = bass.AP(tensor=e3.tensor, offset=e3.offset,
                           ap=[e3.ap[0], e3.ap[1], [0, j1 - j0], e3.ap[2]])
            nc.vector.tensor_tensor(
                out=et2[:, :, j0:j1, :], in0=et[:, :, j0:j1, :],
                in1=eb_b, op=Alu.mult,
            )
        for jj, j in enumerate(js):
            psa = pools["ps_tok"].tile([VPP, NH, HD + 1], FP32, tag="tok",
                                       name="psa")
            for h in range(NH):
                nc.tensor.matmul(
                    psa[:, h, :], lhsT=et2[:, 2 * (h % 4) + h // 4, jj, :],
                    rhs=vp[:, j, h, :],
                    start=True, stop=True,
                )
            rec = pools["statp"].tile([VPP, NH], FP32, tag="rec", name="rec")
            nc.vector.tensor_scalar_max(out=rec[:], in0=psa[:, :, 0],
                                        scalar1=1e-30)
            nc.vector.reciprocal(rec[:], rec[:])
            rec_b = bass.AP(tensor=rec[:].tensor, offset=rec[:].offset,
                            ap=list(rec[:].ap) + [[0, HD]])
            nc.vector.tensor_mul(
                attn_t[0:VPP, j, :].rearrange("p (h d) -> p h d", h=NH),
                psa[:, :, 1:HD + 1], rec_b,
            )
    s["attn_t"] = attn_t


def _phase_proj(tc, ci, cst, pools, s):
    """attn transpose, proj + resid1, LN2, transpose -> xn2T (fp8)."""
    nc = tc.nc
    attnT = pools["attnT"].tile([128, 2, PPC, 128], BF16_DT, tag="attnT",
                                name="attnT")
    _transposes(nc, pools, s["attn_t"], attnT, nc.vector)

    x2_t = pools["x2p"].tile([128, PPC, C], BF16_DT, tag="x2", name="x2_t")
    for j in range(0, PPC, 2):
        ps = pools["ps_tok"].tile([128, 2, C], FP32, tag="tok", name="prps")
        for jj in range(2):
            for kt in range(2):
                nc.tensor.matmul(
                    ps[:, jj, :], lhsT=attnT[:, kt, j + jj, :],
                    rhs=cst["wp"][:, kt, :],
                    start=(kt == 0), stop=(kt == 1),
                )
            if cst["pbbc"] is not None:
                nc.vector.tensor_add(ps[:, jj, :], ps[:, jj, :], cst["pbbc"][:])
        nc.vector.scalar_tensor_tensor(
            out=x2_t[:, j:j + 2, :], in0=ps[:], scalar=1.0,
            in1=s["x_t"][:, j:j + 2, :], op0=Alu.mult, op1=Alu.add,
        )
    s["x2_t"] = x2_t

    xn2_t = pools["xnp"].tile([128, PPC, C], BF16_DT, tag="xn", name="xn2_t")
    _layernorm(nc, pools, x2_t, xn2_t)
    xn2T = pools["xn2T"].tile([128, 2, PPC, 128], FP8_DT, tag="xn2T", name="xn2T")
    _transposes(nc, pools, xn2_t, xn2T, nc.scalar)
    s["xn2T"] = xn2T


def _phase_mlp(tc, ci, ext, cst, pools, s):
    """MLP (fp8 DoubleRow) + resid2, store."""
    nc = tc.nc
    xn2T = s["xn2T"]
    hT = pools["hp"].tile([128, 8, PPC, 128], FP8_DT, tag="hT", name="hT")
    if ci < 2:
        nc.vector.memset(hT[:, :, :, VPP:128], 0.0)
    for s2 in range(2):
        jsl = slice(4 * s2, 4 * s2 + 4)
        for m in range(8):
            ps = pools["ps_wide"].tile([128, 4, 128], FP32, tag="wide",
                                       name="m1ps")
            nc.tensor.matmul(
                ps[:], lhsT=cst["w1"][:, :, 128 * m:128 * (m + 1)],
                rhs=xn2T[:, :, jsl, :],
                start=True, stop=True,
                perf_mode=mybir.MatmulPerfMode.DoubleRow,
            )
            bias = (cst["b1"][:, m:m + 1] if cst["b1"] is not None else 0.0)
            import os
            gelu_f = ActF.Identity if os.environ.get("GELU_ID") else ActF.Gelu
            nc.scalar.activation(
                hT[:, m, jsl, 0:VPP], ps[:, :, 0:VPP],
                gelu_f, bias=bias, scale=1.0 / W8SCALE,
            )

    out_t = pools["xp"].tile([128, PPC, C], FP32, tag="xo", name="out_t")
    for j in range(0, PPC, 2):
        ps = pools["ps_tok"].tile([128, 2, C], FP32, tag="tok", name="m2ps")
        for jj in range(2):
            for k2 in range(4):
                nc.tensor.matmul(
                    ps[:, jj, :], lhsT=hT[:, 2 * k2:2 * k2 + 2, j + jj, :],
                    rhs=cst["w2"][:, 2 * k2:2 * k2 + 2, :],
                    start=(k2 == 0), stop=(k2 == 3),
                    perf_mode=mybir.MatmulPerfMode.DoubleRow,
                )
            if cst["b2bc"] is not None:
                nc.vector.tensor_add(ps[:, jj, :], ps[:, jj, :], cst["b2bc"][:])
        nc.vector.scalar_tensor_tensor(
            out=out_t[:, j:j + 2, :], in0=ps[:], scalar=1.0 / W8SCALE,
            in1=s["x2_t"][:, j:j + 2, :], op0=Alu.mult, op1=Alu.add,
        )

    # compact store: pair (ci*PPC + j) valid rows 0:VPP
    dst = ext["out"][ci * VPC:, :]
    dst_ap = bass.AP(
        tensor=dst.tensor, offset=dst.offset,
        ap=[[C, VPP], [VPP * C, PPC], [1, C]],
    )
    nc.sync.dma_start(out=dst_ap, in_=out_t[0:VPP, :, :])


# --------------------------------------------------------------------------
# entry point
# --------------------------------------------------------------------------

_CACHE = {}


def _get_program(key_flags):
    if key_flags not in _CACHE:
        _CACHE[key_flags] = build_program(NPAIR, flags=dict(key_flags))
    return _CACHE[key_flags]


def kernel(**inputs):
    in_maps, extra = host_prep(inputs)
    nc = _get_program(tuple(sorted(extra.items())))
    res = run_bass_kernel_spmd(nc, in_maps, core_ids=list(range(B)))
    out = np.stack([res.results[i]["out"] for i in range(B)], axis=0)
    return out.reshape(B, NWIN, S, C).astype(np.float32)


# revision 55
# speedup vs baseline: 2.2643x; 1.0146x over previous
"""Swin-style windowed-attention block on 8 TRN2 NeuronCores (data-parallel over batch).

v2: compact-pair layout (both windows' 49 valid tokens at rows 0:98 of a 128-row
pair tile, zero pad rows 98:128). Pair-wide attention with post-exp multiplicative
bias (exp(S+b) = exp(S)*exp(b) with a host-precomputed exp(bias) table that also
zeroes cross-window blocks and pad rows), no augmented-K matmuls, no head-realign
or v-shift DMAs. Activation-table thrash removed (DVE Newton rsqrt; chunk-pair
interleaving batches exp/gelu). Elementwise spread across ACT / DVE / Pool.
"""

import sys

sys.path.insert(0, "/opt/trn_rl_repo")

import numpy as np
import ml_dtypes

import concourse.bass as bass
import concourse.bacc as bacc
import concourse.tile as tile
import concourse.mybir as mybir
from concourse.bass_utils import run_bass_kernel_spmd

BF16 = ml_dtypes.bfloat16
FP8 = ml_dtypes.float8_e4m3
FP32 = mybir.dt.float32
BF16_DT = mybir.dt.bfloat16
FP8_DT = mybir.dt.float8e4
INT32 = mybir.dt.int32
W8SCALE = 64.0

# ---- static geometry ----
WH, WW = 7, 7
S = 49                     # valid tokens per window
C = 256                    # channels
NH = 8                     # heads
HD = 32                    # head dim
NWIN = 256                 # windows per batch image
B = 8                      # batch == number of cores
GRID = 16                  # 16x16 window grid
SCALE = HD ** -0.5
EPS = 1e-5
MASK_VAL = -30000.0

NPAIR = NWIN // 2          # 128 window pairs per core
PPC = 8                    # pairs per chunk
NCHUNK = NPAIR // PPC      # 16 chunks
TPP = 128                  # tile rows per pair (98 valid + 30 zero pad)
VPP = 2 * S                # 98 valid tokens per pair
TPC = PPC * TPP            # 1024 padded tokens per chunk
VPC = PPC * VPP            # 784 valid tokens per chunk
NTOK = NWIN * S            # 12544 valid tokens per core
NTOKP = NPAIR * TPP        # 16384 padded tokens per core

RSQRT_MAGIC = 0x5F3759DF

ActF = mybir.ActivationFunctionType
Alu = mybir.AluOpType


# --------------------------------------------------------------------------
# host-side preparation
# --------------------------------------------------------------------------

def _relative_position_index():
    ch, cw = np.arange(WH), np.arange(WW)
    coords = np.stack(np.meshgrid(ch, cw, indexing="ij")).reshape(2, -1)
    rel = coords[:, :, None] - coords[:, None, :]
    rel = rel.transpose(1, 2, 0).astype(np.int64)
    rel[..., 0] += WH - 1
    rel[..., 1] += WW - 1
    rel[..., 0] *= 2 * WW - 1
    return rel.sum(-1)                                    # (S, S)


def _window_mask_types():
    """Per-window mask type: 0 none, 1 bottom-row, 2 right-col, 3 corner."""
    h = w = GRID
    s1, s2 = WH - WH // 2, WW - WW // 2
    m = np.zeros((h, w, WH, WW, WH, WW), dtype=bool)
    m[-1, :, :s1, :, s1:, :] = True
    m[-1, :, s1:, :, :s1, :] = True
    m[:, -1, :, :s2, :, s2:] = True
    m[:, -1, :, s2:, :, :s2] = True
    m = m.reshape(h * w, S, S)
    types = np.zeros(NWIN, dtype=np.int64)
    rr, cc = np.divmod(np.arange(NWIN), GRID)
    types[(rr == GRID - 1) & (cc < GRID - 1)] = 1
    types[(rr < GRID - 1) & (cc == GRID - 1)] = 2
    types[(rr == GRID - 1) & (cc == GRID - 1)] = 3
    masks = np.zeros((4, S, S), dtype=np.float32)
    masks[1] = np.where(m[GRID * (GRID - 1)], MASK_VAL, 0.0)
    masks[2] = np.where(m[GRID - 1], MASK_VAL, 0.0)
    masks[3] = np.where(m[NWIN - 1], MASK_VAL, 0.0)
    return types, masks


def _pair_types():
    types, _ = _window_mask_types()
    combos = []
    ptype = np.zeros(NPAIR, dtype=np.int64)
    for j in range(NPAIR):
        c = (int(types[2 * j]), int(types[2 * j + 1]))
        if c not in combos:
            combos.append(c)
        ptype[j] = combos.index(c)
    assert len(combos) <= 4, combos
    while len(combos) < 4:
        combos.append((0, 0))
    return ptype, combos


_PTYPE, _PCOMBOS = _pair_types()


def _tile_kxoc(wT):
    """[K, OC] -> [128, K//128, OC] with K = 128*kt + p."""
    K, OC = wT.shape
    return np.ascontiguousarray(wT.reshape(K // 128, 128, OC).transpose(1, 0, 2))


def host_prep(inputs):
    x = np.asarray(inputs["x"], dtype=np.float32)          # (B, N, S, C)
    qkv_w = np.asarray(inputs["qkv_w"], dtype=np.float32)
    qkv_b = np.asarray(inputs["qkv_b"], dtype=np.float32)
    proj_w = np.asarray(inputs["proj_w"], dtype=np.float32)
    proj_b = np.asarray(inputs["proj_b"], dtype=np.float32)
    n1g = np.asarray(inputs["norm1_g"], dtype=np.float32)
    n1b = np.asarray(inputs["norm1_b"], dtype=np.float32)
    n2g = np.asarray(inputs["norm2_g"], dtype=np.float32)
    n2b = np.asarray(inputs["norm2_b"], dtype=np.float32)
    w1 = np.asarray(inputs["mlp_w1"], dtype=np.float32)
    b1 = np.asarray(inputs["mlp_b1"], dtype=np.float32)
    w2 = np.asarray(inputs["mlp_w2"], dtype=np.float32)
    b2 = np.asarray(inputs["mlp_b2"], dtype=np.float32)
    table = np.asarray(inputs["bias_table"], dtype=np.float32)

    # fold layernorm affine into the following matmuls
    qkv_w_f = qkv_w * n1g[None, :]
    qkv_b_f = qkv_b + qkv_w @ n1b
    w1_f = w1 * n2g[None, :]
    b1_f = b1 + w1 @ n2b

    wq = qkv_w_f[0:C] * SCALE
    bq = qkv_b_f[0:C] * SCALE
    wk = qkv_w_f[C:2 * C]
    bk = qkv_b_f[C:2 * C]
    wv = qkv_w_f[2 * C:3 * C]
    bv = qkv_b_f[2 * C:3 * C]

    common = {
        "wq": _tile_kxoc(wq.T).astype(BF16),
        "wk": _tile_kxoc(wk.T).astype(BF16),
        "wv": _tile_kxoc(wv.T).astype(BF16),
        "wp": _tile_kxoc(proj_w.T).astype(BF16),
        "w1": _tile_kxoc(w1_f.T * W8SCALE).astype(FP8),
        "w2": _tile_kxoc(w2.T * W8SCALE).astype(FP8),
    }

    # exp(bias + mask) multiplicative table: eb[t_row, ptype, h, q_row]
    # t_row/q_row = 49*r + local; zero on cross-window blocks and pad rows.
    rel = _relative_position_index()
    bias_sht = table[rel].transpose(2, 0, 1)               # [h, s, t]
    _, masks = _window_mask_types()                        # [4, s, t]
    eb = np.zeros((TPP, 4, NH, VPP), dtype=np.float32)
    for pt, (tA, tB) in enumerate(_PCOMBOS):
        for r, wt in ((0, tA), (1, tB)):
            blk = np.exp(bias_sht + masks[wt][None])       # [h, s, t]
            eb[S * r:S * r + S, pt, :, S * r:S * r + S] = blk.transpose(2, 0, 1)
    # h-axis reordered to (h4, hh)-major to match the S-psum bank layout
    perm = [4 * hh + h4 for h4 in range(4) for hh in range(2)]
    common["eb"] = eb[:, :, perm, :].astype(BF16)

    extra = {
        "bq_nz": bool(np.any(bq != 0.0) or np.any(bk != 0.0)),
        "bv_nz": bool(np.any(bv != 0.0)),
        "pb_nz": bool(np.any(proj_b != 0.0)),
        "b1_nz": bool(np.any(b1_f != 0.0)),
        "b2_nz": bool(np.any(b2 != 0.0)),
    }
    if extra["bq_nz"]:
        common["bq"] = np.ascontiguousarray(bq.reshape(2, 128).T).astype(np.float32)
        common["bk"] = np.ascontiguousarray(bk.reshape(2, 128).T).astype(np.float32)
    if extra["b1_nz"]:
        common["b1"] = np.ascontiguousarray(b1_f.reshape(8, 128).T).astype(np.float32)
    if extra["bv_nz"]:
        common["bvbc"] = np.tile(bv[None, :], (128, 1)).astype(np.float32)
    if extra["pb_nz"]:
        common["pbbc"] = np.tile(proj_b[None, :], (128, 1)).astype(np.float32)
    if extra["b2_nz"]:
        common["b2bc"] = np.tile(b2[None, :], (128, 1)).astype(np.float32)

    in_maps = []
    for b in range(B):
        m = dict(common)
        xp = np.zeros((NPAIR, TPP, C), dtype=BF16)
        xp[:, :VPP, :] = x[b].reshape(NPAIR, VPP, C).astype(BF16)
        m["x"] = xp.reshape(NTOKP, C)
        in_maps.append(m)
    return in_maps, extra


# --------------------------------------------------------------------------
# kernel builder
# --------------------------------------------------------------------------

def build_program(n_pairs=NPAIR, flags=None):
    flags = flags or {}
    assert n_pairs % (2 * PPC) == 0
    n_chunks = n_pairs // PPC

    nc = bacc.Bacc("TRN2", target_bir_lowering=False, debug=False)

    ext = {}
    ext["x"] = nc.dram_tensor("x", [n_pairs * TPP, C], BF16_DT, kind="ExternalInput")
    ext["out"] = nc.dram_tensor("out", [n_pairs * VPP, C], FP32, kind="ExternalOutput")
    ext["wq"] = nc.dram_tensor("wq", [128, 2, C], BF16_DT, kind="ExternalInput")
    ext["wk"] = nc.dram_tensor("wk", [128, 2, C], BF16_DT, kind="ExternalInput")
    ext["wv"] = nc.dram_tensor("wv", [128, 2, C], BF16_DT, kind="ExternalInput")
    ext["wp"] = nc.dram_tensor("wp", [128, 2, C], BF16_DT, kind="ExternalInput")
    ext["w1"] = nc.dram_tensor("w1", [128, 2, 4 * C], FP8_DT, kind="ExternalInput")
    ext["w2"] = nc.dram_tensor("w2", [128, 8, C], FP8_DT, kind="ExternalInput")
    ext["eb"] = nc.dram_tensor("eb", [TPP, 4, NH, VPP], BF16_DT, kind="ExternalInput")
    for name, shape, flg in (
        ("bq", [128, 2], "bq_nz"), ("bk", [128, 2], "bq_nz"),
        ("b1", [128, 8], "b1_nz"),
        ("bvbc", [128, C], "bv_nz"), ("pbbc", [128, C], "pb_nz"),
        ("b2bc", [128, C], "b2_nz"),
    ):
        ext[name] = (nc.dram_tensor(name, shape, FP32, kind="ExternalInput")
                     if flags.get(flg) else None)

    with tile.TileContext(nc) as tc:
        _body(tc, n_chunks, ext)

    nc.compile()
    return nc


def _body(tc, n_chunks, ext):
    nc = tc.nc
    import contextlib
    with contextlib.ExitStack() as ctx:
        const = ctx.enter_context(tc.tile_pool(name="const", bufs=1))
        cst = {}
        for name, shape, dt in (
            ("wq", [128, 2, C], BF16_DT), ("wk", [128, 2, C], BF16_DT),
            ("wv", [128, 2, C], BF16_DT), ("wp", [128, 2, C], BF16_DT),
            ("w1", [128, 2, 4 * C], FP8_DT), ("w2", [128, 8, C], FP8_DT),
            ("eb", [TPP, 4, NH, VPP], BF16_DT),
            ("bq", [128, 2], FP32), ("bk", [128, 2], FP32),
            ("b1", [128, 8], FP32),
            ("bvbc", [128, C], FP32), ("pbbc", [128, C], FP32),
            ("b2bc", [128, C], FP32),
        ):
            if ext.get(name) is None:
                cst[name] = None
                continue
            t = const.tile(shape, dt, tag=name, name=name)
            nc.sync.dma_start(out=t[:], in_=ext[name].ap())
            cst[name] = t
        ident = const.tile([128, 128], BF16_DT, tag="ident", name="ident")
        from concourse.masks import make_identity
        make_identity(nc, ident[:])
        cst["ident"] = ident

        pools = {}
        for name, bufs in (("xp", 2), ("xbp", 2), ("xnp", 2), ("xnT", 2),
                           ("attnT", 2), ("xn2T", 4), ("qkp", 2), ("qk3p", 2),
                           ("vsp", 2), ("etp", 2), ("et2p", 2), ("atp", 2),
                           ("x2p", 4), ("hp", 2), ("statp", 2)):
            pools[name] = ctx.enter_context(tc.tile_pool(name=name, bufs=bufs))
        # PSUM budget (8 banks): ps_S 4 (one bank per concurrent PE row-tile),
        # ps_wide 2, ps_tok 2 (shared ring: v/psa/proj/mlp2/transpose drains)
        for name, bufs in (("ps_wide", 2), ("ps_tok", 2), ("ps_S", 1)):
            pools[name] = ctx.enter_context(
                tc.tile_pool(name=name, bufs=bufs, space="PSUM"))
        pools["ps_tr"] = pools["ps_wide"]
        pools["_ident"] = cst["ident"]

        import os
        n_phases = int(os.environ.get("PHASES", "4"))

        def _store_dbg(ci, t):
            dst = ext["out"][ci * VPC:, :]
            dst_ap = bass.AP(tensor=dst.tensor, offset=dst.offset,
                             ap=[[C, VPP], [VPP * C, PPC], [1, C]])
            nc.sync.dma_start(out=dst_ap, in_=t[0:VPP, :, :])

        # software pipeline: the MLP of pair-group N is emitted after the
        # in/attn/proj of pair-group N+1, so the PE always has matmul work
        # while the DVE runs the next group's LN chains.
        st = {}
        prev = None
        for cp in range(n_chunks // 2):
            a, b = 2 * cp, 2 * cp + 1
            st[a] = _phase_in(tc, a, ext, cst, pools)
            st[b] = _phase_in(tc, b, ext, cst, pools)
            if n_phases < 2:
                _store_dbg(a, st[a]["x_t"])
                _store_dbg(b, st[b]["x_t"])
                del st[a], st[b]
                continue
            _phase_attn(tc, a, cst, pools, st[a])
            _phase_attn(tc, b, cst, pools, st[b])
            if n_phases < 3:
                _store_dbg(a, st[a]["x_t"])
                _store_dbg(b, st[b]["x_t"])
                del st[a], st[b]
                continue
            _phase_proj(tc, a, cst, pools, st[a])
            _phase_proj(tc, b, cst, pools, st[b])
            if n_phases < 4:
                _store_dbg(a, st[a]["x2_t"])
                _store_dbg(b, st[b]["x2_t"])
                del st[a], st[b]
                continue
            if prev is not None:
                for p in prev:
                    _phase_mlp(tc, p, ext, cst, pools, st[p])
                    del st[p]
            prev = (a, b)
        if n_phases >= 4 and prev is not None:
            for p in prev:
                _phase_mlp(tc, p, ext, cst, pools, st[p])
                del st[p]


def _layernorm(nc, pools, x_t, xn_t):
    """x_t [128, PPC, 256] bf16 -> xn_t bf16 ((x-mu)*rstd).

    Batched stats on DVE (bn_stats 2 pairs/instr), rstd via quake-rsqrt +
    2 Newton steps on DVE (avoids the Sqrt activation-table load), apply on
    DVE (2x/4x with bf16 operands)."""
    statp = pools["statp"]
    mv = statp.tile([128, PPC, 2], FP32, tag="mv", name="mv")
    for j in range(PPC):
        bnst = statp.tile([128, 6], FP32, tag="bnst", name="bnst")
        nc.vector.bn_stats(bnst[:], x_t[:, j, :])
        nc.vector.bn_aggr(mv[:, j, :], bnst[:])
    var = statp.tile([128, PPC], FP32, tag="var", name="var")
    rst = statp.tile([128, PPC], FP32, tag="rst", name="rst")
    tmp = statp.tile([128, PPC], FP32, tag="tmp", name="tmp")
    nc.vector.tensor_scalar(out=var[:], in0=mv[:, :, 1], scalar1=EPS,
                            scalar2=None, op0=Alu.add)
    nc.vector.tensor_scalar(out=rst[:].bitcast(INT32), in0=var[:].bitcast(INT32),
                            scalar1=1, scalar2=None, op0=Alu.logical_shift_right)
    nc.vector.tensor_scalar(out=rst[:].bitcast(INT32), in0=rst[:].bitcast(INT32),
                            scalar1=-1, scalar2=RSQRT_MAGIC,
                            op0=Alu.mult, op1=Alu.add)
    for _ in range(1):
        nc.vector.tensor_tensor(out=tmp[:], in0=rst[:], in1=rst[:], op=Alu.mult)
        nc.vector.tensor_tensor(out=tmp[:], in0=tmp[:], in1=var[:], op=Alu.mult)
        nc.vector.tensor_scalar(out=tmp[:], in0=tmp[:], scalar1=-0.5,
                                scalar2=1.5, op0=Alu.mult, op1=Alu.add)
        nc.vector.tensor_tensor(out=rst[:], in0=rst[:], in1=tmp[:], op=Alu.mult)
    for j in range(PPC):
        nc.vector.tensor_scalar(
            out=xn_t[:, j, :], in0=x_t[:, j, :],
            scalar1=mv[:, j, 0:1], scalar2=rst[:, j:j + 1],
            op0=Alu.subtract, op1=Alu.mult,
        )


def _transposes(nc, pools, src_t, dst_T, drain_eng):
    """src_t [128, PPC, 256] -> dst_T [128, 2, PPC, 128] via PE + drain.

    Two pairs share one psum tile and one drain."""
    ident = pools["_ident"]
    for j in range(0, PPC, 2):
        ps = pools["ps_tr"].tile([128, 2, 2, 128], BF16_DT, tag="wide",
                                 name="trps")
        for jj in range(2):
            for ch in range(2):
                nc.tensor.transpose(
                    ps[:, jj, ch, :],
                    src_t[:, j + jj, 128 * ch:128 * (ch + 1)], ident[:, :])
        src = ps[:].rearrange("p a b t -> p b a t")
        if hasattr(drain_eng, "tensor_copy"):
            drain_eng.tensor_copy(out=dst_T[:, :, j:j + 2, :], in_=src)
        else:
            drain_eng.copy(out=dst_T[:, :, j:j + 2, :], in_=src)


def _phase_in(tc, ci, ext, cst, pools):
    """Load x, LN1, transpose, QKV -> qT/kT/vp."""
    nc = tc.nc
    s = {}

    x_t = pools["xbp"].tile([128, PPC, C], BF16_DT, tag="xb", name="x_t")
    nc.sync.dma_start(
        out=x_t[:],
        in_=ext["x"][ci * TPC:(ci + 1) * TPC, :].rearrange("(j p) c -> p j c", p=TPP),
    )
    s["x_t"] = x_t

    xn_t = pools["xnp"].tile([128, PPC, C], BF16_DT, tag="xn", name="xn_t")
    _layernorm(nc, pools, x_t, xn_t)
    xnT = pools["xnT"].tile([128, 2, PPC, 128], BF16_DT, tag="xnT", name="xnT")
    _transposes(nc, pools, xn_t, xnT, nc.vector)

    # q, k: channel-major slabs; drain on ACT (identity/copy, bias optional)
    qT = pools["qkp"].tile([128, 2, PPC, 128], BF16_DT, tag="qT", name="qT")
    kT = pools["qkp"].tile([128, 2, PPC, 128], BF16_DT, tag="kT", name="kT")
    for s2 in range(2):
        jsl = slice(4 * s2, 4 * s2 + 4)
        for (dstT, wname, bname) in ((qT, "wq", "bq"), (kT, "wk", "bk")):
            w_sb = cst[wname]
            for octl in range(2):
                ps = pools["ps_wide"].tile([128, 4, 128], FP32, tag="wide",
                                           name="qkps")
                for kt in range(2):
                    nc.tensor.matmul(
                        ps[:],
                        lhsT=w_sb[:, kt, 128 * octl:128 * (octl + 1)],
                        rhs=xnT[:, kt, jsl, :],
                        start=(kt == 0), stop=(kt == 1),
                    )
                if cst[bname] is not None:
                    nc.scalar.activation(
                        dstT[:, octl, jsl, :], ps[:], ActF.Identity,
                        bias=cst[bname][:, octl:octl + 1])
                else:
                    nc.scalar.activation(dstT[:, octl, jsl, :], ps[:], ActF.Copy)
    s["qT"], s["kT"] = qT, kT

    # PE matmul row-tile base 96 is unsupported; realign the hm==3 head rows
    # (h = 3, 7) of q/k to partition base 0 via one small DMA each.
    qk3 = pools["qk3p"].tile([32, 2, 2, PPC, 128], BF16_DT, tag="qk3", name="qk3")
    nc.sync.dma_start(out=qk3[0:32, 0], in_=qT[96:128, :, :, :])
    nc.sync.dma_start(out=qk3[0:32, 1], in_=kT[96:128, :, :, :])
    s["qk3"] = qk3

    # v: token-major, drain straight into pair-local vp (no base shift);
    # two pairs share one psum tile and one drain
    vp = pools["vsp"].tile([128, PPC, NH, HD + 1], BF16_DT, tag="vp", name="vp")
    for j in range(0, PPC, 2):
        ps = pools["ps_tok"].tile([128, 2, C], FP32, tag="tok", name="vps")
        for jj in range(2):
            for kt in range(2):
                nc.tensor.matmul(
                    ps[:, jj, :], lhsT=xnT[:, kt, j + jj, :],
                    rhs=cst["wv"][:, kt, :],
                    start=(kt == 0), stop=(kt == 1),
                )
        nc.vector.memset(vp[:, j:j + 2, :, 0:1], 1.0)
        if cst["bvbc"] is not None:
            for jj in range(2):
                nc.vector.tensor_add(ps[:, jj, :], ps[:, jj, :], cst["bvbc"][:])
        nc.scalar.copy(
            out=vp[:, j:j + 2, :, 1:HD + 1],
            in_=ps[:].rearrange("p a (h d) -> p a h d", h=NH),
        )
    s["vp"] = vp
    return s


def _phase_attn(tc, ci, cst, pools, s):
    """S = K^T Q pair-wide, exp, *exp(bias), A = et2 @ [1|v]."""
    nc = tc.nc
    qT, kT, vp = s["qT"], s["kT"], s["vp"]
    attn_t = pools["atp"].tile([128, PPC, C], BF16_DT, tag="attn", name="attn_t")
    if ci < 2:
        # first use of each ring buffer: seed pad rows (never valid-read,
        # but must be finite/initialized for the pair transposes). 96-aligned
        # partition start; rows 96:98 are re-written by the attn drain below.
        nc.vector.memset(attn_t[96:128, :, :], 0.0)
    qk3 = s["qk3"]
    JB = 2                                      # pairs per exp batch
    for jg in range(PPC // JB):
        js = list(range(JB * jg, JB * jg + JB))
        # et holds the batch, h-axis (h4, hh)-major to match the psum layout
        et = pools["etp"].tile([128, NH, JB, VPP], BF16_DT, tag="et", name="et")
        # all 8 heads x JB pairs in one psum tile: [h4(bank), hh, jj, col];
        # h4-stride = 2KB so each concurrent PE row-tile owns its own bank
        # (same-bank writes h and h+4 run on the same row-tile => serialized).
        pss = pools["ps_S"].tile([128, 4, 2 * JB, 128], FP32, tag="S", name="pss")
        for jj, j in enumerate(js):
            for h in range(NH):
                hh, hm = h // 4, h % 4
                octl = h // 4
                if hm == 3:
                    lhsT = qk3[0:32, 1, octl, j, :]
                    rhs = qk3[0:32, 0, octl, j, 0:VPP]
                    base = 0
                else:
                    base = 32 * hm
                    lhsT = kT[base:base + 32, octl, j, :]
                    rhs = qT[base:base + 32, octl, j, 0:VPP]
                nc.tensor.matmul(
                    pss[:, hm, JB * hh + jj, 0:VPP], lhsT=lhsT, rhs=rhs,
                    start=True, stop=True,
                    tile_position=(base, 0),
                )
        nc.scalar.activation(
            et[:].rearrange("p (a b) j q -> p a (b j) q", a=4),
            pss[:, :, :, 0:VPP], ActF.Exp)
        # eb multiply, batched over runs of equal pair-type
        et2 = pools["et2p"].tile([128, NH, JB, VPP], BF16_DT, tag="et2",
                                 name="et2")
        runs = []
        for jj, j in enumerate(js):
            pt = int(_PTYPE[ci * PPC + j])
            if runs and runs[-1][0] == pt:
                runs[-1][2] = jj + 1
            else:
                runs.append([pt, jj, jj + 1])
        for pt, j0, j1 in runs:
            e3 = cst["eb"][:, pt, :, :]
            eb_b = bass.AP(tensor=e3.tensor, offset=e3.offset,
                           ap=[e3.ap[0], e3.ap[1], [0, j1 - j0], e3.ap[2]])
            nc.vector.tensor_tensor(
                out=et2[:, :, j0:j1, :], in0=et[:, :, j0:j1, :],
                in1=eb_b, op=Alu.mult,
            )
        for jj, j in enumerate(js):
            psa = pools["ps_tok"].tile([VPP, NH, HD + 1], FP32, tag="tok",
                                       name="psa")
            for h in range(NH):
                nc.tensor.matmul(
                    psa[:, h, :], lhsT=et2[:, 2 * (h % 4) + h // 4, jj, :],
                    rhs=vp[:, j, h, :],
                    start=True, stop=True,
                )
            rec = pools["statp"].tile([VPP, NH], FP32, tag="rec", name="rec")
            nc.vector.reciprocal(rec[:], psa[:, :, 0])
            rec_b = bass.AP(tensor=rec[:].tensor, offset=rec[:].offset,
                            ap=list(rec[:].ap) + [[0, HD]])
            nc.vector.tensor_mul(
                attn_t[0:VPP, j, :].rearrange("p (h d) -> p h d", h=NH),
                psa[:, :, 1:HD + 1], rec_b,
            )
    s["attn_t"] = attn_t


def _phase_proj(tc, ci, cst, pools, s):
    """attn transpose, proj + resid1, LN2, transpose -> xn2T (fp8)."""
    nc = tc.nc
    attnT = pools["attnT"].tile([128, 2, PPC, 128], BF16_DT, tag="attnT",
                                name="attnT")
    _transposes(nc, pools, s["attn_t"], attnT, nc.vector)

    x2_t = pools["x2p"].tile([128, PPC, C], BF16_DT, tag="x2", name="x2_t")
    for j in range(0, PPC, 2):
        ps = pools["ps_tok"].tile([128, 2, C], FP32, tag="tok", name="prps")
        for jj in range(2):
            for kt in range(2):
                nc.tensor.matmul(
                    ps[:, jj, :], lhsT=attnT[:, kt, j + jj, :],
                    rhs=cst["wp"][:, kt, :],
                    start=(kt == 0), stop=(kt == 1),
                )
            if cst["pbbc"] is not None:
                nc.vector.tensor_add(ps[:, jj, :], ps[:, jj, :], cst["pbbc"][:])
        nc.vector.scalar_tensor_tensor(
            out=x2_t[:, j:j + 2, :], in0=ps[:], scalar=1.0,
            in1=s["x_t"][:, j:j + 2, :], op0=Alu.mult, op1=Alu.add,
        )
    s["x2_t"] = x2_t

    xn2_t = pools["xnp"].tile([128, PPC, C], BF16_DT, tag="xn", name="xn2_t")
    _layernorm(nc, pools, x2_t, xn2_t)
    xn2T = pools["xn2T"].tile([128, 2, PPC, 128], FP8_DT, tag="xn2T", name="xn2T")
    _transposes(nc, pools, xn2_t, xn2T, nc.scalar)
    s["xn2T"] = xn2T


def _phase_mlp(tc, ci, ext, cst, pools, s):
    """MLP (fp8 DoubleRow) + resid2, store."""
    nc = tc.nc
    xn2T = s["xn2T"]
    hT = pools["hp"].tile([128, 8, PPC, 128], FP8_DT, tag="hT", name="hT")
    if ci < 2:
        nc.vector.memset(hT[:, :, :, VPP:128], 0.0)
    for s2 in range(2):
        jsl = slice(4 * s2, 4 * s2 + 4)
        for m in range(8):
            ps = pools["ps_wide"].tile([128, 4, 128], FP32, tag="wide",
                                       name="m1ps")
            nc.tensor.matmul(
                ps[:], lhsT=cst["w1"][:, :, 128 * m:128 * (m + 1)],
                rhs=xn2T[:, :, jsl, :],
                start=True, stop=True,
                perf_mode=mybir.MatmulPerfMode.DoubleRow,
            )
            bias = (cst["b1"][:, m:m + 1] if cst["b1"] is not None else 0.0)
            import os
            gelu_f = ActF.Identity if os.environ.get("GELU_ID") else ActF.Gelu
            nc.scalar.activation(
                hT[:, m, jsl, 0:VPP], ps[:, :, 0:VPP],
                gelu_f, bias=bias, scale=1.0 / W8SCALE,
            )

    out_t = pools["xp"].tile([128, PPC, C], FP32, tag="xo", name="out_t")
    for j in range(0, PPC, 2):
        ps = pools["ps_tok"].tile([128, 2, C], FP32, tag="tok", name="m2ps")
        for jj in range(2):
            for k2 in range(4):
                nc.tensor.matmul(
                    ps[:, jj, :], lhsT=hT[:, 2 * k2:2 * k2 + 2, j + jj, :],
                    rhs=cst["w2"][:, 2 * k2:2 * k2 + 2, :],
                    start=(k2 == 0), stop=(k2 == 3),
                    perf_mode=mybir.MatmulPerfMode.DoubleRow,
                )
            if cst["b2bc"] is not None:
                nc.vector.tensor_add(ps[:, jj, :], ps[:, jj, :], cst["b2bc"][:])
        nc.vector.scalar_tensor_tensor(
            out=out_t[:, j:j + 2, :], in0=ps[:], scalar=1.0 / W8SCALE,
            in1=s["x2_t"][:, j:j + 2, :], op0=Alu.mult, op1=Alu.add,
        )

    # compact store: pair (ci*PPC + j) valid rows 0:VPP
    dst = ext["out"][ci * VPC:, :]
    dst_ap = bass.AP(
        tensor=dst.tensor, offset=dst.offset,
        ap=[[C, VPP], [VPP * C, PPC], [1, C]],
    )
    nc.sync.dma_start(out=dst_ap, in_=out_t[0:VPP, :, :])


# --------------------------------------------------------------------------
# entry point
# --------------------------------------------------------------------------

_CACHE = {}


def _get_program(key_flags):
    if key_flags not in _CACHE:
        _CACHE[key_flags] = build_program(NPAIR, flags=dict(key_flags))
    return _CACHE[key_flags]


def kernel(**inputs):
    in_maps, extra = host_prep(inputs)
    nc = _get_program(tuple(sorted(extra.items())))
    res = run_bass_kernel_spmd(nc, in_maps, core_ids=list(range(B)))
    out = np.stack([res.results[i]["out"] for i in range(B)], axis=0)
    return out.reshape(B, NWIN, S, C).astype(np.float32)
